# revision 6
# baseline (speedup 1.0000x reference)
"""BatchGGNNEncoder Trainium2 kernel: 8-core SPMD, dst-sharded message passing.

Full inputs in, full output out. Internally:
  - core c owns nodes [c*4096, (c+1)*4096) = graphs [4c, 4c+4) (data parallel).
  - aggregate-first GGNN layer:
        A_t[v] = sum_{e: dst=v, type=t} h[src_e]         (one-hot matmuls, PSUM)
        m      = sum_t A_t @ Wm[t].T + counts_t * bm[t]  (dense matmuls)
        h      = GRU(m, h)                               (matmuls + DVE/ACT)
  - h table (bf16, node-major) lives in DRAM; rebuilt per layer via TWO
    AllGathers (first half fires mid-layer to hide latency); per-edge h[src]
    rows fetched with dma_gather (the Q7 descriptor-emission stream is the
    critical path: ~8.5us per 1024 rows, so everything else hides under it).
  - nodes are permuted within each graph to balance (type, 128-dst-window)
    group sizes so the compiled program structure is identical on all 8 cores.
"""
import numpy as np
import ml_dtypes

import concourse.bass as bass
import concourse.bacc as bacc
import concourse.mybir as mybir
import concourse.tile as tile
from concourse.bass_utils import run_bass_kernel_spmd

BF16 = ml_dtypes.bfloat16

# problem constants (hardcoded per harness contract)
MAXN, F, H, T, L = 1024, 215, 256, 8, 3
NCORES = 8
WIN = 128                     # dst window (one-hot free width)
WPG = MAXN // WIN             # 8 windows per graph
GPREF = 5                     # gather groups in flight


def _balance_graph(deg):
    """Assign 1024 nodes (deg: [1024, T] type-degrees) to 8 windows of 128.
    Heavy nodes go to window 7 (cap 384/type = 3 chunks); the rest fill
    windows 0..5 under a hard 256/type cap (2 chunks); overflow goes to
    window 6 (cap 384). Keeping the 3-chunk windows at the same positions
    across all graphs aligns the cross-core max that sets the budget."""
    tot = deg.sum(1)
    order = np.argsort(-tot, kind="stable")
    wsum = np.zeros((WPG, T), np.float64)
    wcnt = np.zeros(WPG, np.int64)
    members = [[] for _ in range(WPG)]
    CAP, CAP6, CAP7 = 256.0, 384.0, 384.0
    rest = []
    for nd in order:
        if wcnt[7] < 128 and ((wsum[7] + deg[nd]) <= CAP7).all():
            members[7].append(nd)
            wsum[7] += deg[nd]
            wcnt[7] += 1
        else:
            rest.append(nd)
    for nd in rest:
        d = deg[nd]
        ns = wsum[:6] + d
        feas = (wcnt[:6] < 128) & (ns <= CAP).all(axis=1)
        if feas.any():
            load = np.where(feas, ns.max(axis=1), np.inf)
            best = int(np.argmin(load))
        elif wcnt[6] < 128 and ((wsum[6] + d) <= CAP6).all():
            best = 6
        else:
            nsall = wsum + d
            dcost = (np.ceil(nsall / 128) - np.ceil(wsum / 128)).sum(axis=1)
            dcost[wcnt >= 128] = np.inf
            best = int(np.argmin(dcost))
        members[best].append(nd)
        wsum[best] += d
        wcnt[best] += 1
    return [np.array(m, np.int64) for m in members]


def _prep(node_features, edge_index, edge_type, Wp, bp, Wm, bm, Wih, Whh, bih, bhh):
    """Host-side sharding/packing. Returns (meta, in_maps)."""
    x = np.asarray(node_features, np.float32)
    B = x.shape[0]
    N = B * MAXN
    GPC = B // NCORES             # graphs per core
    NB = GPC * MAXN               # nodes per core
    NWIN = GPC * WPG              # windows per core
    HALF = NB // 2                # nodes per AG half (2 graphs)
    src = np.asarray(edge_index[0]).astype(np.int64)
    dst = np.asarray(edge_index[1]).astype(np.int64)
    et = np.asarray(edge_type).astype(np.int64)

    # per-(node, type) in-degree
    cnt = np.zeros((N, T), np.int64)
    np.add.at(cnt, (dst, et), 1)

    # balance windows within each graph -> node permutation
    old2new = np.empty(N, np.int64)
    for g in range(B):
        mem = _balance_graph(cnt[g * MAXN:(g + 1) * MAXN])
        for w in range(WPG):
            pos = g * MAXN + w * WIN + np.arange(WIN)
            old2new[g * MAXN + mem[w]] = pos
    new2old = np.argsort(old2new)

    src_n = old2new[src]
    dst_n = old2new[dst]

    src_row = src_n                # table row = global node id (single AG)

    # group edges per core: key = ((gslot*WPG + w)*T + t)
    core = dst_n // NB
    rel = dst_n % NB
    col = rel % WIN
    key = (rel // WIN) * T + et
    NGRP = NWIN * T

    gsizes = np.zeros((NCORES, NGRP), np.int64)
    for c in range(NCORES):
        m = core == c
        gsizes[c] = np.bincount(key[m], minlength=NGRP)
    budget = np.ceil(gsizes.max(axis=0) / 128).astype(np.int64)  # chunks/group
    budget = np.maximum(budget, 1)
    ctot = int(budget.sum())
    ngg = (ctot + 7) // 8          # gather groups of <=8 chunks
    rem_last = ctot - 8 * (ngg - 1)
    nslots = ctot * 128
    gbase = np.concatenate([[0], np.cumsum(budget)])[:-1] * 128  # slot base

    # per-core slot arrays
    idx_maps, smat_maps = [], []
    counts_maps, xT_maps = [], []
    for c in range(NCORES):
        m = core == c
        kc, cc, sc = key[m], col[m], src_row[m]
        order = np.argsort(kc, kind="stable")
        kc, cc, sc = kc[order], cc[order], sc[order]
        grp_start = np.searchsorted(kc, np.arange(NGRP), side="left")
        rank = np.arange(kc.size) - grp_start[kc]
        slot = gbase[kc] + rank
        src16 = np.zeros(nslots, np.int16)
        scol = np.full(nslots, -1, np.int64)
        src16[slot] = sc.astype(np.int16)
        scol[slot] = cc
        # idx: wrapped [16, nslots/16] replicated to 128 partitions
        idx = np.tile(src16.reshape(nslots // 16, 16).T, (8, 1)).copy()
        idx_maps.append(idx)
        # one-hot S: [ngg, 128, 8, WIN] bf16 (last group zero-padded)
        smat = np.zeros((ngg * 8 * 128, WIN), BF16)
        valid = scol >= 0
        smat[np.nonzero(valid)[0], scol[valid]] = 1
        smat = smat.reshape(ngg, 8, 128, WIN)
        smat = np.ascontiguousarray(smat.transpose(0, 2, 1, 3))  # [ngg,128,8,WIN]
        smat_maps.append(smat)
        # counts (new order), [T, NB] bf16
        cslice = cnt[new2old[c * NB:(c + 1) * NB]]
        counts_maps.append(np.ascontiguousarray(cslice.T).astype(BF16))
        # xT [128, 2, NB] bf16: [p, k, node] = x[node, k*128+p]
        xs = x.reshape(N, F)[new2old[c * NB:(c + 1) * NB]]
        xp = np.zeros((NB, 2 * 128), np.float32)
        xp[:, :F] = xs
        xT = np.ascontiguousarray(xp.reshape(NB, 2, 128).transpose(2, 1, 0))
        xT_maps.append(xT.astype(BF16))

    # weights (shared across cores)
    Wp = np.asarray(Wp, np.float32); bp_ = np.asarray(bp, np.float32)
    Wm_ = np.asarray(Wm, np.float32); bm_ = np.asarray(bm, np.float32)
    Wih_ = np.asarray(Wih, np.float32); Whh_ = np.asarray(Whh, np.float32)
    bih_ = np.asarray(bih, np.float32); bhh_ = np.asarray(bhh, np.float32)

    wpT = np.zeros((128, 2, H), np.float32)          # [p, fk, h']
    wpt = Wp.T                                       # [F, H]
    wpT[:, 0, :] = wpt[0:128]
    wpT[:F - 128, 1, :] = wpt[128:F]
    wp_in = wpT.astype(BF16)
    bp_in = np.ascontiguousarray(bp_.reshape(2, 128).T)          # [128, 2]

    wm_in = np.ascontiguousarray(                     # [L, 128, 2, T, H]
        Wm_.transpose(0, 1, 3, 2)
        .reshape(L, T, 2, 128, H).transpose(0, 3, 2, 1, 4)).astype(BF16)
    bm_in = bm_.astype(BF16)                          # [L, T, H]
    wih_in = np.ascontiguousarray(                    # [L, 128, 2, 3H]
        Wih_.transpose(0, 2, 1).reshape(L, 2, 128, 3 * H).transpose(0, 2, 1, 3)
    ).astype(BF16)
    whh_in = np.ascontiguousarray(
        Whh_.transpose(0, 2, 1).reshape(L, 2, 128, 3 * H).transpose(0, 2, 1, 3)
    ).astype(BF16)
    brz = bih_[:, :2 * H] + bhh_[:, :2 * H]
    brz_in = np.ascontiguousarray(brz.reshape(L, 4, 128).transpose(0, 2, 1))
    bin_in = np.ascontiguousarray(bih_[:, 2 * H:].reshape(L, 2, 128).transpose(0, 2, 1))
    bhn_in = np.ascontiguousarray(bhh_[:, 2 * H:].reshape(L, 2, 128).transpose(0, 2, 1))
    id128 = np.eye(128, dtype=BF16)

    in_maps = []
    for c in range(NCORES):
        in_maps.append({
            "xT": xT_maps[c], "idx": idx_maps[c], "smat": smat_maps[c],
            "countsT": counts_maps[c],
            "wpT": wp_in, "bp": bp_in, "wmT": wm_in, "bmT": bm_in,
            "wihT": wih_in, "whhT": whh_in,
            "brz": brz_in, "bin_": bin_in, "bhn": bhn_in, "id128": id128,
        })
    meta = dict(B=B, N=N, GPC=GPC, NB=NB, NWIN=NWIN, HALF=HALF,
                budget=budget.reshape(NWIN, T), ctot=ctot, ngg=ngg,
                rem_last=rem_last, new2old=new2old)
    return meta, in_maps


def _build(meta):
    """Build the SPMD Bass program (identical across cores)."""
    dt = mybir.dt
    N, NB, GPC, NWIN = meta["N"], meta["NB"], meta["GPC"], meta["NWIN"]
    HALF = meta["HALF"]
    budget, ngg, ctot = meta["budget"], meta["ngg"], meta["ctot"]
    rem_last = meta["rem_last"]
    SLOT16 = ctot * 8

    nc = bacc.Bacc("TRN2", target_bir_lowering=False, debug=False,
                   enable_asserts=False, num_devices=NCORES)

    # ---- I/O
    xT_in = nc.dram_tensor("xT", [128, 2, NB], dt.bfloat16, kind="ExternalInput").ap()
    idx_in = nc.dram_tensor("idx", [128, SLOT16], dt.int16, kind="ExternalInput").ap()
    smat_in = nc.dram_tensor("smat", [ngg, 128, 8, WIN], dt.bfloat16, kind="ExternalInput").ap()
    counts_in = nc.dram_tensor("countsT", [T, NB], dt.bfloat16, kind="ExternalInput").ap()
    wp_in = nc.dram_tensor("wpT", [128, 2, H], dt.bfloat16, kind="ExternalInput").ap()
    bp_in = nc.dram_tensor("bp", [128, 2], dt.float32, kind="ExternalInput").ap()
    wm_in = nc.dram_tensor("wmT", [L, 128, 2, T, H], dt.bfloat16, kind="ExternalInput").ap()
    bm_in = nc.dram_tensor("bmT", [L, T, H], dt.bfloat16, kind="ExternalInput").ap()
    wih_in = nc.dram_tensor("wihT", [L, 128, 2, 3 * H], dt.bfloat16, kind="ExternalInput").ap()
    whh_in = nc.dram_tensor("whhT", [L, 128, 2, 3 * H], dt.bfloat16, kind="ExternalInput").ap()
    brz_in = nc.dram_tensor("brz", [L, 128, 4], dt.float32, kind="ExternalInput").ap()
    bin_in = nc.dram_tensor("bin_", [L, 128, 2], dt.float32, kind="ExternalInput").ap()
    bhn_in = nc.dram_tensor("bhn", [L, 128, 2], dt.float32, kind="ExternalInput").ap()
    id_in = nc.dram_tensor("id128", [128, 128], dt.bfloat16, kind="ExternalInput").ap()
    out_t = nc.dram_tensor("outT", [2, 128, GPC], dt.float32, kind="ExternalOutput").ap()

    groups = [list(range(NCORES))]

    with tile.TileContext(nc) as tc:
        with (
            tc.tile_pool(name="per", bufs=1) as per,       # persistent SBUF
            tc.tile_pool(name="wts", bufs=2) as wts,       # per-layer weights
            tc.tile_pool(name="gth", bufs=GPREF) as gth,   # gather/S stream
            tc.tile_pool(name="wrk", bufs=2) as wrk,       # A/mT/staging
            tc.tile_pool(name="gru", bufs=2) as grup,      # GRU temps
            tc.tile_pool(name="ps", bufs=1, space="PSUM") as ps,
            tc.tile_pool(name="dram", bufs=2, space="DRAM") as dram,
        ):
            # persistent loads
            idx_sb = per.tile([128, SLOT16], dt.int16)
            nc.sync.dma_start(idx_sb[:], idx_in[:])
            counts_sb = per.tile([T, NB], dt.bfloat16)
            nc.sync.dma_start(counts_sb[:], counts_in[:])
            wp_sb = per.tile([128, 2, H], dt.bfloat16)
            nc.sync.dma_start(wp_sb[:], wp_in[:])
            bp_sb = per.tile([128, 2], dt.float32)
            nc.sync.dma_start(bp_sb[:], bp_in[:])
            id_sb = per.tile([128, 128], dt.bfloat16)
            nc.sync.dma_start(id_sb[:], id_in[:])
            hT_sb = per.tile([128, 2, NB], dt.bfloat16)
            outsb = per.tile([128, 2, GPC], dt.float32)
            nc.vector.memset(outsb[:], 0.0)

            # per-layer table + AG staging rings (DRAM)
            tbls, agins = [], []
            for l in range(L):
                tbls.append(dram.tile([N, H], dt.bfloat16, tag="tbl", bufs=2,
                                      addr_space="Shared", name=f"tbl{l}"))
                agins.append(dram.tile([NB, H], dt.bfloat16, tag="agin",
                                       bufs=2, name=f"agin{l}"))

            def stage_graph(l, q):
                """PE-transpose graph q's h (h-major) to node-major, DMA to
                agin, and fire the half-AllGathers for layer l's table."""
                agin, tbl = agins[l], tbls[l]
                stg = wrk.tile([128, WPG, H], dt.bfloat16, tag="stg", bufs=2)
                for wl in range(WPG):
                    nb = q * MAXN + wl * WIN
                    for hc in range(2):
                        tp = ps.tile([128, 128], dt.bfloat16, tag="agg", bufs=2)
                        nc.tensor.transpose(tp[:], hT_sb[:, hc, nb:nb + WIN],
                                            id_sb[:])
                        nc.scalar.copy(stg[:, wl, hc * 128:(hc + 1) * 128], tp[:])
                nc.sync.dma_start(
                    agin[q * MAXN:(q + 1) * MAXN].rearrange(
                        "(w p) h -> p w h", p=128), stg[:])
                if q == GPC - 1:
                    nc.gpsimd.collective_compute(
                        "AllGather", mybir.AluOpType.bypass,
                        replica_groups=groups,
                        ins=[agin.opt()], outs=[tbl.opt()])

            # ---- input projection: hT = Wp @ xT + bp, stage per graph
            xs_ld = []
            for s in range(NB // 512):
                xs = wrk.tile([128, 2, 512], dt.bfloat16, tag="xs", bufs=3)
                nc.sync.dma_start(xs[:], xT_in[:, :, s * 512:(s + 1) * 512])
                for hm in range(2):
                    pm = ps.tile([128, 512], dt.float32, tag="mT", bufs=2)
                    nc.tensor.matmul(pm[:], wp_sb[:, 0, hm * 128:(hm + 1) * 128],
                                     xs[:, 0, :], start=True, stop=False)
                    nc.tensor.matmul(pm[:], wp_sb[:, 1, hm * 128:(hm + 1) * 128],
                                     xs[:, 1, :], start=False, stop=True)
                    nc.vector.tensor_scalar_add(
                        hT_sb[:, hm, s * 512:(s + 1) * 512],
                        pm[:], bp_sb[:, hm:hm + 1])
                if s % 2 == 1:
                    stage_graph(0, s // 2)

            rsums = {}
            for l in range(L):
                tbl = tbls[l]
                # ---- layer weights
                wm_sb = wts.tile([128, 2, T, H], dt.bfloat16, tag="wm")
                nc.sync.dma_start(wm_sb[:], wm_in[l])
                bm_sb = wts.tile([T, H], dt.bfloat16, tag="bm")
                nc.sync.dma_start(bm_sb[:], bm_in[l])
                wih_sb = wts.tile([128, 2, 3 * H], dt.bfloat16, tag="wih")
                nc.sync.dma_start(wih_sb[:], wih_in[l])
                whh_sb = wts.tile([128, 2, 3 * H], dt.bfloat16, tag="whh")
                nc.sync.dma_start(whh_sb[:], whh_in[l])
                brz_sb = wts.tile([128, 4], dt.float32, tag="brz")
                nc.sync.dma_start(brz_sb[:], brz_in[l])
                bin_sb = wts.tile([128, 2], dt.float32, tag="bin")
                nc.sync.dma_start(bin_sb[:], bin_in[l])
                bhn_sb = wts.tile([128, 2], dt.float32, tag="bhn")
                nc.sync.dma_start(bhn_sb[:], bhn_in[l])

                # ---- gather-group streaming
                cglob = 0
                gg_tiles = {}

                def need(c, l=l, tbl=tbl, gg_tiles=gg_tiles):
                    gg = c // 8
                    while len(gg_tiles) == 0 or max(gg_tiles) < gg:
                        g_ = 0 if not gg_tiles else max(gg_tiles) + 1
                        rem = 8 if g_ < ngg - 1 else rem_last
                        Gt = gth.tile([128, rem, H], dt.bfloat16, tag="G",
                                      bufs=GPREF, name=f"G_{l}_{g_}")
                        nc.gpsimd.dma_gather(
                            Gt[:], tbl[:],
                            idx_sb[:, g_ * 64:g_ * 64 + rem * 8],
                            num_idxs=rem * 128, num_idxs_reg=rem * 128,
                            elem_size=H)
                        St = gth.tile([128, 8, WIN], dt.bfloat16, tag="S",
                                      bufs=GPREF, name=f"S_{l}_{g_}")
                        nc.sync.dma_start(St[:], smat_in[g_])
                        gg_tiles[g_] = (Gt, St)
                        if len(gg_tiles) > GPREF - 1:
                            del gg_tiles[min(gg_tiles)]
                    return gg_tiles[gg], c % 8

                # ---- aggregation + message + GRU per half-graph (512 nodes)
                for q in range(GPC):
                    for s2 in range(2):
                        A_sb = wrk.tile([128, T, 2, 4, WIN], dt.bfloat16,
                                        tag="A", bufs=2)
                        for wl in range(4):
                            w = q * WPG + s2 * 4 + wl
                            for th in range(T // 2):
                                pa = ps.tile([128, 512], dt.float32,
                                             tag="agg", bufs=2)
                                for ti in range(2):
                                    t = th * 2 + ti
                                    nchunks = int(budget[w, t])
                                    for hc in range(2):
                                        off = (ti * 2 + hc) * 128
                                        for ci in range(nchunks):
                                            (Gt, St), j = need(cglob + ci)
                                            nc.tensor.matmul(
                                                pa[:, off:off + 128],
                                                Gt[:, j, hc * 128:(hc + 1) * 128],
                                                St[:, j, :],
                                                start=(ci == 0),
                                                stop=(ci == nchunks - 1))
                                    cglob += nchunks
                                dst_ap = A_sb[:, th * 2:th * 2 + 2, :, wl, :]
                                src_ap = pa.rearrange("p (t c k) -> p t c k",
                                                      t=2, c=2)
                                if th % 2 == 0:
                                    nc.scalar.copy(dst_ap, src_ap)
                                else:
                                    nc.vector.tensor_copy(dst_ap, src_ap)

                        # ---- message matmuls for this 512-node slice
                        mT_sb = wrk.tile([128, 2, 512], dt.bfloat16,
                                         tag="mT", bufs=2)
                        nbase = q * MAXN + s2 * 512
                        for hm in range(2):
                            pm = ps.tile([128, 512], dt.float32, tag="mT",
                                         bufs=2)
                            nc.tensor.matmul(
                                pm[:], bm_sb[:, hm * 128:(hm + 1) * 128],
                                counts_sb[:, nbase:nbase + 512],
                                start=True, stop=False)
                            for t in range(T):
                                for hk in range(2):
                                    nc.tensor.matmul(
                                        pm[:],
                                        wm_sb[:, hk, t, hm * 128:(hm + 1) * 128],
                                        A_sb[:, t, hk, :, :],
                                        start=False, stop=(t == T - 1 and hk == 1))
                            nc.vector.tensor_copy(mT_sb[:, hm, :], pm[:])

                        # ---- GRU for these 512 nodes
                        nsl = slice(nbase, nbase + 512)
                        r_sb = grup.tile([128, 2, 512], dt.float32, tag="r",
                                         bufs=2)
                        z_sb = grup.tile([128, 2, 512], dt.float32, tag="z",
                                         bufs=2)
                        for gm in range(4):
                            pg = ps.tile([128, 512], dt.float32, tag="gru",
                                         bufs=3)
                            gsl = slice(gm * 128, (gm + 1) * 128)
                            nc.tensor.matmul(pg[:], wih_sb[:, 0, gsl],
                                             mT_sb[:, 0, :],
                                             start=True, stop=False)
                            nc.tensor.matmul(pg[:], wih_sb[:, 1, gsl],
                                             mT_sb[:, 1, :],
                                             start=False, stop=False)
                            nc.tensor.matmul(pg[:], whh_sb[:, 0, gsl],
                                             hT_sb[:, 0, nsl],
                                             start=False, stop=False)
                            nc.tensor.matmul(pg[:], whh_sb[:, 1, gsl],
                                             hT_sb[:, 1, nsl],
                                             start=False, stop=True)
                            dstg = r_sb[:, gm, :] if gm < 2 else z_sb[:, gm - 2, :]
                            nc.scalar.activation(
                                dstg, pg[:],
                                mybir.ActivationFunctionType.Sigmoid,
                                bias=brz_sb[:, gm:gm + 1])
                        nns, zds = [], []
                        for hc in range(2):
                            gsl = slice((4 + hc) * 128, (5 + hc) * 128)
                            ph = ps.tile([128, 512], dt.float32, tag="gru",
                                         bufs=3)
                            nc.tensor.matmul(ph[:], whh_sb[:, 0, gsl],
                                             hT_sb[:, 0, nsl],
                                             start=True, stop=False)
                            nc.tensor.matmul(ph[:], whh_sb[:, 1, gsl],
                                             hT_sb[:, 1, nsl],
                                             start=False, stop=True)
                            hnb = grup.tile([128, 512], dt.float32, tag="gt",
                                            bufs=3)
                            nc.vector.tensor_scalar_add(hnb[:], ph[:],
                                                        bhn_sb[:, hc:hc + 1])
                            rhn = grup.tile([128, 512], dt.float32, tag="gt",
                                            bufs=3)
                            nc.vector.tensor_mul(rhn[:], r_sb[:, hc, :], hnb[:])
                            pi = ps.tile([128, 512], dt.float32, tag="gru",
                                         bufs=3)
                            nc.tensor.matmul(pi[:], wih_sb[:, 0, gsl],
                                             mT_sb[:, 0, :],
                                             start=True, stop=False)
                            nc.tensor.matmul(pi[:], wih_sb[:, 1, gsl],
                                             mT_sb[:, 1, :],
                                             start=False, stop=True)
                            tsum = grup.tile([128, 512], dt.float32, tag="gt",
                                             bufs=3)
                            nc.vector.tensor_add(tsum[:], pi[:], rhn[:])
                            nn = grup.tile([128, 512], dt.float32, tag="nnb",
                                           bufs=2)
                            nc.scalar.activation(
                                nn[:], tsum[:],
                                mybir.ActivationFunctionType.Tanh,
                                bias=bin_sb[:, hc:hc + 1])
                            hprev = grup.tile([128, 512], dt.float32, tag="gt",
                                              bufs=3)
                            nc.vector.tensor_copy(hprev[:], hT_sb[:, hc, nsl])
                            d_ = grup.tile([128, 512], dt.float32, tag="gt",
                                           bufs=3)
                            nc.vector.tensor_sub(d_[:], hprev[:], nn[:])
                            zd = grup.tile([128, 512], dt.float32, tag="zdb",
                                           bufs=2)
                            nc.vector.tensor_mul(zd[:], z_sb[:, hc, :], d_[:])
                            nns.append(nn)
                            zds.append(zd)
                        for hc in range(2):
                            if l < L - 1:
                                nc.vector.tensor_add(hT_sb[:, hc, nsl],
                                                     nns[hc][:], zds[hc][:])
                            else:
                                hf = grup.tile([128, 512], dt.float32,
                                               tag="hf", bufs=2)
                                nc.vector.tensor_add(hf[:], nns[hc][:], zds[hc][:])
                                rs = grup.tile([128, 1], dt.float32, tag="rs",
                                               bufs=16)
                                nc.vector.tensor_reduce(
                                    rs[:], hf[:], axis=mybir.AxisListType.X,
                                    op=mybir.AluOpType.add)
                                rsums[(q, hc, s2)] = rs
                    # graph q's h fully updated -> stage for next layer's table
                    if l < L - 1:
                        stage_graph(l + 1, q)
                assert cglob == ctot, (cglob, ctot)

            # ---- readout
            for q in range(GPC):
                for hc in range(2):
                    nc.vector.tensor_add(outsb[:, hc, q:q + 1],
                                         rsums[(q, hc, 0)][:],
                                         rsums[(q, hc, 1)][:])
            nc.sync.dma_start(out_t.rearrange("c p g -> p c g"), outsb[:])

    nc.compile()
    return nc


def kernel(**inputs):
    meta, in_maps = _prep(**inputs)
    nc = _build(meta)
    res = run_bass_kernel_spmd(nc, in_maps, core_ids=list(range(NCORES)))
    GPC = meta["GPC"]
    out = np.zeros((meta["B"], H), np.float32)
    for c in range(NCORES):
        ot = res.results[c]["outT"]          # [2, 128, GPC]
        for g in range(GPC):
            out[c * GPC + g] = np.concatenate([ot[0, :, g], ot[1, :, g]])
    return out


# revision 7
# speedup vs baseline: 1.4058x; 1.4058x over previous
"""BatchGGNNEncoder Trainium2 kernel: 8-core SPMD, dst-sharded message passing.

Full inputs in, full output out. Internally:
  - core c owns nodes [c*4096, (c+1)*4096) = graphs [4c, 4c+4) (data parallel).
  - aggregate-first GGNN layer:
        A_t[v] = sum_{e: dst=v, type=t} h[src_e]         (one-hot matmuls, PSUM)
        m      = sum_t A_t @ Wm[t].T + counts_t * bm[t]  (dense matmuls)
        h      = GRU(m, h)                               (matmuls + DVE/ACT)
  - h table (bf16, node-major) lives in DRAM; rebuilt per layer via TWO
    AllGathers (first half fires mid-layer to hide latency); per-edge h[src]
    rows fetched with dma_gather (the Q7 descriptor-emission stream is the
    critical path: ~8.5us per 1024 rows, so everything else hides under it).
  - nodes are permuted within each graph to balance (type, 128-dst-window)
    group sizes so the compiled program structure is identical on all 8 cores.
"""
import numpy as np
import ml_dtypes

import concourse.bass as bass
import concourse.bacc as bacc
import concourse.mybir as mybir
import concourse.tile as tile
from concourse.bass_utils import run_bass_kernel_spmd

BF16 = ml_dtypes.bfloat16

# problem constants (hardcoded per harness contract)
MAXN, F, H, T, L = 1024, 215, 256, 8, 3
NCORES = 8
WIN = 128                     # dst window (one-hot free width)
WPG = MAXN // WIN             # 8 windows per graph
GPREF = 5                     # gather groups in flight


def _balance_graph(deg):
    """Assign 1024 nodes (deg: [1024, T] type-degrees) to 8 windows of 128.
    Window 7 takes the heaviest 128 nodes (cap 381/type); the rest fill
    windows 0..6 under a 256/type cap (2 chunks). A repair pass swaps nodes
    to clear residual over-cap windows, then windows are reordered so any
    remaining 3-chunk windows sit at the highest positions — aligning the
    cross-core max that sets the budget."""
    tot = deg.sum(1)
    order = np.argsort(-tot, kind="stable")
    wsum = np.zeros((WPG, T), np.float64)
    wcnt = np.zeros(WPG, np.int64)
    members = [[] for _ in range(WPG)]
    CAP, CAP7 = 256.0, 381.0
    rest = []
    for nd in order:
        if wcnt[7] < 128 and ((wsum[7] + deg[nd]) <= CAP7).all():
            members[7].append(nd)
            wsum[7] += deg[nd]
            wcnt[7] += 1
        else:
            rest.append(nd)
    for nd in rest:
        d = deg[nd]
        ns = wsum[:7] + d
        feas = (wcnt[:7] < 128) & (ns <= CAP).all(axis=1)
        if feas.any():
            load = np.where(feas, ns.max(axis=1), np.inf)
            best = int(np.argmin(load))
        else:
            nsall = wsum + d
            dcost = (np.ceil(nsall / 128) - np.ceil(wsum / 128)).sum(axis=1)
            dcost[wcnt >= 128] = np.inf
            best = int(np.argmin(dcost))
        members[best].append(nd)
        wsum[best] += d
        wcnt[best] += 1

    # repair: swap nodes to pull windows 0..6 under 256 per type (w7 < 384)
    wof = [np.array(m, np.int64) for m in members]
    dg = [deg[m] for m in wof]
    cap = np.full(WPG, CAP)
    cap[7] = 383.0
    for _ in range(400):
        ws = np.stack([d.sum(0) for d in dg])
        over = ws - cap[:, None]
        if (over <= 0).all():
            break
        w, t = np.unravel_index(np.argmax(over), over.shape)
        done = False
        for w2 in np.argsort(ws[:, t]):
            if w2 == w:
                continue
            da, db = dg[w], dg[w2]
            gain = da[:, t][:, None] - db[:, t][None, :]   # [na, nb]
            nsw = ws[w][None, None, :] - da[:, None, :] + db[None, :, :]
            nsw2 = ws[w2][None, None, :] + da[:, None, :] - db[None, :, :]
            ok = ((nsw <= np.maximum(ws[w], cap[w])[None, None, :]).all(2)
                  & (nsw2 <= cap[w2][None, None]).all(0 * 0 + 2)
                  & (gain > 0))
            if ok.any():
                a, b = np.unravel_index(
                    np.argmax(np.where(ok, gain, -1)), gain.shape)
                wof[w][a], wof[w2][b] = wof[w2][b], wof[w][a]
                dg[w] = deg[wof[w]]
                dg[w2] = deg[wof[w2]]
                done = True
                break
        if not done:
            break
    # reorder windows: fewest chunks first, heavy windows at high positions
    chunks = np.array([np.ceil(d.sum(0) / 128).sum() for d in dg])
    perm = np.argsort(chunks, kind="stable")
    return [wof[p] for p in perm]


def _prep(node_features, edge_index, edge_type, Wp, bp, Wm, bm, Wih, Whh, bih, bhh):
    """Host-side sharding/packing. Returns (meta, in_maps)."""
    x = np.asarray(node_features, np.float32)
    B = x.shape[0]
    N = B * MAXN
    GPC = B // NCORES             # graphs per core
    NB = GPC * MAXN               # nodes per core
    NWIN = GPC * WPG              # windows per core
    HALF = NB // 2                # nodes per AG half (2 graphs)
    src = np.asarray(edge_index[0]).astype(np.int64)
    dst = np.asarray(edge_index[1]).astype(np.int64)
    et = np.asarray(edge_type).astype(np.int64)

    # per-(node, type) in-degree
    cnt = np.zeros((N, T), np.int64)
    np.add.at(cnt, (dst, et), 1)

    # balance windows within each graph -> node permutation
    old2new = np.empty(N, np.int64)
    for g in range(B):
        mem = _balance_graph(cnt[g * MAXN:(g + 1) * MAXN])
        for w in range(WPG):
            pos = g * MAXN + w * WIN + np.arange(WIN)
            old2new[g * MAXN + mem[w]] = pos
    new2old = np.argsort(old2new)

    src_n = old2new[src]
    dst_n = old2new[dst]

    src_row = src_n                # table row = global node id (single AG)

    # group edges per core: key = ((gslot*WPG + w)*T + t)
    core = dst_n // NB
    rel = dst_n % NB
    col = rel % WIN
    key = (rel // WIN) * T + et
    NGRP = NWIN * T

    gsizes = np.zeros((NCORES, NGRP), np.int64)
    for c in range(NCORES):
        m = core == c
        gsizes[c] = np.bincount(key[m], minlength=NGRP)
    budget = np.ceil(gsizes.max(axis=0) / 128).astype(np.int64)  # chunks/group
    budget = np.maximum(budget, 1)
    ctot = int(budget.sum())
    ngg = (ctot + 7) // 8          # gather groups of <=8 chunks
    rem_last = ctot - 8 * (ngg - 1)
    nslots = ctot * 128
    gbase = np.concatenate([[0], np.cumsum(budget)])[:-1] * 128  # slot base

    # per-core slot arrays
    idx_maps, smat_maps = [], []
    counts_maps, xT_maps = [], []
    for c in range(NCORES):
        m = core == c
        kc, cc, sc = key[m], col[m], src_row[m]
        order = np.argsort(kc, kind="stable")
        kc, cc, sc = kc[order], cc[order], sc[order]
        grp_start = np.searchsorted(kc, np.arange(NGRP), side="left")
        rank = np.arange(kc.size) - grp_start[kc]
        slot = gbase[kc] + rank
        src16 = np.zeros(nslots, np.int16)
        scol = np.full(nslots, -1, np.int64)
        src16[slot] = sc.astype(np.int16)
        scol[slot] = cc
        # idx: wrapped [16, nslots/16] replicated to 128 partitions
        idx = np.tile(src16.reshape(nslots // 16, 16).T, (8, 1)).copy()
        idx_maps.append(idx)
        # one-hot S: [ngg, 128, 8, WIN] bf16 (last group zero-padded)
        smat = np.zeros((ngg * 8 * 128, WIN), BF16)
        valid = scol >= 0
        smat[np.nonzero(valid)[0], scol[valid]] = 1
        smat = smat.reshape(ngg, 8, 128, WIN)
        smat = np.ascontiguousarray(smat.transpose(0, 2, 1, 3))  # [ngg,128,8,WIN]
        smat_maps.append(smat)
        # counts (new order), [T, NB] bf16
        cslice = cnt[new2old[c * NB:(c + 1) * NB]]
        counts_maps.append(np.ascontiguousarray(cslice.T).astype(BF16))
        # xT [128, 2, NB] bf16: [p, k, node] = x[node, k*128+p]
        xs = x.reshape(N, F)[new2old[c * NB:(c + 1) * NB]]
        xp = np.zeros((NB, 2 * 128), np.float32)
        xp[:, :F] = xs
        xT = np.ascontiguousarray(xp.reshape(NB, 2, 128).transpose(2, 1, 0))
        xT_maps.append(xT.astype(BF16))

    # weights (shared across cores)
    Wp = np.asarray(Wp, np.float32); bp_ = np.asarray(bp, np.float32)
    Wm_ = np.asarray(Wm, np.float32); bm_ = np.asarray(bm, np.float32)
    Wih_ = np.asarray(Wih, np.float32); Whh_ = np.asarray(Whh, np.float32)
    bih_ = np.asarray(bih, np.float32); bhh_ = np.asarray(bhh, np.float32)

    wpT = np.zeros((128, 2, H), np.float32)          # [p, fk, h']
    wpt = Wp.T                                       # [F, H]
    wpT[:, 0, :] = wpt[0:128]
    wpT[:F - 128, 1, :] = wpt[128:F]
    wp_in = wpT.astype(BF16)
    bp_in = np.ascontiguousarray(bp_.reshape(2, 128).T)          # [128, 2]

    wm_in = np.ascontiguousarray(                     # [L, 128, 2, T, H]
        Wm_.transpose(0, 1, 3, 2)
        .reshape(L, T, 2, 128, H).transpose(0, 3, 2, 1, 4)).astype(BF16)
    bm_in = bm_.astype(BF16)                          # [L, T, H]
    wih_in = np.ascontiguousarray(                    # [L, 128, 2, 3H]
        Wih_.transpose(0, 2, 1).reshape(L, 2, 128, 3 * H).transpose(0, 2, 1, 3)
    ).astype(BF16)
    whh_in = np.ascontiguousarray(
        Whh_.transpose(0, 2, 1).reshape(L, 2, 128, 3 * H).transpose(0, 2, 1, 3)
    ).astype(BF16)
    brz = bih_[:, :2 * H] + bhh_[:, :2 * H]
    brz_in = np.ascontiguousarray(brz.reshape(L, 4, 128).transpose(0, 2, 1))
    bin_in = np.ascontiguousarray(bih_[:, 2 * H:].reshape(L, 2, 128).transpose(0, 2, 1))
    bhn_in = np.ascontiguousarray(bhh_[:, 2 * H:].reshape(L, 2, 128).transpose(0, 2, 1))
    id128 = np.eye(128, dtype=BF16)

    in_maps = []
    for c in range(NCORES):
        in_maps.append({
            "xT": xT_maps[c], "idx": idx_maps[c], "smat": smat_maps[c],
            "countsT": counts_maps[c],
            "wpT": wp_in, "bp": bp_in, "wmT": wm_in, "bmT": bm_in,
            "wihT": wih_in, "whhT": whh_in,
            "brz": brz_in, "bin_": bin_in, "bhn": bhn_in, "id128": id128,
        })
    meta = dict(B=B, N=N, GPC=GPC, NB=NB, NWIN=NWIN, HALF=HALF,
                budget=budget.reshape(NWIN, T), ctot=ctot, ngg=ngg,
                rem_last=rem_last, new2old=new2old)
    return meta, in_maps


def _build(meta):
    """Build the SPMD Bass program (identical across cores)."""
    dt = mybir.dt
    N, NB, GPC, NWIN = meta["N"], meta["NB"], meta["GPC"], meta["NWIN"]
    HALF = meta["HALF"]
    budget, ngg, ctot = meta["budget"], meta["ngg"], meta["ctot"]
    rem_last = meta["rem_last"]
    SLOT16 = ctot * 8

    nc = bacc.Bacc("TRN2", target_bir_lowering=False, debug=False,
                   enable_asserts=False, num_devices=NCORES)

    # ---- I/O
    xT_in = nc.dram_tensor("xT", [128, 2, NB], dt.bfloat16, kind="ExternalInput").ap()
    idx_in = nc.dram_tensor("idx", [128, SLOT16], dt.int16, kind="ExternalInput").ap()
    smat_in = nc.dram_tensor("smat", [ngg, 128, 8, WIN], dt.bfloat16, kind="ExternalInput").ap()
    counts_in = nc.dram_tensor("countsT", [T, NB], dt.bfloat16, kind="ExternalInput").ap()
    wp_in = nc.dram_tensor("wpT", [128, 2, H], dt.bfloat16, kind="ExternalInput").ap()
    bp_in = nc.dram_tensor("bp", [128, 2], dt.float32, kind="ExternalInput").ap()
    wm_in = nc.dram_tensor("wmT", [L, 128, 2, T, H], dt.bfloat16, kind="ExternalInput").ap()
    bm_in = nc.dram_tensor("bmT", [L, T, H], dt.bfloat16, kind="ExternalInput").ap()
    wih_in = nc.dram_tensor("wihT", [L, 128, 2, 3 * H], dt.bfloat16, kind="ExternalInput").ap()
    whh_in = nc.dram_tensor("whhT", [L, 128, 2, 3 * H], dt.bfloat16, kind="ExternalInput").ap()
    brz_in = nc.dram_tensor("brz", [L, 128, 4], dt.float32, kind="ExternalInput").ap()
    bin_in = nc.dram_tensor("bin_", [L, 128, 2], dt.float32, kind="ExternalInput").ap()
    bhn_in = nc.dram_tensor("bhn", [L, 128, 2], dt.float32, kind="ExternalInput").ap()
    id_in = nc.dram_tensor("id128", [128, 128], dt.bfloat16, kind="ExternalInput").ap()
    out_t = nc.dram_tensor("outT", [2, 128, GPC], dt.float32, kind="ExternalOutput").ap()

    groups = [list(range(NCORES))]

    with tile.TileContext(nc) as tc:
        with (
            tc.tile_pool(name="per", bufs=1) as per,       # persistent SBUF
            tc.tile_pool(name="wts", bufs=2) as wts,       # per-layer weights
            tc.tile_pool(name="gth", bufs=GPREF) as gth,   # gather/S stream
            tc.tile_pool(name="wrk", bufs=2) as wrk,       # A/mT/staging
            tc.tile_pool(name="gru", bufs=2) as grup,      # GRU temps
            tc.tile_pool(name="ps", bufs=1, space="PSUM") as ps,
            tc.tile_pool(name="dram", bufs=2, space="DRAM") as dram,
        ):
            # persistent loads
            idx_sb = per.tile([128, SLOT16], dt.int16)
            nc.sync.dma_start(idx_sb[:], idx_in[:])
            counts_sb = per.tile([T, NB], dt.bfloat16)
            nc.sync.dma_start(counts_sb[:], counts_in[:])
            wp_sb = per.tile([128, 2, H], dt.bfloat16)
            nc.sync.dma_start(wp_sb[:], wp_in[:])
            bp_sb = per.tile([128, 2], dt.float32)
            nc.sync.dma_start(bp_sb[:], bp_in[:])
            id_sb = per.tile([128, 128], dt.bfloat16)
            nc.sync.dma_start(id_sb[:], id_in[:])
            hT_sb = per.tile([128, 2, NB], dt.bfloat16)
            outsb = per.tile([128, 2, GPC], dt.float32)
            nc.vector.memset(outsb[:], 0.0)

            # per-layer table + AG staging rings (DRAM)
            tbls, agins = [], []
            for l in range(L):
                tbls.append(dram.tile([N, H], dt.bfloat16, tag="tbl", bufs=2,
                                      addr_space="Shared", name=f"tbl{l}"))
                agins.append(dram.tile([NB, H], dt.bfloat16, tag="agin",
                                       bufs=2, name=f"agin{l}"))

            def stage_graph(l, q):
                """PE-transpose graph q's h (h-major) to node-major, DMA to
                agin, and fire the half-AllGathers for layer l's table."""
                agin, tbl = agins[l], tbls[l]
                stg = wrk.tile([128, WPG, H], dt.bfloat16, tag="stg", bufs=2)
                for wl in range(WPG):
                    nb = q * MAXN + wl * WIN
                    for hc in range(2):
                        tp = ps.tile([128, 128], dt.bfloat16, tag="agg", bufs=2)
                        nc.tensor.transpose(tp[:], hT_sb[:, hc, nb:nb + WIN],
                                            id_sb[:])
                        nc.scalar.copy(stg[:, wl, hc * 128:(hc + 1) * 128], tp[:])
                nc.sync.dma_start(
                    agin[q * MAXN:(q + 1) * MAXN].rearrange(
                        "(w p) h -> p w h", p=128), stg[:])
                if q == GPC - 1:
                    nc.gpsimd.collective_compute(
                        "AllGather", mybir.AluOpType.bypass,
                        replica_groups=groups,
                        ins=[agin.opt()], outs=[tbl.opt()])

            # ---- input projection: hT = Wp @ xT + bp, stage per graph
            xs_ld = []
            for s in range(NB // 512):
                xs = wrk.tile([128, 2, 512], dt.bfloat16, tag="xs", bufs=3)
                nc.sync.dma_start(xs[:], xT_in[:, :, s * 512:(s + 1) * 512])
                for hm in range(2):
                    pm = ps.tile([128, 512], dt.float32, tag="mT", bufs=2)
                    nc.tensor.matmul(pm[:], wp_sb[:, 0, hm * 128:(hm + 1) * 128],
                                     xs[:, 0, :], start=True, stop=False)
                    nc.tensor.matmul(pm[:], wp_sb[:, 1, hm * 128:(hm + 1) * 128],
                                     xs[:, 1, :], start=False, stop=True)
                    nc.vector.tensor_scalar_add(
                        hT_sb[:, hm, s * 512:(s + 1) * 512],
                        pm[:], bp_sb[:, hm:hm + 1])
                if s % 2 == 1:
                    stage_graph(0, s // 2)

            rsums = {}
            for l in range(L):
                tbl = tbls[l]
                # ---- layer weights
                wm_sb = wts.tile([128, 2, T, H], dt.bfloat16, tag="wm")
                nc.sync.dma_start(wm_sb[:], wm_in[l])
                bm_sb = wts.tile([T, H], dt.bfloat16, tag="bm")
                nc.sync.dma_start(bm_sb[:], bm_in[l])
                wih_sb = wts.tile([128, 2, 3 * H], dt.bfloat16, tag="wih")
                nc.sync.dma_start(wih_sb[:], wih_in[l])
                whh_sb = wts.tile([128, 2, 3 * H], dt.bfloat16, tag="whh")
                nc.sync.dma_start(whh_sb[:], whh_in[l])
                brz_sb = wts.tile([128, 4], dt.float32, tag="brz")
                nc.sync.dma_start(brz_sb[:], brz_in[l])
                bin_sb = wts.tile([128, 2], dt.float32, tag="bin")
                nc.sync.dma_start(bin_sb[:], bin_in[l])
                bhn_sb = wts.tile([128, 2], dt.float32, tag="bhn")
                nc.sync.dma_start(bhn_sb[:], bhn_in[l])

                # ---- gather-group streaming
                cglob = 0
                gg_tiles = {}

                def need(c, l=l, tbl=tbl, gg_tiles=gg_tiles):
                    gg = c // 8
                    while len(gg_tiles) == 0 or max(gg_tiles) < gg:
                        g_ = 0 if not gg_tiles else max(gg_tiles) + 1
                        rem = 8 if g_ < ngg - 1 else rem_last
                        Gt = gth.tile([128, rem, H], dt.bfloat16, tag="G",
                                      bufs=GPREF, name=f"G_{l}_{g_}")
                        nc.gpsimd.dma_gather(
                            Gt[:], tbl[:],
                            idx_sb[:, g_ * 64:g_ * 64 + rem * 8],
                            num_idxs=rem * 128, num_idxs_reg=rem * 128,
                            elem_size=H)
                        St = gth.tile([128, 8, WIN], dt.bfloat16, tag="S",
                                      bufs=GPREF, name=f"S_{l}_{g_}")
                        nc.sync.dma_start(St[:], smat_in[g_])
                        gg_tiles[g_] = (Gt, St)
                        if len(gg_tiles) > GPREF - 1:
                            del gg_tiles[min(gg_tiles)]
                    return gg_tiles[gg], c % 8

                # ---- aggregation + message + GRU per half-graph (512 nodes)
                for q in range(GPC):
                    for s2 in range(2):
                        A_sb = wrk.tile([128, T, 2, 4, WIN], dt.bfloat16,
                                        tag="A", bufs=2)
                        for wl in range(4):
                            w = q * WPG + s2 * 4 + wl
                            for th in range(T // 2):
                                pa = ps.tile([128, 512], dt.float32,
                                             tag="agg", bufs=2)
                                for ti in range(2):
                                    t = th * 2 + ti
                                    nchunks = int(budget[w, t])
                                    for hc in range(2):
                                        off = (ti * 2 + hc) * 128
                                        for ci in range(nchunks):
                                            (Gt, St), j = need(cglob + ci)
                                            nc.tensor.matmul(
                                                pa[:, off:off + 128],
                                                Gt[:, j, hc * 128:(hc + 1) * 128],
                                                St[:, j, :],
                                                start=(ci == 0),
                                                stop=(ci == nchunks - 1))
                                    cglob += nchunks
                                dst_ap = A_sb[:, th * 2:th * 2 + 2, :, wl, :]
                                src_ap = pa.rearrange("p (t c k) -> p t c k",
                                                      t=2, c=2)
                                if th % 2 == 0:
                                    nc.scalar.copy(dst_ap, src_ap)
                                else:
                                    nc.vector.tensor_copy(dst_ap, src_ap)

                        # ---- message matmuls for this 512-node slice
                        mT_sb = wrk.tile([128, 2, 512], dt.bfloat16,
                                         tag="mT", bufs=2)
                        nbase = q * MAXN + s2 * 512
                        for hm in range(2):
                            pm = ps.tile([128, 512], dt.float32, tag="mT",
                                         bufs=2)
                            nc.tensor.matmul(
                                pm[:], bm_sb[:, hm * 128:(hm + 1) * 128],
                                counts_sb[:, nbase:nbase + 512],
                                start=True, stop=False)
                            for t in range(T):
                                for hk in range(2):
                                    nc.tensor.matmul(
                                        pm[:],
                                        wm_sb[:, hk, t, hm * 128:(hm + 1) * 128],
                                        A_sb[:, t, hk, :, :],
                                        start=False, stop=(t == T - 1 and hk == 1))
                            nc.vector.tensor_copy(mT_sb[:, hm, :], pm[:])

                        # ---- GRU for these 512 nodes
                        nsl = slice(nbase, nbase + 512)
                        r_sb = grup.tile([128, 2, 512], dt.float32, tag="r",
                                         bufs=2)
                        z_sb = grup.tile([128, 2, 512], dt.float32, tag="z",
                                         bufs=2)
                        for gm in range(4):
                            pg = ps.tile([128, 512], dt.float32, tag="gru",
                                         bufs=3)
                            gsl = slice(gm * 128, (gm + 1) * 128)
                            nc.tensor.matmul(pg[:], wih_sb[:, 0, gsl],
                                             mT_sb[:, 0, :],
                                             start=True, stop=False)
                            nc.tensor.matmul(pg[:], wih_sb[:, 1, gsl],
                                             mT_sb[:, 1, :],
                                             start=False, stop=False)
                            nc.tensor.matmul(pg[:], whh_sb[:, 0, gsl],
                                             hT_sb[:, 0, nsl],
                                             start=False, stop=False)
                            nc.tensor.matmul(pg[:], whh_sb[:, 1, gsl],
                                             hT_sb[:, 1, nsl],
                                             start=False, stop=True)
                            dstg = r_sb[:, gm, :] if gm < 2 else z_sb[:, gm - 2, :]
                            nc.scalar.activation(
                                dstg, pg[:],
                                mybir.ActivationFunctionType.Sigmoid,
                                bias=brz_sb[:, gm:gm + 1])
                        nns, zds = [], []
                        for hc in range(2):
                            gsl = slice((4 + hc) * 128, (5 + hc) * 128)
                            ph = ps.tile([128, 512], dt.float32, tag="gru",
                                         bufs=3)
                            nc.tensor.matmul(ph[:], whh_sb[:, 0, gsl],
                                             hT_sb[:, 0, nsl],
                                             start=True, stop=False)
                            nc.tensor.matmul(ph[:], whh_sb[:, 1, gsl],
                                             hT_sb[:, 1, nsl],
                                             start=False, stop=True)
                            hnb = grup.tile([128, 512], dt.float32, tag="gt",
                                            bufs=3)
                            nc.vector.tensor_scalar_add(hnb[:], ph[:],
                                                        bhn_sb[:, hc:hc + 1])
                            rhn = grup.tile([128, 512], dt.float32, tag="gt",
                                            bufs=3)
                            nc.vector.tensor_mul(rhn[:], r_sb[:, hc, :], hnb[:])
                            pi = ps.tile([128, 512], dt.float32, tag="gru",
                                         bufs=3)
                            nc.tensor.matmul(pi[:], wih_sb[:, 0, gsl],
                                             mT_sb[:, 0, :],
                                             start=True, stop=False)
                            nc.tensor.matmul(pi[:], wih_sb[:, 1, gsl],
                                             mT_sb[:, 1, :],
                                             start=False, stop=True)
                            tsum = grup.tile([128, 512], dt.float32, tag="gt",
                                             bufs=3)
                            nc.vector.tensor_add(tsum[:], pi[:], rhn[:])
                            nn = grup.tile([128, 512], dt.float32, tag="nnb",
                                           bufs=2)
                            nc.scalar.activation(
                                nn[:], tsum[:],
                                mybir.ActivationFunctionType.Tanh,
                                bias=bin_sb[:, hc:hc + 1])
                            hprev = grup.tile([128, 512], dt.float32, tag="gt",
                                              bufs=3)
                            nc.vector.tensor_copy(hprev[:], hT_sb[:, hc, nsl])
                            d_ = grup.tile([128, 512], dt.float32, tag="gt",
                                           bufs=3)
                            nc.vector.tensor_sub(d_[:], hprev[:], nn[:])
                            zd = grup.tile([128, 512], dt.float32, tag="zdb",
                                           bufs=2)
                            nc.vector.tensor_mul(zd[:], z_sb[:, hc, :], d_[:])
                            nns.append(nn)
                            zds.append(zd)
                        for hc in range(2):
                            if l < L - 1:
                                nc.vector.tensor_add(hT_sb[:, hc, nsl],
                                                     nns[hc][:], zds[hc][:])
                            else:
                                hf = grup.tile([128, 512], dt.float32,
                                               tag="hf", bufs=2)
                                nc.vector.tensor_add(hf[:], nns[hc][:], zds[hc][:])
                                rs = grup.tile([128, 1], dt.float32, tag="rs",
                                               bufs=16)
                                nc.vector.tensor_reduce(
                                    rs[:], hf[:], axis=mybir.AxisListType.X,
                                    op=mybir.AluOpType.add)
                                rsums[(q, hc, s2)] = rs
                    # graph q's h fully updated -> stage for next layer's table
                    if l < L - 1:
                        stage_graph(l + 1, q)
                assert cglob == ctot, (cglob, ctot)

            # ---- readout
            for q in range(GPC):
                for hc in range(2):
                    nc.vector.tensor_add(outsb[:, hc, q:q + 1],
                                         rsums[(q, hc, 0)][:],
                                         rsums[(q, hc, 1)][:])
            nc.sync.dma_start(out_t.rearrange("c p g -> p c g"), outsb[:])

    nc.compile()
    return nc


def kernel(**inputs):
    meta, in_maps = _prep(**inputs)
    nc = _build(meta)
    res = run_bass_kernel_spmd(nc, in_maps, core_ids=list(range(NCORES)))
    GPC = meta["GPC"]
    out = np.zeros((meta["B"], H), np.float32)
    for c in range(NCORES):
        ot = res.results[c]["outT"]          # [2, 128, GPC]
        for g in range(GPC):
            out[c * GPC + g] = np.concatenate([ot[0, :, g], ot[1, :, g]])
    return out


# revision 14
# speedup vs baseline: 1.4119x; 1.0043x over previous
"""BatchGGNNEncoder Trainium2 kernel: 8-core SPMD, dst-sharded message passing.

Full inputs in, full output out. Internally:
  - core c owns nodes [c*4096, (c+1)*4096) = graphs [4c, 4c+4) (data parallel).
  - aggregate-first GGNN layer:
        A_t[v] = sum_{e: dst=v, type=t} h[src_e]         (one-hot matmuls, PSUM)
        m      = sum_t A_t @ Wm[t].T + counts_t * bm[t]  (dense matmuls)
        h      = GRU(m, h)                               (matmuls + DVE/ACT)
  - h table (bf16, node-major) lives in DRAM; rebuilt per layer via TWO
    AllGathers (first half fires mid-layer to hide latency); per-edge h[src]
    rows fetched with dma_gather (the Q7 descriptor-emission stream is the
    critical path: ~8.5us per 1024 rows, so everything else hides under it).
  - nodes are permuted within each graph to balance (type, 128-dst-window)
    group sizes so the compiled program structure is identical on all 8 cores.
"""
import numpy as np
import ml_dtypes

import concourse.bass as bass
import concourse.bacc as bacc
import concourse.mybir as mybir
import concourse.tile as tile
from concourse.bass_utils import run_bass_kernel_spmd

BF16 = ml_dtypes.bfloat16

# problem constants (hardcoded per harness contract)
MAXN, F, H, T, L = 1024, 215, 256, 8, 3
NCORES = 8
WIN = 128                     # dst window (one-hot free width)
WPG = MAXN // WIN             # 8 windows per graph
GPREF = 5                     # gather groups in flight


def _balance_graph(deg):
    """Assign 1024 nodes (deg: [1024, T] type-degrees) to 8 windows of 128.
    Window 7 takes the heaviest 128 nodes (cap 381/type); the rest fill
    windows 0..6 under a 256/type cap (2 chunks). A repair pass swaps nodes
    to clear residual over-cap windows, then windows are reordered so any
    remaining 3-chunk windows sit at the highest positions — aligning the
    cross-core max that sets the budget."""
    tot = deg.sum(1)
    order = np.argsort(-tot, kind="stable")
    wsum = np.zeros((WPG, T), np.float64)
    wcnt = np.zeros(WPG, np.int64)
    members = [[] for _ in range(WPG)]
    CAP, CAP7 = 256.0, 381.0
    rest = []
    for nd in order:
        if wcnt[7] < 128 and ((wsum[7] + deg[nd]) <= CAP7).all():
            members[7].append(nd)
            wsum[7] += deg[nd]
            wcnt[7] += 1
        else:
            rest.append(nd)
    for nd in rest:
        d = deg[nd]
        ns = wsum[:7] + d
        feas = (wcnt[:7] < 128) & (ns <= CAP).all(axis=1)
        if feas.any():
            load = np.where(feas, ns.max(axis=1), np.inf)
            best = int(np.argmin(load))
        else:
            nsall = wsum + d
            dcost = (np.ceil(nsall / 128) - np.ceil(wsum / 128)).sum(axis=1)
            dcost[wcnt >= 128] = np.inf
            best = int(np.argmin(dcost))
        members[best].append(nd)
        wsum[best] += d
        wcnt[best] += 1

    # repair: swap nodes to pull windows 0..6 under 256 per type (w7 < 384)
    wof = [np.array(m, np.int64) for m in members]
    dg = [deg[m] for m in wof]
    cap = np.full(WPG, CAP)
    cap[7] = 383.0
    for _ in range(400):
        ws = np.stack([d.sum(0) for d in dg])
        over = ws - cap[:, None]
        if (over <= 0).all():
            break
        w, t = np.unravel_index(np.argmax(over), over.shape)
        done = False
        for w2 in np.argsort(ws[:, t]):
            if w2 == w:
                continue
            da, db = dg[w], dg[w2]
            gain = da[:, t][:, None] - db[:, t][None, :]   # [na, nb]
            nsw = ws[w][None, None, :] - da[:, None, :] + db[None, :, :]
            nsw2 = ws[w2][None, None, :] + da[:, None, :] - db[None, :, :]
            ok = ((nsw <= np.maximum(ws[w], cap[w])[None, None, :]).all(2)
                  & (nsw2 <= cap[w2][None, None]).all(0 * 0 + 2)
                  & (gain > 0))
            if ok.any():
                a, b = np.unravel_index(
                    np.argmax(np.where(ok, gain, -1)), gain.shape)
                wof[w][a], wof[w2][b] = wof[w2][b], wof[w][a]
                dg[w] = deg[wof[w]]
                dg[w2] = deg[wof[w2]]
                done = True
                break
        if not done:
            break
    # reorder windows: fewest chunks first, heavy windows at high positions
    chunks = np.array([np.ceil(d.sum(0) / 128).sum() for d in dg])
    perm = np.argsort(chunks, kind="stable")
    return [wof[p] for p in perm]


def _prep(node_features, edge_index, edge_type, Wp, bp, Wm, bm, Wih, Whh, bih, bhh):
    """Host-side sharding/packing. Returns (meta, in_maps)."""
    x = np.asarray(node_features, np.float32)
    B = x.shape[0]
    N = B * MAXN
    GPC = B // NCORES             # graphs per core
    NB = GPC * MAXN               # nodes per core
    NWIN = GPC * WPG              # windows per core
    HALF = NB // 2                # nodes per AG half (2 graphs)
    src = np.asarray(edge_index[0]).astype(np.int64)
    dst = np.asarray(edge_index[1]).astype(np.int64)
    et = np.asarray(edge_type).astype(np.int64)

    # per-(node, type) in-degree
    cnt = np.zeros((N, T), np.int64)
    np.add.at(cnt, (dst, et), 1)

    # balance windows within each graph -> node permutation
    old2new = np.empty(N, np.int64)
    for g in range(B):
        mem = _balance_graph(cnt[g * MAXN:(g + 1) * MAXN])
        for w in range(WPG):
            pos = g * MAXN + w * WIN + np.arange(WIN)
            old2new[g * MAXN + mem[w]] = pos
    new2old = np.argsort(old2new)

    src_n = old2new[src]
    dst_n = old2new[dst]

    src_row = src_n                # table row = global node id (single AG)

    # group edges per core: key = ((gslot*WPG + w)*T + t)
    core = dst_n // NB
    rel = dst_n % NB
    col = rel % WIN
    key = (rel // WIN) * T + et
    NGRP = NWIN * T

    gsizes = np.zeros((NCORES, NGRP), np.int64)
    for c in range(NCORES):
        m = core == c
        gsizes[c] = np.bincount(key[m], minlength=NGRP)
    budget = np.ceil(gsizes.max(axis=0) / 128).astype(np.int64)  # chunks/group
    budget = np.maximum(budget, 1)
    ctot = int(budget.sum())
    ngg = (ctot + 7) // 8          # gather groups of <=8 chunks
    rem_last = ctot - 8 * (ngg - 1)
    nslots = ctot * 128
    gbase = np.concatenate([[0], np.cumsum(budget)])[:-1] * 128  # slot base

    # per-core slot arrays
    idx_maps, smat_maps = [], []
    counts_maps, xT_maps = [], []
    for c in range(NCORES):
        m = core == c
        kc, cc, sc = key[m], col[m], src_row[m]
        order = np.argsort(kc, kind="stable")
        kc, cc, sc = kc[order], cc[order], sc[order]
        grp_start = np.searchsorted(kc, np.arange(NGRP), side="left")
        rank = np.arange(kc.size) - grp_start[kc]
        slot = gbase[kc] + rank
        src16 = np.zeros(nslots, np.int16)
        scol = np.full(nslots, -1, np.int64)
        src16[slot] = sc.astype(np.int16)
        scol[slot] = cc
        # idx: wrapped [16, nslots/16] replicated to 128 partitions
        idx = np.tile(src16.reshape(nslots // 16, 16).T, (8, 1)).copy()
        idx_maps.append(idx)
        # one-hot S: [ngg, 128, 8, WIN] bf16 (last group zero-padded)
        smat = np.zeros((ngg * 8 * 128, WIN), BF16)
        valid = scol >= 0
        smat[np.nonzero(valid)[0], scol[valid]] = 1
        smat = smat.reshape(ngg, 8, 128, WIN)
        smat = np.ascontiguousarray(smat.transpose(0, 2, 1, 3))  # [ngg,128,8,WIN]
        smat_maps.append(smat)
        # counts (new order), [T, NB] bf16
        cslice = cnt[new2old[c * NB:(c + 1) * NB]]
        counts_maps.append(np.ascontiguousarray(cslice.T).astype(BF16))
        # xT [128, 2, NB] bf16: [p, k, node] = x[node, k*128+p]
        xs = x.reshape(N, F)[new2old[c * NB:(c + 1) * NB]]
        xp = np.zeros((NB, 2 * 128), np.float32)
        xp[:, :F] = xs
        xT = np.ascontiguousarray(xp.reshape(NB, 2, 128).transpose(2, 1, 0))
        xT_maps.append(xT.astype(BF16))

    # node-major padded x table (same for all cores): layer 0 aggregates x
    # directly (aggregation is linear in h0 = Wp x + bp, so Wp folds into
    # Wm[0] and bp into bm[0] on the host)
    xtab = np.zeros((N, 2 * 128), np.float32)
    xtab[:, :F] = x.reshape(N, F)[new2old]
    xtab = xtab.astype(BF16)

    # weights (shared across cores)
    Wp = np.asarray(Wp, np.float32); bp_ = np.asarray(bp, np.float32)
    Wm_ = np.asarray(Wm, np.float32); bm_ = np.asarray(bm, np.float32)
    Wih_ = np.asarray(Wih, np.float32); Whh_ = np.asarray(Whh, np.float32)
    bih_ = np.asarray(bih, np.float32); bhh_ = np.asarray(bhh, np.float32)

    wpT = np.zeros((128, 2, H), np.float32)          # [p, fk, h']
    wpt = Wp.T                                       # [F, H]
    wpT[:, 0, :] = wpt[0:128]
    wpT[:F - 128, 1, :] = wpt[128:F]
    wp_in = wpT.astype(BF16)
    bp_in = np.ascontiguousarray(bp_.reshape(2, 128).T)          # [128, 2]

    wm_in = np.ascontiguousarray(                     # [L, 128, 2, T, H]
        Wm_.transpose(0, 1, 3, 2)
        .reshape(L, T, 2, 128, H).transpose(0, 3, 2, 1, 4)).astype(BF16)
    bm_in = bm_.astype(BF16).copy()                   # [L, T, H]
    # layer 0 in x-space: Wt[t] = Wm[0,t] @ Wp (pad in-dim F->256),
    # bm~[0,t] = bm[0,t] + Wm[0,t] @ bp
    wt0 = np.einsum("toh,hf->tfo", Wm_[0], Wp)        # [T, F, H]
    wt0p = np.zeros((T, 256, H), np.float32)
    wt0p[:, :F, :] = wt0
    wm_in[0] = np.ascontiguousarray(
        wt0p.reshape(T, 2, 128, H).transpose(2, 1, 0, 3)).astype(BF16)
    bm_in[0] = (bm_[0] + np.einsum("toh,h->to", Wm_[0], bp_)).astype(BF16)
    wih_in = np.ascontiguousarray(                    # [L, 128, 2, 3H]
        Wih_.transpose(0, 2, 1).reshape(L, 2, 128, 3 * H).transpose(0, 2, 1, 3)
    ).astype(BF16)
    whh_in = np.ascontiguousarray(
        Whh_.transpose(0, 2, 1).reshape(L, 2, 128, 3 * H).transpose(0, 2, 1, 3)
    ).astype(BF16)
    brz = bih_[:, :2 * H] + bhh_[:, :2 * H]
    brz_in = np.ascontiguousarray(brz.reshape(L, 4, 128).transpose(0, 2, 1))
    bin_in = np.ascontiguousarray(bih_[:, 2 * H:].reshape(L, 2, 128).transpose(0, 2, 1))
    bhn_in = np.ascontiguousarray(bhh_[:, 2 * H:].reshape(L, 2, 128).transpose(0, 2, 1))
    id128 = np.eye(128, dtype=BF16)

    in_maps = []
    for c in range(NCORES):
        in_maps.append({
            "xT": xT_maps[c], "xtab": xtab, "idx": idx_maps[c],
            "smat": smat_maps[c], "countsT": counts_maps[c],
            "wpT": wp_in, "bp": bp_in, "wmT": wm_in, "bmT": bm_in,
            "wihT": wih_in, "whhT": whh_in,
            "brz": brz_in, "bin_": bin_in, "bhn": bhn_in, "id128": id128,
        })
    meta = dict(B=B, N=N, GPC=GPC, NB=NB, NWIN=NWIN, HALF=HALF,
                budget=budget.reshape(NWIN, T), ctot=ctot, ngg=ngg,
                rem_last=rem_last, new2old=new2old)
    return meta, in_maps


def _build(meta):
    """Build the SPMD Bass program (identical across cores)."""
    dt = mybir.dt
    N, NB, GPC, NWIN = meta["N"], meta["NB"], meta["GPC"], meta["NWIN"]
    HALF = meta["HALF"]
    budget, ngg, ctot = meta["budget"], meta["ngg"], meta["ctot"]
    rem_last = meta["rem_last"]
    SLOT16 = ctot * 8

    nc = bacc.Bacc("TRN2", target_bir_lowering=False, debug=False,
                   enable_asserts=False, num_devices=NCORES)

    # ---- I/O
    xT_in = nc.dram_tensor("xT", [128, 2, NB], dt.bfloat16, kind="ExternalInput").ap()
    xtab_in = nc.dram_tensor("xtab", [N, 2 * 128], dt.bfloat16, kind="ExternalInput").ap()
    idx_in = nc.dram_tensor("idx", [128, SLOT16], dt.int16, kind="ExternalInput").ap()
    smat_in = nc.dram_tensor("smat", [ngg, 128, 8, WIN], dt.bfloat16, kind="ExternalInput").ap()
    counts_in = nc.dram_tensor("countsT", [T, NB], dt.bfloat16, kind="ExternalInput").ap()
    wp_in = nc.dram_tensor("wpT", [128, 2, H], dt.bfloat16, kind="ExternalInput").ap()
    bp_in = nc.dram_tensor("bp", [128, 2], dt.float32, kind="ExternalInput").ap()
    wm_in = nc.dram_tensor("wmT", [L, 128, 2, T, H], dt.bfloat16, kind="ExternalInput").ap()
    bm_in = nc.dram_tensor("bmT", [L, T, H], dt.bfloat16, kind="ExternalInput").ap()
    wih_in = nc.dram_tensor("wihT", [L, 128, 2, 3 * H], dt.bfloat16, kind="ExternalInput").ap()
    whh_in = nc.dram_tensor("whhT", [L, 128, 2, 3 * H], dt.bfloat16, kind="ExternalInput").ap()
    brz_in = nc.dram_tensor("brz", [L, 128, 4], dt.float32, kind="ExternalInput").ap()
    bin_in = nc.dram_tensor("bin_", [L, 128, 2], dt.float32, kind="ExternalInput").ap()
    bhn_in = nc.dram_tensor("bhn", [L, 128, 2], dt.float32, kind="ExternalInput").ap()
    id_in = nc.dram_tensor("id128", [128, 128], dt.bfloat16, kind="ExternalInput").ap()
    out_t = nc.dram_tensor("outT", [2, 128, GPC], dt.float32, kind="ExternalOutput").ap()

    groups = [list(range(NCORES))]

    with tile.TileContext(nc) as tc:
        with (
            tc.tile_pool(name="per", bufs=1) as per,       # persistent SBUF
            tc.tile_pool(name="wts", bufs=2) as wts,       # per-layer weights
            tc.tile_pool(name="gth", bufs=GPREF) as gth,   # gather/S stream
            tc.tile_pool(name="wrk", bufs=2) as wrk,       # A/mT/staging
            tc.tile_pool(name="gru", bufs=2) as grup,      # GRU temps
            tc.tile_pool(name="ps", bufs=1, space="PSUM") as ps,
            tc.tile_pool(name="dram", bufs=2, space="DRAM") as dram,
        ):
            # persistent loads
            idx_sb = per.tile([128, SLOT16], dt.int16)
            nc.sync.dma_start(idx_sb[:], idx_in[:])
            counts_sb = per.tile([T, NB], dt.bfloat16)
            nc.sync.dma_start(counts_sb[:], counts_in[:])
            wp_sb = per.tile([128, 2, H], dt.bfloat16)
            nc.sync.dma_start(wp_sb[:], wp_in[:])
            bp_sb = per.tile([128, 2], dt.float32)
            nc.sync.dma_start(bp_sb[:], bp_in[:])
            id_sb = per.tile([128, 128], dt.bfloat16)
            nc.sync.dma_start(id_sb[:], id_in[:])
            hT_sb = per.tile([128, 2, NB], dt.bfloat16)
            outsb = per.tile([128, 2, GPC], dt.float32)
            nc.vector.memset(outsb[:], 0.0)

            # layer-0 gathers read the (node-major, padded) x table directly;
            # dma_gather hangs on kernel-IO tensors, so copy it in-DRAM first
            xtab_d = dram.tile([N, 2 * 128], dt.bfloat16, tag="xtab", bufs=1,
                               name="xtab_d")
            nc.sync.dma_start(xtab_d[:], xtab_in[:])
            # per-layer table + AG staging rings (DRAM), layers 1..L-1
            tbls, agins = [None], [None]
            for l in range(1, L):
                tbls.append(dram.tile([N, H], dt.bfloat16, tag="tbl", bufs=2,
                                      addr_space="Shared", name=f"tbl{l}"))
                agins.append(dram.tile([NB, H], dt.bfloat16, tag="agin",
                                       bufs=2, name=f"agin{l}"))

            def stage_graph(l, q):
                """PE-transpose graph q's h (h-major) to node-major, DMA to
                agin, and fire the half-AllGathers for layer l's table."""
                agin, tbl = agins[l], tbls[l]
                stg = wrk.tile([128, WPG, H], dt.bfloat16, tag="stg", bufs=2)
                for wl in range(WPG):
                    nb = q * MAXN + wl * WIN
                    for hc in range(2):
                        tp = ps.tile([128, 128], dt.bfloat16, tag="agg", bufs=2)
                        nc.tensor.transpose(tp[:], hT_sb[:, hc, nb:nb + WIN],
                                            id_sb[:])
                        nc.scalar.copy(stg[:, wl, hc * 128:(hc + 1) * 128], tp[:])
                nc.sync.dma_start(
                    agin[q * MAXN:(q + 1) * MAXN].rearrange(
                        "(w p) h -> p w h", p=128), stg[:])
                if q == GPC - 1:
                    nc.gpsimd.collective_compute(
                        "AllGather", mybir.AluOpType.bypass,
                        replica_groups=groups,
                        ins=[agin.opt()], outs=[tbl.opt()])

            # ---- input projection: hT = Wp @ xT + bp, stage per graph
            xs_ld = []
            for s in range(NB // 512):
                xs = wrk.tile([128, 2, 512], dt.bfloat16, tag="xs", bufs=3)
                nc.sync.dma_start(xs[:], xT_in[:, :, s * 512:(s + 1) * 512])
                for hm in range(2):
                    pm = ps.tile([128, 512], dt.float32, tag="mT", bufs=2)
                    nc.tensor.matmul(pm[:], wp_sb[:, 0, hm * 128:(hm + 1) * 128],
                                     xs[:, 0, :], start=True, stop=False)
                    nc.tensor.matmul(pm[:], wp_sb[:, 1, hm * 128:(hm + 1) * 128],
                                     xs[:, 1, :], start=False, stop=True)
                    nc.vector.tensor_scalar_add(
                        hT_sb[:, hm, s * 512:(s + 1) * 512],
                        pm[:], bp_sb[:, hm:hm + 1])

            rsums = {}
            for l in range(L):
                tbl = xtab_d if l == 0 else tbls[l]
                # ---- layer weights
                wm_sb = wts.tile([128, 2, T, H], dt.bfloat16, tag="wm")
                nc.sync.dma_start(wm_sb[:], wm_in[l])
                bm_sb = wts.tile([T, H], dt.bfloat16, tag="bm")
                nc.sync.dma_start(bm_sb[:], bm_in[l])
                wih_sb = wts.tile([128, 2, 3 * H], dt.bfloat16, tag="wih")
                nc.sync.dma_start(wih_sb[:], wih_in[l])
                whh_sb = wts.tile([128, 2, 3 * H], dt.bfloat16, tag="whh")
                nc.sync.dma_start(whh_sb[:], whh_in[l])
                brz_sb = wts.tile([128, 4], dt.float32, tag="brz")
                nc.sync.dma_start(brz_sb[:], brz_in[l])
                bin_sb = wts.tile([128, 2], dt.float32, tag="bin")
                nc.sync.dma_start(bin_sb[:], bin_in[l])
                bhn_sb = wts.tile([128, 2], dt.float32, tag="bhn")
                nc.sync.dma_start(bhn_sb[:], bhn_in[l])

                # ---- gather-group streaming
                cglob = 0
                gg_tiles = {}

                def need(c, l=l, tbl=tbl, gg_tiles=gg_tiles):
                    gg = c // 8
                    while len(gg_tiles) == 0 or max(gg_tiles) < gg:
                        g_ = 0 if not gg_tiles else max(gg_tiles) + 1
                        rem = 8 if g_ < ngg - 1 else rem_last
                        Gt = gth.tile([128, rem, H], dt.bfloat16, tag="G",
                                      bufs=GPREF, name=f"G_{l}_{g_}")
                        nc.gpsimd.dma_gather(
                            Gt[:], tbl[:],
                            idx_sb[:, g_ * 64:g_ * 64 + rem * 8],
                            num_idxs=rem * 128, num_idxs_reg=rem * 128,
                            elem_size=H)
                        St = gth.tile([128, 8, WIN], dt.bfloat16, tag="S",
                                      bufs=GPREF, name=f"S_{l}_{g_}")
                        nc.sync.dma_start(St[:], smat_in[g_])
                        gg_tiles[g_] = (Gt, St)
                        if len(gg_tiles) > GPREF - 1:
                            del gg_tiles[min(gg_tiles)]
                    return gg_tiles[gg], c % 8

                # ---- aggregation + message + GRU per half-graph (512 nodes)
                for q in range(GPC):
                    for s2 in range(2):
                        A_sb = wrk.tile([128, T, 2, 4, WIN], dt.bfloat16,
                                        tag="A", bufs=2)
                        for wl in range(4):
                            w = q * WPG + s2 * 4 + wl
                            for th in range(T // 2):
                                pa = ps.tile([128, 512], dt.float32,
                                             tag="agg", bufs=2)
                                for ti in range(2):
                                    t = th * 2 + ti
                                    nchunks = int(budget[w, t])
                                    for hc in range(2):
                                        off = (ti * 2 + hc) * 128
                                        for ci in range(nchunks):
                                            (Gt, St), j = need(cglob + ci)
                                            nc.tensor.matmul(
                                                pa[:, off:off + 128],
                                                Gt[:, j, hc * 128:(hc + 1) * 128],
                                                St[:, j, :],
                                                start=(ci == 0),
                                                stop=(ci == nchunks - 1))
                                    cglob += nchunks
                                dst_ap = A_sb[:, th * 2:th * 2 + 2, :, wl, :]
                                src_ap = pa.rearrange("p (t c k) -> p t c k",
                                                      t=2, c=2)
                                if th % 2 == 0:
                                    nc.scalar.copy(dst_ap, src_ap)
                                else:
                                    nc.vector.tensor_copy(dst_ap, src_ap)

                        # ---- message matmuls for this 512-node slice
                        mT_sb = wrk.tile([128, 2, 512], dt.bfloat16,
                                         tag="mT", bufs=2)
                        nbase = q * MAXN + s2 * 512
                        for hm in range(2):
                            pm = ps.tile([128, 512], dt.float32, tag="mT",
                                         bufs=2)
                            nc.tensor.matmul(
                                pm[:], bm_sb[:, hm * 128:(hm + 1) * 128],
                                counts_sb[:, nbase:nbase + 512],
                                start=True, stop=False)
                            for t in range(T):
                                for hk in range(2):
                                    nc.tensor.matmul(
                                        pm[:],
                                        wm_sb[:, hk, t, hm * 128:(hm + 1) * 128],
                                        A_sb[:, t, hk, :, :],
                                        start=False, stop=(t == T - 1 and hk == 1))
                            nc.vector.tensor_copy(mT_sb[:, hm, :], pm[:])

                        # ---- GRU for these 512 nodes
                        nsl = slice(nbase, nbase + 512)
                        r_sb = grup.tile([128, 2, 512], dt.float32, tag="r",
                                         bufs=2)
                        z_sb = grup.tile([128, 2, 512], dt.float32, tag="z",
                                         bufs=2)
                        for gm in range(4):
                            pg = ps.tile([128, 512], dt.float32, tag="gru",
                                         bufs=3)
                            gsl = slice(gm * 128, (gm + 1) * 128)
                            nc.tensor.matmul(pg[:], wih_sb[:, 0, gsl],
                                             mT_sb[:, 0, :],
                                             start=True, stop=False)
                            nc.tensor.matmul(pg[:], wih_sb[:, 1, gsl],
                                             mT_sb[:, 1, :],
                                             start=False, stop=False)
                            nc.tensor.matmul(pg[:], whh_sb[:, 0, gsl],
                                             hT_sb[:, 0, nsl],
                                             start=False, stop=False)
                            nc.tensor.matmul(pg[:], whh_sb[:, 1, gsl],
                                             hT_sb[:, 1, nsl],
                                             start=False, stop=True)
                            dstg = r_sb[:, gm, :] if gm < 2 else z_sb[:, gm - 2, :]
                            nc.scalar.activation(
                                dstg, pg[:],
                                mybir.ActivationFunctionType.Sigmoid,
                                bias=brz_sb[:, gm:gm + 1])
                        nns, zds = [], []
                        for hc in range(2):
                            gsl = slice((4 + hc) * 128, (5 + hc) * 128)
                            ph = ps.tile([128, 512], dt.float32, tag="gru",
                                         bufs=3)
                            nc.tensor.matmul(ph[:], whh_sb[:, 0, gsl],
                                             hT_sb[:, 0, nsl],
                                             start=True, stop=False)
                            nc.tensor.matmul(ph[:], whh_sb[:, 1, gsl],
                                             hT_sb[:, 1, nsl],
                                             start=False, stop=True)
                            hnb = grup.tile([128, 512], dt.float32, tag="gt",
                                            bufs=3)
                            nc.vector.tensor_scalar_add(hnb[:], ph[:],
                                                        bhn_sb[:, hc:hc + 1])
                            rhn = grup.tile([128, 512], dt.float32, tag="gt",
                                            bufs=3)
                            nc.vector.tensor_mul(rhn[:], r_sb[:, hc, :], hnb[:])
                            pi = ps.tile([128, 512], dt.float32, tag="gru",
                                         bufs=3)
                            nc.tensor.matmul(pi[:], wih_sb[:, 0, gsl],
                                             mT_sb[:, 0, :],
                                             start=True, stop=False)
                            nc.tensor.matmul(pi[:], wih_sb[:, 1, gsl],
                                             mT_sb[:, 1, :],
                                             start=False, stop=True)
                            tsum = grup.tile([128, 512], dt.float32, tag="gt",
                                             bufs=3)
                            nc.vector.tensor_add(tsum[:], pi[:], rhn[:])
                            nn = grup.tile([128, 512], dt.float32, tag="nnb",
                                           bufs=2)
                            nc.scalar.activation(
                                nn[:], tsum[:],
                                mybir.ActivationFunctionType.Tanh,
                                bias=bin_sb[:, hc:hc + 1])
                            hprev = grup.tile([128, 512], dt.float32, tag="gt",
                                              bufs=3)
                            nc.vector.tensor_copy(hprev[:], hT_sb[:, hc, nsl])
                            d_ = grup.tile([128, 512], dt.float32, tag="gt",
                                           bufs=3)
                            nc.vector.tensor_sub(d_[:], hprev[:], nn[:])
                            zd = grup.tile([128, 512], dt.float32, tag="zdb",
                                           bufs=2)
                            nc.vector.tensor_mul(zd[:], z_sb[:, hc, :], d_[:])
                            nns.append(nn)
                            zds.append(zd)
                        for hc in range(2):
                            if l < L - 1:
                                nc.vector.tensor_add(hT_sb[:, hc, nsl],
                                                     nns[hc][:], zds[hc][:])
                            else:
                                hf = grup.tile([128, 512], dt.float32,
                                               tag="hf", bufs=2)
                                nc.vector.tensor_add(hf[:], nns[hc][:], zds[hc][:])
                                rs = grup.tile([128, 1], dt.float32, tag="rs",
                                               bufs=16)
                                nc.vector.tensor_reduce(
                                    rs[:], hf[:], axis=mybir.AxisListType.X,
                                    op=mybir.AluOpType.add)
                                rsums[(q, hc, s2)] = rs
                    # graph q's h fully updated -> stage for next layer's table
                    if l < L - 1:
                        stage_graph(l + 1, q)
                assert cglob == ctot, (cglob, ctot)

            # ---- readout
            for q in range(GPC):
                for hc in range(2):
                    nc.vector.tensor_add(outsb[:, hc, q:q + 1],
                                         rsums[(q, hc, 0)][:],
                                         rsums[(q, hc, 1)][:])
            nc.sync.dma_start(out_t.rearrange("c p g -> p c g"), outsb[:])

    nc.compile()
    return nc


def kernel(**inputs):
    meta, in_maps = _prep(**inputs)
    nc = _build(meta)
    res = run_bass_kernel_spmd(nc, in_maps, core_ids=list(range(NCORES)))
    GPC = meta["GPC"]
    out = np.zeros((meta["B"], H), np.float32)
    for c in range(NCORES):
        ot = res.results[c]["outT"]          # [2, 128, GPC]
        for g in range(GPC):
            out[c * GPC + g] = np.concatenate([ot[0, :, g], ot[1, :, g]])
    return out


# revision 30
# speedup vs baseline: 1.4340x; 1.0156x over previous
"""BatchGGNNEncoder Trainium2 kernel: 8-core SPMD, dst-sharded message passing.

Full inputs in, full output out. Internally:
  - core c owns nodes [c*4096, (c+1)*4096) = graphs [4c, 4c+4) (data parallel).
  - aggregate-first GGNN layer:
        A_t[v] = sum_{e: dst=v, type=t} h[src_e]         (one-hot matmuls, PSUM)
        m      = sum_t A_t @ Wm[t].T + counts_t * bm[t]  (dense matmuls)
        h      = GRU(m, h)                               (matmuls + DVE/ACT)
  - h table (bf16, node-major) lives in DRAM; rebuilt per layer via TWO
    AllGathers (first half fires mid-layer to hide latency); per-edge h[src]
    rows fetched with dma_gather (the Q7 descriptor-emission stream is the
    critical path: ~8.5us per 1024 rows, so everything else hides under it).
  - nodes are permuted within each graph to balance (type, 128-dst-window)
    group sizes so the compiled program structure is identical on all 8 cores.
"""
import numpy as np
import ml_dtypes

import concourse.bass as bass
import concourse.bacc as bacc
import concourse.mybir as mybir
import concourse.tile as tile
from concourse.bass_utils import run_bass_kernel_spmd

BF16 = ml_dtypes.bfloat16

# problem constants (hardcoded per harness contract)
MAXN, F, H, T, L = 1024, 215, 256, 8, 3
NCORES = 8
WIN = 128                     # dst window (one-hot free width)
WPG = MAXN // WIN             # 8 windows per graph
GPREF = 6                     # gather groups in flight
KPRE = 6                      # groups batch-prepped during each AG wait


def _balance_graph(deg):
    """Assign 1024 nodes (deg: [1024, T] type-degrees) to 8 windows of 128.
    Window 7 takes the heaviest 128 nodes (cap 381/type); the rest fill
    windows 0..6 under a 256/type cap (2 chunks). A repair pass swaps nodes
    to clear residual over-cap windows, then windows are reordered so any
    remaining 3-chunk windows sit at the highest positions — aligning the
    cross-core max that sets the budget."""
    tot = deg.sum(1)
    order = np.argsort(-tot, kind="stable")
    wsum = np.zeros((WPG, T), np.float64)
    wcnt = np.zeros(WPG, np.int64)
    members = [[] for _ in range(WPG)]
    CAP, CAP7 = 256.0, 381.0
    rest = []
    for nd in order:
        if wcnt[7] < 128 and ((wsum[7] + deg[nd]) <= CAP7).all():
            members[7].append(nd)
            wsum[7] += deg[nd]
            wcnt[7] += 1
        else:
            rest.append(nd)
    for nd in rest:
        d = deg[nd]
        ns = wsum[:7] + d
        feas = (wcnt[:7] < 128) & (ns <= CAP).all(axis=1)
        if feas.any():
            load = np.where(feas, ns.max(axis=1), np.inf)
            best = int(np.argmin(load))
        else:
            nsall = wsum + d
            dcost = (np.ceil(nsall / 128) - np.ceil(wsum / 128)).sum(axis=1)
            dcost[wcnt >= 128] = np.inf
            best = int(np.argmin(dcost))
        members[best].append(nd)
        wsum[best] += d
        wcnt[best] += 1

    # repair: swap nodes to pull windows 0..6 under 256 per type (w7 < 384)
    wof = [np.array(m, np.int64) for m in members]
    dg = [deg[m] for m in wof]
    cap = np.full(WPG, CAP)
    cap[7] = 383.0
    for _ in range(400):
        ws = np.stack([d.sum(0) for d in dg])
        over = ws - cap[:, None]
        if (over <= 0).all():
            break
        w, t = np.unravel_index(np.argmax(over), over.shape)
        done = False
        for w2 in np.argsort(ws[:, t]):
            if w2 == w:
                continue
            da, db = dg[w], dg[w2]
            gain = da[:, t][:, None] - db[:, t][None, :]   # [na, nb]
            nsw = ws[w][None, None, :] - da[:, None, :] + db[None, :, :]
            nsw2 = ws[w2][None, None, :] + da[:, None, :] - db[None, :, :]
            ok = ((nsw <= np.maximum(ws[w], cap[w])[None, None, :]).all(2)
                  & (nsw2 <= cap[w2][None, None]).all(0 * 0 + 2)
                  & (gain > 0))
            if ok.any():
                a, b = np.unravel_index(
                    np.argmax(np.where(ok, gain, -1)), gain.shape)
                wof[w][a], wof[w2][b] = wof[w2][b], wof[w][a]
                dg[w] = deg[wof[w]]
                dg[w2] = deg[wof[w2]]
                done = True
                break
        if not done:
            break
    # reorder windows: fewest chunks first, heavy windows at high positions
    chunks = np.array([np.ceil(d.sum(0) / 128).sum() for d in dg])
    perm = np.argsort(chunks, kind="stable")
    return [wof[p] for p in perm]


def _prep(node_features, edge_index, edge_type, Wp, bp, Wm, bm, Wih, Whh, bih, bhh):
    """Host-side sharding/packing. Returns (meta, in_maps)."""
    x = np.asarray(node_features, np.float32)
    B = x.shape[0]
    N = B * MAXN
    GPC = B // NCORES             # graphs per core
    NB = GPC * MAXN               # nodes per core
    NWIN = GPC * WPG              # windows per core
    HALF = NB // 2                # nodes per AG half (2 graphs)
    src = np.asarray(edge_index[0]).astype(np.int64)
    dst = np.asarray(edge_index[1]).astype(np.int64)
    et = np.asarray(edge_type).astype(np.int64)

    # per-(node, type) in-degree
    cnt = np.zeros((N, T), np.int64)
    np.add.at(cnt, (dst, et), 1)

    # balance windows within each graph -> node permutation
    old2new = np.empty(N, np.int64)
    for g in range(B):
        mem = _balance_graph(cnt[g * MAXN:(g + 1) * MAXN])
        for w in range(WPG):
            pos = g * MAXN + w * WIN + np.arange(WIN)
            old2new[g * MAXN + mem[w]] = pos
    new2old = np.argsort(old2new)

    src_n = old2new[src]
    dst_n = old2new[dst]

    src_row = src_n                # table row = global node id (single AG)

    # group edges per core: key = ((gslot*WPG + w)*T + t)
    core = dst_n // NB
    rel = dst_n % NB
    col = rel % WIN
    key = (rel // WIN) * T + et
    NGRP = NWIN * T

    gsizes = np.zeros((NCORES, NGRP), np.int64)
    for c in range(NCORES):
        m = core == c
        gsizes[c] = np.bincount(key[m], minlength=NGRP)
    budget = np.ceil(gsizes.max(axis=0) / 128).astype(np.int64)  # chunks/group
    budget = np.maximum(budget, 1)
    ctot = int(budget.sum())
    ngg = (ctot + 7) // 8          # gather groups of <=8 chunks
    rem_last = ctot - 8 * (ngg - 1)
    nslots = ctot * 128
    gbase = np.concatenate([[0], np.cumsum(budget)])[:-1] * 128  # slot base

    # per-core slot arrays
    idx_maps, smat_maps = [], []
    counts_maps, xT_maps = [], []
    for c in range(NCORES):
        m = core == c
        kc, cc, sc = key[m], col[m], src_row[m]
        order = np.argsort(kc, kind="stable")
        kc, cc, sc = kc[order], cc[order], sc[order]
        grp_start = np.searchsorted(kc, np.arange(NGRP), side="left")
        rank = np.arange(kc.size) - grp_start[kc]
        slot = gbase[kc] + rank
        src16 = np.zeros(nslots, np.int16)
        scol = np.full(nslots, -1, np.int64)
        src16[slot] = sc.astype(np.int16)
        scol[slot] = cc
        # idx: wrapped [16, nslots/16] replicated to 128 partitions
        idx = np.tile(src16.reshape(nslots // 16, 16).T, (8, 1)).copy()
        idx_maps.append(idx)
        # one-hot S: [ngg, 128, 8, WIN] bf16 (last group zero-padded)
        smat = np.zeros((ngg * 8 * 128, WIN), BF16)
        valid = scol >= 0
        smat[np.nonzero(valid)[0], scol[valid]] = 1
        smat = smat.reshape(ngg, 8, 128, WIN)
        smat = np.ascontiguousarray(smat.transpose(0, 2, 1, 3))  # [ngg,128,8,WIN]
        smat_maps.append(smat)
        # counts (new order), [T, NB] bf16
        cslice = cnt[new2old[c * NB:(c + 1) * NB]]
        counts_maps.append(np.ascontiguousarray(cslice.T).astype(BF16))
        # xT [128, 2, NB] bf16: [p, k, node] = x[node, k*128+p]
        xs = x.reshape(N, F)[new2old[c * NB:(c + 1) * NB]]
        xp = np.zeros((NB, 2 * 128), np.float32)
        xp[:, :F] = xs
        xT = np.ascontiguousarray(xp.reshape(NB, 2, 128).transpose(2, 1, 0))
        xT_maps.append(xT.astype(BF16))

    # node-major padded x shards: layer 0 aggregates x directly (aggregation
    # is linear in h0 = Wp x + bp, so Wp folds into Wm[0] and bp into bm[0]
    # on the host); the device AllGathers the shards into the layer-0 table
    xtab = np.zeros((N, 2 * 128), np.float32)
    xtab[:, :F] = x.reshape(N, F)[new2old]
    xtab = xtab.astype(BF16)

    # weights (shared across cores)
    Wp = np.asarray(Wp, np.float32); bp_ = np.asarray(bp, np.float32)
    Wm_ = np.asarray(Wm, np.float32); bm_ = np.asarray(bm, np.float32)
    Wih_ = np.asarray(Wih, np.float32); Whh_ = np.asarray(Whh, np.float32)
    bih_ = np.asarray(bih, np.float32); bhh_ = np.asarray(bhh, np.float32)

    wpT = np.zeros((128, 2, H), np.float32)          # [p, fk, h']
    wpt = Wp.T                                       # [F, H]
    wpT[:, 0, :] = wpt[0:128]
    wpT[:F - 128, 1, :] = wpt[128:F]
    wp_in = wpT.astype(BF16)
    bp_in = np.ascontiguousarray(bp_.reshape(2, 128).T)          # [128, 2]

    wm_in = np.ascontiguousarray(                     # [L, 128, 2, T, H]
        Wm_.transpose(0, 1, 3, 2)
        .reshape(L, T, 2, 128, H).transpose(0, 3, 2, 1, 4)).astype(BF16)
    bm_in = bm_.astype(BF16).copy()                   # [L, T, H]
    # layer 0 in x-space: Wt[t] = Wm[0,t] @ Wp (pad in-dim F->256),
    # bm~[0,t] = bm[0,t] + Wm[0,t] @ bp
    wt0 = np.einsum("toh,hf->tfo", Wm_[0], Wp)        # [T, F, H]
    wt0p = np.zeros((T, 256, H), np.float32)
    wt0p[:, :F, :] = wt0
    wm_in[0] = np.ascontiguousarray(
        wt0p.reshape(T, 2, 128, H).transpose(2, 1, 0, 3)).astype(BF16)
    bm_in[0] = (bm_[0] + np.einsum("toh,h->to", Wm_[0], bp_)).astype(BF16)
    wih_in = np.ascontiguousarray(                    # [L, 128, 2, 3H]
        Wih_.transpose(0, 2, 1).reshape(L, 2, 128, 3 * H).transpose(0, 2, 1, 3)
    ).astype(BF16)
    whh_in = np.ascontiguousarray(
        Whh_.transpose(0, 2, 1).reshape(L, 2, 128, 3 * H).transpose(0, 2, 1, 3)
    ).astype(BF16)
    brz = bih_[:, :2 * H] + bhh_[:, :2 * H]
    brz_in = np.ascontiguousarray(brz.reshape(L, 4, 128).transpose(0, 2, 1))
    bin_in = np.ascontiguousarray(bih_[:, 2 * H:].reshape(L, 2, 128).transpose(0, 2, 1))
    bhn_in = np.ascontiguousarray(bhh_[:, 2 * H:].reshape(L, 2, 128).transpose(0, 2, 1))
    id128 = np.eye(128, dtype=BF16)

    in_maps = []
    for c in range(NCORES):
        in_maps.append({
            "xT": xT_maps[c], "xshard": xtab[c * NB:(c + 1) * NB],
            "idx": idx_maps[c],
            "smat": smat_maps[c], "countsT": counts_maps[c],
            "wpT": wp_in, "bp": bp_in, "wmT": wm_in, "bmT": bm_in,
            "wihT": wih_in, "whhT": whh_in,
            "brz": brz_in, "bin_": bin_in, "bhn": bhn_in, "id128": id128,
        })
    meta = dict(B=B, N=N, GPC=GPC, NB=NB, NWIN=NWIN, HALF=HALF,
                budget=budget.reshape(NWIN, T), ctot=ctot, ngg=ngg,
                rem_last=rem_last, new2old=new2old)
    return meta, in_maps


def _build(meta):
    """Build the SPMD Bass program (identical across cores)."""
    dt = mybir.dt
    N, NB, GPC, NWIN = meta["N"], meta["NB"], meta["GPC"], meta["NWIN"]
    HALF = meta["HALF"]
    budget, ngg, ctot = meta["budget"], meta["ngg"], meta["ctot"]
    rem_last = meta["rem_last"]
    SLOT16 = ctot * 8

    nc = bacc.Bacc("TRN2", target_bir_lowering=False, debug=False,
                   enable_asserts=False, num_devices=NCORES)

    # ---- I/O
    xT_in = nc.dram_tensor("xT", [128, 2, NB], dt.bfloat16, kind="ExternalInput").ap()
    xshard_in = nc.dram_tensor("xshard", [NB, 2 * 128], dt.bfloat16, kind="ExternalInput").ap()
    idx_in = nc.dram_tensor("idx", [128, SLOT16], dt.int16, kind="ExternalInput").ap()
    smat_in = nc.dram_tensor("smat", [ngg, 128, 8, WIN], dt.bfloat16, kind="ExternalInput").ap()
    counts_in = nc.dram_tensor("countsT", [T, NB], dt.bfloat16, kind="ExternalInput").ap()
    wp_in = nc.dram_tensor("wpT", [128, 2, H], dt.bfloat16, kind="ExternalInput").ap()
    bp_in = nc.dram_tensor("bp", [128, 2], dt.float32, kind="ExternalInput").ap()
    wm_in = nc.dram_tensor("wmT", [L, 128, 2, T, H], dt.bfloat16, kind="ExternalInput").ap()
    bm_in = nc.dram_tensor("bmT", [L, T, H], dt.bfloat16, kind="ExternalInput").ap()
    wih_in = nc.dram_tensor("wihT", [L, 128, 2, 3 * H], dt.bfloat16, kind="ExternalInput").ap()
    whh_in = nc.dram_tensor("whhT", [L, 128, 2, 3 * H], dt.bfloat16, kind="ExternalInput").ap()
    brz_in = nc.dram_tensor("brz", [L, 128, 4], dt.float32, kind="ExternalInput").ap()
    bin_in = nc.dram_tensor("bin_", [L, 128, 2], dt.float32, kind="ExternalInput").ap()
    bhn_in = nc.dram_tensor("bhn", [L, 128, 2], dt.float32, kind="ExternalInput").ap()
    id_in = nc.dram_tensor("id128", [128, 128], dt.bfloat16, kind="ExternalInput").ap()
    out_t = nc.dram_tensor("outT", [2, 128, GPC], dt.float32, kind="ExternalOutput").ap()

    groups = [list(range(NCORES))]

    with tile.TileContext(nc) as tc:
        with (
            tc.tile_pool(name="per", bufs=1) as per,       # persistent SBUF
            tc.tile_pool(name="wts", bufs=2) as wts,       # per-layer weights
            tc.tile_pool(name="gth", bufs=GPREF) as gth,   # gather/S stream
            tc.tile_pool(name="wrk", bufs=2) as wrk,       # A/mT/staging
            tc.tile_pool(name="gru", bufs=2) as grup,      # GRU temps
            tc.tile_pool(name="ps", bufs=1, space="PSUM") as ps,
            tc.tile_pool(name="dram", bufs=2, space="DRAM") as dram,
        ):
            # persistent loads
            idx_sb = per.tile([128, SLOT16], dt.int16)
            nc.sync.dma_start(idx_sb[:], idx_in[:])
            counts_sb = per.tile([T, NB], dt.bfloat16)
            nc.sync.dma_start(counts_sb[:], counts_in[:])
            wp_sb = per.tile([128, 2, H], dt.bfloat16)
            nc.sync.dma_start(wp_sb[:], wp_in[:])
            bp_sb = per.tile([128, 2], dt.float32)
            nc.sync.dma_start(bp_sb[:], bp_in[:])
            id_sb = per.tile([128, 128], dt.bfloat16)
            nc.sync.dma_start(id_sb[:], id_in[:])
            hT_sb = per.tile([128, 2, NB], dt.bfloat16)
            outsb = per.tile([128, 2, GPC], dt.float32)
            nc.vector.memset(outsb[:], 0.0)

            # layer-0 gathers read the (node-major, padded) x table, built by
            # AllGathering each core's x shard (cheaper than a 16MB in-DRAM
            # copy of a replicated input; collectives can't read kernel IO,
            # so bounce the shard through a DRAM tile first)
            agin0 = dram.tile([NB, 2 * 128], dt.bfloat16, tag="agin0", bufs=1,
                              name="agin0")
            nc.sync.dma_start(agin0[:], xshard_in[:])
            xtab_d = dram.tile([N, 2 * 128], dt.bfloat16, tag="xtab", bufs=1,
                               addr_space="Shared", name="xtab_d")
            nc.gpsimd.collective_compute(
                "AllGather", mybir.AluOpType.bypass, replica_groups=groups,
                ins=[agin0.opt()], outs=[xtab_d.opt()])
            # per-layer table + AG staging rings (DRAM), layers 1..L-1
            tbls, agins = [None], [None]
            for l in range(1, L):
                tbls.append(dram.tile([N, H], dt.bfloat16, tag="tbl", bufs=2,
                                      addr_space="Shared", name=f"tbl{l}"))
                agins.append(dram.tile([NB, H], dt.bfloat16, tag="agin",
                                       bufs=2, name=f"agin{l}"))

            def stage_graph(l, q):
                """PE-transpose graph q's h (h-major) to node-major, DMA to
                agin, and fire the half-AllGathers for layer l's table."""
                agin, tbl = agins[l], tbls[l]
                stg = wrk.tile([128, WPG, H], dt.bfloat16, tag="stg", bufs=2)
                for wl in range(WPG):
                    nb = q * MAXN + wl * WIN
                    for hc in range(2):
                        tp = ps.tile([128, 128], dt.bfloat16, tag="agg", bufs=2)
                        nc.tensor.transpose(tp[:], hT_sb[:, hc, nb:nb + WIN],
                                            id_sb[:])
                        nc.scalar.copy(stg[:, wl, hc * 128:(hc + 1) * 128], tp[:])
                nc.sync.dma_start(
                    agin[q * MAXN:(q + 1) * MAXN].rearrange(
                        "(w p) h -> p w h", p=128), stg[:])
                if q == GPC - 1:
                    nc.gpsimd.collective_compute(
                        "AllGather", mybir.AluOpType.bypass,
                        replica_groups=groups,
                        ins=[agin.opt()], outs=[tbl.opt()])

            # ---- input projection: hT = Wp @ xT + bp, stage per graph
            xs_ld = []
            for s in range(NB // 512):
                xs = wrk.tile([128, 2, 512], dt.bfloat16, tag="xs", bufs=3)
                nc.sync.dma_start(xs[:], xT_in[:, :, s * 512:(s + 1) * 512])
                for hm in range(2):
                    pm = ps.tile([128, 512], dt.float32, tag="mT", bufs=2)
                    nc.tensor.matmul(pm[:], wp_sb[:, 0, hm * 128:(hm + 1) * 128],
                                     xs[:, 0, :], start=True, stop=False)
                    nc.tensor.matmul(pm[:], wp_sb[:, 1, hm * 128:(hm + 1) * 128],
                                     xs[:, 1, :], start=False, stop=True)
                    nc.vector.tensor_scalar_add(
                        hT_sb[:, hm, s * 512:(s + 1) * 512],
                        pm[:], bp_sb[:, hm:hm + 1])

            rsums = {}
            for l in range(L):
                tbl = xtab_d if l == 0 else tbls[l]
                # ---- layer weights
                wm_sb = wts.tile([128, 2, T, H], dt.bfloat16, tag="wm")
                nc.sync.dma_start(wm_sb[:], wm_in[l])
                bm_sb = wts.tile([T, H], dt.bfloat16, tag="bm")
                nc.sync.dma_start(bm_sb[:], bm_in[l])
                wih_sb = wts.tile([128, 2, 3 * H], dt.bfloat16, tag="wih")
                nc.sync.dma_start(wih_sb[:], wih_in[l])
                whh_sb = wts.tile([128, 2, 3 * H], dt.bfloat16, tag="whh")
                nc.sync.dma_start(whh_sb[:], whh_in[l])
                brz_sb = wts.tile([128, 4], dt.float32, tag="brz")
                nc.sync.dma_start(brz_sb[:], brz_in[l])
                bin_sb = wts.tile([128, 2], dt.float32, tag="bin")
                nc.sync.dma_start(bin_sb[:], bin_in[l])
                bhn_sb = wts.tile([128, 2], dt.float32, tag="bhn")
                nc.sync.dma_start(bhn_sb[:], bhn_in[l])

                # ---- gather-group streaming
                cglob = 0
                gg_tiles = {}

                def need(c, l=l, tbl=tbl, gg_tiles=gg_tiles):
                    gg = c // 8
                    while len(gg_tiles) == 0 or max(gg_tiles) < gg:
                        g_ = 0 if not gg_tiles else max(gg_tiles) + 1
                        rem = 8 if g_ < ngg - 1 else rem_last
                        Gt = gth.tile([128, rem, H], dt.bfloat16, tag="G",
                                      bufs=GPREF, name=f"G_{l}_{g_}")
                        nc.gpsimd.dma_gather(
                            Gt[:], tbl[:],
                            idx_sb[:, g_ * 64:g_ * 64 + rem * 8],
                            num_idxs=rem * 128, num_idxs_reg=rem * 128,
                            elem_size=H)
                        St = gth.tile([128, 8, WIN], dt.bfloat16, tag="S",
                                      bufs=GPREF, name=f"S_{l}_{g_}")
                        nc.sync.dma_start(St[:], smat_in[g_])
                        gg_tiles[g_] = (Gt, St)
                        if len(gg_tiles) > GPREF:
                            del gg_tiles[min(gg_tiles)]
                    return gg_tiles[gg], c % 8

                # ---- aggregation + message + GRU per half-graph (512 nodes)
                for q in range(GPC):
                    for s2 in range(2):
                        A_sb = wrk.tile([128, T, 2, 4, WIN], dt.bfloat16,
                                        tag="A", bufs=2)
                        for wl in range(4):
                            w = q * WPG + s2 * 4 + wl
                            for th in range(T // 2):
                                pa = ps.tile([128, 512], dt.float32,
                                             tag="agg", bufs=2)
                                for ti in range(2):
                                    t = th * 2 + ti
                                    nchunks = int(budget[w, t])
                                    for hc in range(2):
                                        off = (ti * 2 + hc) * 128
                                        for ci in range(nchunks):
                                            (Gt, St), j = need(cglob + ci)
                                            nc.tensor.matmul(
                                                pa[:, off:off + 128],
                                                Gt[:, j, hc * 128:(hc + 1) * 128],
                                                St[:, j, :],
                                                start=(ci == 0),
                                                stop=(ci == nchunks - 1))
                                    cglob += nchunks
                                dst_ap = A_sb[:, th * 2:th * 2 + 2, :, wl, :]
                                src_ap = pa.rearrange("p (t c k) -> p t c k",
                                                      t=2, c=2)
                                if th % 2 == 0:
                                    nc.scalar.copy(dst_ap, src_ap)
                                else:
                                    nc.vector.tensor_copy(dst_ap, src_ap)

                        # ---- message matmuls for this 512-node slice
                        mT_sb = wrk.tile([128, 2, 512], dt.bfloat16,
                                         tag="mT", bufs=2)
                        nbase = q * MAXN + s2 * 512
                        for hm in range(2):
                            pm = ps.tile([128, 512], dt.float32, tag="mT",
                                         bufs=2)
                            nc.tensor.matmul(
                                pm[:], bm_sb[:, hm * 128:(hm + 1) * 128],
                                counts_sb[:, nbase:nbase + 512],
                                start=True, stop=False)
                            for t in range(T):
                                for hk in range(2):
                                    nc.tensor.matmul(
                                        pm[:],
                                        wm_sb[:, hk, t, hm * 128:(hm + 1) * 128],
                                        A_sb[:, t, hk, :, :],
                                        start=False, stop=(t == T - 1 and hk == 1))
                            nc.vector.tensor_copy(mT_sb[:, hm, :], pm[:])

                        # ---- GRU for these 512 nodes
                        nsl = slice(nbase, nbase + 512)
                        r_sb = grup.tile([128, 2, 512], dt.float32, tag="r",
                                         bufs=2)
                        z_sb = grup.tile([128, 2, 512], dt.float32, tag="z",
                                         bufs=2)
                        for gm in range(4):
                            pg = ps.tile([128, 512], dt.float32, tag="gru",
                                         bufs=3)
                            gsl = slice(gm * 128, (gm + 1) * 128)
                            nc.tensor.matmul(pg[:], wih_sb[:, 0, gsl],
                                             mT_sb[:, 0, :],
                                             start=True, stop=False)
                            nc.tensor.matmul(pg[:], wih_sb[:, 1, gsl],
                                             mT_sb[:, 1, :],
                                             start=False, stop=False)
                            nc.tensor.matmul(pg[:], whh_sb[:, 0, gsl],
                                             hT_sb[:, 0, nsl],
                                             start=False, stop=False)
                            nc.tensor.matmul(pg[:], whh_sb[:, 1, gsl],
                                             hT_sb[:, 1, nsl],
                                             start=False, stop=True)
                            dstg = r_sb[:, gm, :] if gm < 2 else z_sb[:, gm - 2, :]
                            nc.scalar.activation(
                                dstg, pg[:],
                                mybir.ActivationFunctionType.Sigmoid,
                                bias=brz_sb[:, gm:gm + 1])
                        nns, zds = [], []
                        for hc in range(2):
                            gsl = slice((4 + hc) * 128, (5 + hc) * 128)
                            ph = ps.tile([128, 512], dt.float32, tag="gru",
                                         bufs=3)
                            nc.tensor.matmul(ph[:], whh_sb[:, 0, gsl],
                                             hT_sb[:, 0, nsl],
                                             start=True, stop=False)
                            nc.tensor.matmul(ph[:], whh_sb[:, 1, gsl],
                                             hT_sb[:, 1, nsl],
                                             start=False, stop=True)
                            hnb = grup.tile([128, 512], dt.float32, tag="gt",
                                            bufs=3)
                            nc.vector.tensor_scalar_add(hnb[:], ph[:],
                                                        bhn_sb[:, hc:hc + 1])
                            rhn = grup.tile([128, 512], dt.float32, tag="gt",
                                            bufs=3)
                            nc.vector.tensor_mul(rhn[:], r_sb[:, hc, :], hnb[:])
                            pi = ps.tile([128, 512], dt.float32, tag="gru",
                                         bufs=3)
                            nc.tensor.matmul(pi[:], wih_sb[:, 0, gsl],
                                             mT_sb[:, 0, :],
                                             start=True, stop=False)
                            nc.tensor.matmul(pi[:], wih_sb[:, 1, gsl],
                                             mT_sb[:, 1, :],
                                             start=False, stop=True)
                            tsum = grup.tile([128, 512], dt.float32, tag="gt",
                                             bufs=3)
                            nc.vector.tensor_add(tsum[:], pi[:], rhn[:])
                            nn = grup.tile([128, 512], dt.float32, tag="nnb",
                                           bufs=2)
                            nc.scalar.activation(
                                nn[:], tsum[:],
                                mybir.ActivationFunctionType.Tanh,
                                bias=bin_sb[:, hc:hc + 1])
                            hprev = grup.tile([128, 512], dt.float32, tag="gt",
                                              bufs=3)
                            nc.vector.tensor_copy(hprev[:], hT_sb[:, hc, nsl])
                            d_ = grup.tile([128, 512], dt.float32, tag="gt",
                                           bufs=3)
                            nc.vector.tensor_sub(d_[:], hprev[:], nn[:])
                            zd = grup.tile([128, 512], dt.float32, tag="zdb",
                                           bufs=2)
                            nc.vector.tensor_mul(zd[:], z_sb[:, hc, :], d_[:])
                            nns.append(nn)
                            zds.append(zd)
                        for hc in range(2):
                            if l < L - 1:
                                nc.vector.tensor_add(hT_sb[:, hc, nsl],
                                                     nns[hc][:], zds[hc][:])
                            else:
                                hf = grup.tile([128, 512], dt.float32,
                                               tag="hf", bufs=2)
                                nc.vector.tensor_add(hf[:], nns[hc][:], zds[hc][:])
                                rs = grup.tile([128, 1], dt.float32, tag="rs",
                                               bufs=16)
                                nc.vector.tensor_reduce(
                                    rs[:], hf[:], axis=mybir.AxisListType.X,
                                    op=mybir.AluOpType.add)
                                rsums[(q, hc, s2)] = rs
                    # graph q's h fully updated -> stage for next layer's table
                    if l < L - 1:
                        stage_graph(l + 1, q)
                assert cglob == ctot, (cglob, ctot)

            # ---- readout
            for q in range(GPC):
                for hc in range(2):
                    nc.vector.tensor_add(outsb[:, hc, q:q + 1],
                                         rsums[(q, hc, 0)][:],
                                         rsums[(q, hc, 1)][:])
            nc.sync.dma_start(out_t.rearrange("c p g -> p c g"), outsb[:])

    nc.compile()
    return nc


def kernel(**inputs):
    meta, in_maps = _prep(**inputs)
    nc = _build(meta)
    res = run_bass_kernel_spmd(nc, in_maps, core_ids=list(range(NCORES)))
    GPC = meta["GPC"]
    out = np.zeros((meta["B"], H), np.float32)
    for c in range(NCORES):
        ot = res.results[c]["outT"]          # [2, 128, GPC]
        for g in range(GPC):
            out[c * GPC + g] = np.concatenate([ot[0, :, g], ot[1, :, g]])
    return out


# revision 34
# speedup vs baseline: 2.1750x; 1.5167x over previous
"""BatchGGNNEncoder Trainium2 kernel: 8-core SPMD, dst-sharded message passing.

Full inputs in, full output out. Internally:
  - core c owns nodes [c*4096, (c+1)*4096) = graphs [4c, 4c+4) (data parallel).
  - aggregate-first GGNN layer:
        A_t[v] = sum_{e: dst=v, type=t} h[src_e]         (one-hot matmuls, PSUM)
        m      = sum_t A_t @ Wm[t].T + counts_t * bm[t]  (dense matmuls)
        h      = GRU(m, h)                               (matmuls + DVE/ACT)
  - h table (bf16, node-major) lives in DRAM; rebuilt per layer via TWO
    AllGathers (first half fires mid-layer to hide latency); per-edge h[src]
    rows fetched with dma_gather (the Q7 descriptor-emission stream is the
    critical path: ~8.5us per 1024 rows, so everything else hides under it).
  - nodes are permuted within each graph to balance (type, 128-dst-window)
    group sizes so the compiled program structure is identical on all 8 cores.
"""
import numpy as np
import ml_dtypes

import concourse.bass as bass
import concourse.bacc as bacc
import concourse.mybir as mybir
import concourse.tile as tile
from concourse.bass_utils import run_bass_kernel_spmd

BF16 = ml_dtypes.bfloat16

# problem constants (hardcoded per harness contract)
MAXN, F, H, T, L = 1024, 215, 256, 8, 3
NCORES = 8
WIN = 128                     # dst window (one-hot free width)
WPG = MAXN // WIN             # 8 windows per graph
GPREF = 6                     # gather groups in flight
KPRE = 6                      # groups batch-prepped during each AG wait


def _balance_graph(deg):
    """Assign 1024 nodes (deg: [1024, T] type-degrees) to 8 windows of 128.
    Window 7 takes the heaviest 128 nodes (cap 381/type); the rest fill
    windows 0..6 under a 256/type cap (2 chunks). A repair pass swaps nodes
    to clear residual over-cap windows, then windows are reordered so any
    remaining 3-chunk windows sit at the highest positions — aligning the
    cross-core max that sets the budget."""
    tot = deg.sum(1)
    order = np.argsort(-tot, kind="stable")
    wsum = np.zeros((WPG, T), np.float64)
    wcnt = np.zeros(WPG, np.int64)
    members = [[] for _ in range(WPG)]
    CAP, CAP7 = 256.0, 381.0
    rest = []
    for nd in order:
        if wcnt[7] < 128 and ((wsum[7] + deg[nd]) <= CAP7).all():
            members[7].append(nd)
            wsum[7] += deg[nd]
            wcnt[7] += 1
        else:
            rest.append(nd)
    for nd in rest:
        d = deg[nd]
        ns = wsum[:7] + d
        feas = (wcnt[:7] < 128) & (ns <= CAP).all(axis=1)
        if feas.any():
            load = np.where(feas, ns.max(axis=1), np.inf)
            best = int(np.argmin(load))
        else:
            nsall = wsum + d
            dcost = (np.ceil(nsall / 128) - np.ceil(wsum / 128)).sum(axis=1)
            dcost[wcnt >= 128] = np.inf
            best = int(np.argmin(dcost))
        members[best].append(nd)
        wsum[best] += d
        wcnt[best] += 1

    # repair: swap nodes to pull windows 0..6 under 256 per type (w7 < 384)
    wof = [np.array(m, np.int64) for m in members]
    dg = [deg[m] for m in wof]
    cap = np.full(WPG, CAP)
    cap[7] = 383.0
    for _ in range(400):
        ws = np.stack([d.sum(0) for d in dg])
        over = ws - cap[:, None]
        if (over <= 0).all():
            break
        w, t = np.unravel_index(np.argmax(over), over.shape)
        done = False
        for w2 in np.argsort(ws[:, t]):
            if w2 == w:
                continue
            da, db = dg[w], dg[w2]
            gain = da[:, t][:, None] - db[:, t][None, :]   # [na, nb]
            nsw = ws[w][None, None, :] - da[:, None, :] + db[None, :, :]
            nsw2 = ws[w2][None, None, :] + da[:, None, :] - db[None, :, :]
            ok = ((nsw <= np.maximum(ws[w], cap[w])[None, None, :]).all(2)
                  & (nsw2 <= cap[w2][None, None]).all(0 * 0 + 2)
                  & (gain > 0))
            if ok.any():
                a, b = np.unravel_index(
                    np.argmax(np.where(ok, gain, -1)), gain.shape)
                wof[w][a], wof[w2][b] = wof[w2][b], wof[w][a]
                dg[w] = deg[wof[w]]
                dg[w2] = deg[wof[w2]]
                done = True
                break
        if not done:
            break
    # reorder windows: fewest chunks first, heavy windows at high positions
    chunks = np.array([np.ceil(d.sum(0) / 128).sum() for d in dg])
    perm = np.argsort(chunks, kind="stable")
    return [wof[p] for p in perm]


def _prep(node_features, edge_index, edge_type, Wp, bp, Wm, bm, Wih, Whh, bih, bhh):
    """Host-side sharding/packing. Returns (meta, in_maps)."""
    x = np.asarray(node_features, np.float32)
    B = x.shape[0]
    N = B * MAXN
    GPC = B // NCORES             # graphs per core
    NB = GPC * MAXN               # nodes per core
    NWIN = GPC * WPG              # windows per core
    HALF = NB // 2                # nodes per AG half (2 graphs)
    src = np.asarray(edge_index[0]).astype(np.int64)
    dst = np.asarray(edge_index[1]).astype(np.int64)
    et = np.asarray(edge_type).astype(np.int64)

    # per-(node, type) in-degree
    cnt = np.zeros((N, T), np.int64)
    np.add.at(cnt, (dst, et), 1)

    # balance windows within each graph -> node permutation
    old2new = np.empty(N, np.int64)
    for g in range(B):
        mem = _balance_graph(cnt[g * MAXN:(g + 1) * MAXN])
        for w in range(WPG):
            pos = g * MAXN + w * WIN + np.arange(WIN)
            old2new[g * MAXN + mem[w]] = pos
    new2old = np.argsort(old2new)

    src_n = old2new[src]
    dst_n = old2new[dst]

    src_row = src_n                # table row = global node id (single AG)

    # group edges per core: key = ((gslot*WPG + w)*T + t)
    core = dst_n // NB
    rel = dst_n % NB
    col = rel % WIN
    key = (rel // WIN) * T + et
    NGRP = NWIN * T

    gsizes = np.zeros((NCORES, NGRP), np.int64)
    for c in range(NCORES):
        m = core == c
        gsizes[c] = np.bincount(key[m], minlength=NGRP)
    budget = np.ceil(gsizes.max(axis=0) / 128).astype(np.int64)  # chunks/group
    budget = np.maximum(budget, 1)
    ctot = int(budget.sum())
    ngg = (ctot + 7) // 8          # gather groups of <=8 chunks
    rem_last = ctot - 8 * (ngg - 1)
    nslots = ctot * 128
    gbase = np.concatenate([[0], np.cumsum(budget)])[:-1] * 128  # slot base

    # per-core slot arrays
    idx_maps, smat_maps = [], []
    counts_maps, xT_maps = [], []
    for c in range(NCORES):
        m = core == c
        kc, cc, sc = key[m], col[m], src_row[m]
        order = np.argsort(kc, kind="stable")
        kc, cc, sc = kc[order], cc[order], sc[order]
        grp_start = np.searchsorted(kc, np.arange(NGRP), side="left")
        rank = np.arange(kc.size) - grp_start[kc]
        slot = gbase[kc] + rank
        src16 = np.zeros(nslots, np.int16)
        scol = np.full(nslots, -1, np.int64)
        src16[slot] = sc.astype(np.int16)
        scol[slot] = cc
        # idx: wrapped [16, nslots/16] replicated to 128 partitions
        idx = np.tile(src16.reshape(nslots // 16, 16).T, (8, 1)).copy()
        idx_maps.append(idx)
        # one-hot S: [ngg, 128, 8, WIN] bf16 (last group zero-padded)
        smat = np.zeros((ngg * 8 * 128, WIN), BF16)
        valid = scol >= 0
        smat[np.nonzero(valid)[0], scol[valid]] = 1
        smat = smat.reshape(ngg, 8, 128, WIN)
        smat = np.ascontiguousarray(smat.transpose(0, 2, 1, 3))  # [ngg,128,8,WIN]
        smat_maps.append(smat)
        # counts (new order), [T, NB] bf16
        cslice = cnt[new2old[c * NB:(c + 1) * NB]]
        counts_maps.append(np.ascontiguousarray(cslice.T).astype(BF16))
        # xT [128, 2, NB] bf16: [p, k, node] = x[node, k*128+p]
        xs = x.reshape(N, F)[new2old[c * NB:(c + 1) * NB]]
        xp = np.zeros((NB, 2 * 128), np.float32)
        xp[:, :F] = xs
        xT = np.ascontiguousarray(xp.reshape(NB, 2, 128).transpose(2, 1, 0))
        xT_maps.append(xT.astype(BF16))

    # node-major padded x shards: layer 0 aggregates x directly (aggregation
    # is linear in h0 = Wp x + bp, so Wp folds into Wm[0] and bp into bm[0]
    # on the host); the device AllGathers the shards into the layer-0 table
    xtab = np.zeros((N, 2 * 128), np.float32)
    xtab[:, :F] = x.reshape(N, F)[new2old]
    xtab = xtab.astype(BF16)

    # weights (shared across cores)
    Wp = np.asarray(Wp, np.float32); bp_ = np.asarray(bp, np.float32)
    Wm_ = np.asarray(Wm, np.float32); bm_ = np.asarray(bm, np.float32)
    Wih_ = np.asarray(Wih, np.float32); Whh_ = np.asarray(Whh, np.float32)
    bih_ = np.asarray(bih, np.float32); bhh_ = np.asarray(bhh, np.float32)

    wpT = np.zeros((128, 2, H), np.float32)          # [p, fk, h']
    wpt = Wp.T                                       # [F, H]
    wpT[:, 0, :] = wpt[0:128]
    wpT[:F - 128, 1, :] = wpt[128:F]
    wp_in = wpT.astype(BF16)
    bp_in = np.ascontiguousarray(bp_.reshape(2, 128).T)          # [128, 2]

    wm_in = np.ascontiguousarray(                     # [L, 128, 2, T, H]
        Wm_.transpose(0, 1, 3, 2)
        .reshape(L, T, 2, 128, H).transpose(0, 3, 2, 1, 4)).astype(BF16)
    bm_in = bm_.astype(BF16).copy()                   # [L, T, H]
    # layer 0 in x-space: Wt[t] = Wm[0,t] @ Wp (pad in-dim F->256),
    # bm~[0,t] = bm[0,t] + Wm[0,t] @ bp
    wt0 = np.einsum("toh,hf->tfo", Wm_[0], Wp)        # [T, F, H]
    wt0p = np.zeros((T, 256, H), np.float32)
    wt0p[:, :F, :] = wt0
    wm_in[0] = np.ascontiguousarray(
        wt0p.reshape(T, 2, 128, H).transpose(2, 1, 0, 3)).astype(BF16)
    bm_in[0] = (bm_[0] + np.einsum("toh,h->to", Wm_[0], bp_)).astype(BF16)
    wih_in = np.ascontiguousarray(                    # [L, 128, 2, 3H]
        Wih_.transpose(0, 2, 1).reshape(L, 2, 128, 3 * H).transpose(0, 2, 1, 3)
    ).astype(BF16)
    whh_in = np.ascontiguousarray(
        Whh_.transpose(0, 2, 1).reshape(L, 2, 128, 3 * H).transpose(0, 2, 1, 3)
    ).astype(BF16)
    brz = bih_[:, :2 * H] + bhh_[:, :2 * H]
    brz_in = np.ascontiguousarray(brz.reshape(L, 4, 128).transpose(0, 2, 1))
    bin_in = np.ascontiguousarray(bih_[:, 2 * H:].reshape(L, 2, 128).transpose(0, 2, 1))
    bhn_in = np.ascontiguousarray(bhh_[:, 2 * H:].reshape(L, 2, 128).transpose(0, 2, 1))
    id128 = np.eye(128, dtype=BF16)

    in_maps = []
    for c in range(NCORES):
        in_maps.append({
            "xT": xT_maps[c], "xshard": xtab[c * NB:(c + 1) * NB],
            "idx": idx_maps[c],
            "smat": smat_maps[c], "countsT": counts_maps[c],
            "wpT": wp_in, "bp": bp_in, "wmT": wm_in, "bmT": bm_in,
            "wihT": wih_in, "whhT": whh_in,
            "brz": brz_in, "bin_": bin_in, "bhn": bhn_in, "id128": id128,
        })
    meta = dict(B=B, N=N, GPC=GPC, NB=NB, NWIN=NWIN, HALF=HALF,
                budget=budget.reshape(NWIN, T), ctot=ctot, ngg=ngg,
                rem_last=rem_last, new2old=new2old)
    return meta, in_maps


def _build(meta):
    """Build the SPMD Bass program (identical across cores)."""
    dt = mybir.dt
    N, NB, GPC, NWIN = meta["N"], meta["NB"], meta["GPC"], meta["NWIN"]
    HALF = meta["HALF"]
    budget, ngg, ctot = meta["budget"], meta["ngg"], meta["ctot"]
    rem_last = meta["rem_last"]
    SLOT16 = ctot * 8

    nc = bacc.Bacc("TRN2", target_bir_lowering=False, debug=False,
                   enable_asserts=False, num_devices=NCORES,
                   num_swdge_queues=4)

    # ---- I/O
    xT_in = nc.dram_tensor("xT", [128, 2, NB], dt.bfloat16, kind="ExternalInput").ap()
    xshard_in = nc.dram_tensor("xshard", [NB, 2 * 128], dt.bfloat16, kind="ExternalInput").ap()
    idx_in = nc.dram_tensor("idx", [128, SLOT16], dt.int16, kind="ExternalInput").ap()
    smat_in = nc.dram_tensor("smat", [ngg, 128, 8, WIN], dt.bfloat16, kind="ExternalInput").ap()
    counts_in = nc.dram_tensor("countsT", [T, NB], dt.bfloat16, kind="ExternalInput").ap()
    wp_in = nc.dram_tensor("wpT", [128, 2, H], dt.bfloat16, kind="ExternalInput").ap()
    bp_in = nc.dram_tensor("bp", [128, 2], dt.float32, kind="ExternalInput").ap()
    wm_in = nc.dram_tensor("wmT", [L, 128, 2, T, H], dt.bfloat16, kind="ExternalInput").ap()
    bm_in = nc.dram_tensor("bmT", [L, T, H], dt.bfloat16, kind="ExternalInput").ap()
    wih_in = nc.dram_tensor("wihT", [L, 128, 2, 3 * H], dt.bfloat16, kind="ExternalInput").ap()
    whh_in = nc.dram_tensor("whhT", [L, 128, 2, 3 * H], dt.bfloat16, kind="ExternalInput").ap()
    brz_in = nc.dram_tensor("brz", [L, 128, 4], dt.float32, kind="ExternalInput").ap()
    bin_in = nc.dram_tensor("bin_", [L, 128, 2], dt.float32, kind="ExternalInput").ap()
    bhn_in = nc.dram_tensor("bhn", [L, 128, 2], dt.float32, kind="ExternalInput").ap()
    id_in = nc.dram_tensor("id128", [128, 128], dt.bfloat16, kind="ExternalInput").ap()
    out_t = nc.dram_tensor("outT", [2, 128, GPC], dt.float32, kind="ExternalOutput").ap()

    groups = [list(range(NCORES))]

    with tile.TileContext(nc) as tc:
        with (
            tc.tile_pool(name="per", bufs=1) as per,       # persistent SBUF
            tc.tile_pool(name="wts", bufs=2) as wts,       # per-layer weights
            tc.tile_pool(name="gth", bufs=GPREF) as gth,   # gather/S stream
            tc.tile_pool(name="wrk", bufs=2) as wrk,       # A/mT/staging
            tc.tile_pool(name="gru", bufs=2) as grup,      # GRU temps
            tc.tile_pool(name="ps", bufs=1, space="PSUM") as ps,
            tc.tile_pool(name="dram", bufs=2, space="DRAM") as dram,
        ):
            # persistent loads
            idx_sb = per.tile([128, SLOT16], dt.int16)
            nc.sync.dma_start(idx_sb[:], idx_in[:])
            counts_sb = per.tile([T, NB], dt.bfloat16)
            nc.sync.dma_start(counts_sb[:], counts_in[:])
            wp_sb = per.tile([128, 2, H], dt.bfloat16)
            nc.sync.dma_start(wp_sb[:], wp_in[:])
            bp_sb = per.tile([128, 2], dt.float32)
            nc.sync.dma_start(bp_sb[:], bp_in[:])
            id_sb = per.tile([128, 128], dt.bfloat16)
            nc.sync.dma_start(id_sb[:], id_in[:])
            hT_sb = per.tile([128, 2, NB], dt.bfloat16)
            outsb = per.tile([128, 2, GPC], dt.float32)
            nc.vector.memset(outsb[:], 0.0)

            # layer-0 gathers read the (node-major, padded) x table, built by
            # AllGathering each core's x shard (cheaper than a 16MB in-DRAM
            # copy of a replicated input; collectives can't read kernel IO,
            # so bounce the shard through a DRAM tile first)
            agin0 = dram.tile([NB, 2 * 128], dt.bfloat16, tag="agin0", bufs=1,
                              name="agin0")
            nc.sync.dma_start(agin0[:], xshard_in[:])
            xtab_d = dram.tile([N, 2 * 128], dt.bfloat16, tag="xtab", bufs=1,
                               addr_space="Shared", name="xtab_d")
            nc.gpsimd.collective_compute(
                "AllGather", mybir.AluOpType.bypass, replica_groups=groups,
                ins=[agin0.opt()], outs=[xtab_d.opt()])
            # per-layer table + AG staging rings (DRAM), layers 1..L-1
            tbls, agins = [None], [None]
            for l in range(1, L):
                tbls.append(dram.tile([N, H], dt.bfloat16, tag="tbl", bufs=2,
                                      addr_space="Shared", name=f"tbl{l}"))
                agins.append(dram.tile([NB, H], dt.bfloat16, tag="agin",
                                       bufs=2, name=f"agin{l}"))

            def stage_graph(l, q):
                """PE-transpose graph q's h (h-major) to node-major, DMA to
                agin, and fire the half-AllGathers for layer l's table."""
                agin, tbl = agins[l], tbls[l]
                stg = wrk.tile([128, WPG, H], dt.bfloat16, tag="stg", bufs=2)
                for wl in range(WPG):
                    nb = q * MAXN + wl * WIN
                    for hc in range(2):
                        tp = ps.tile([128, 128], dt.bfloat16, tag="agg", bufs=2)
                        nc.tensor.transpose(tp[:], hT_sb[:, hc, nb:nb + WIN],
                                            id_sb[:])
                        nc.scalar.copy(stg[:, wl, hc * 128:(hc + 1) * 128], tp[:])
                nc.sync.dma_start(
                    agin[q * MAXN:(q + 1) * MAXN].rearrange(
                        "(w p) h -> p w h", p=128), stg[:])
                if q == GPC - 1:
                    nc.gpsimd.collective_compute(
                        "AllGather", mybir.AluOpType.bypass,
                        replica_groups=groups,
                        ins=[agin.opt()], outs=[tbl.opt()])

            # ---- input projection: hT = Wp @ xT + bp, stage per graph
            xs_ld = []
            for s in range(NB // 512):
                xs = wrk.tile([128, 2, 512], dt.bfloat16, tag="xs", bufs=3)
                nc.sync.dma_start(xs[:], xT_in[:, :, s * 512:(s + 1) * 512])
                for hm in range(2):
                    pm = ps.tile([128, 512], dt.float32, tag="mT", bufs=2)
                    nc.tensor.matmul(pm[:], wp_sb[:, 0, hm * 128:(hm + 1) * 128],
                                     xs[:, 0, :], start=True, stop=False)
                    nc.tensor.matmul(pm[:], wp_sb[:, 1, hm * 128:(hm + 1) * 128],
                                     xs[:, 1, :], start=False, stop=True)
                    nc.vector.tensor_scalar_add(
                        hT_sb[:, hm, s * 512:(s + 1) * 512],
                        pm[:], bp_sb[:, hm:hm + 1])

            rsums = {}
            for l in range(L):
                tbl = xtab_d if l == 0 else tbls[l]
                # ---- layer weights
                wm_sb = wts.tile([128, 2, T, H], dt.bfloat16, tag="wm")
                nc.sync.dma_start(wm_sb[:], wm_in[l])
                bm_sb = wts.tile([T, H], dt.bfloat16, tag="bm")
                nc.sync.dma_start(bm_sb[:], bm_in[l])
                wih_sb = wts.tile([128, 2, 3 * H], dt.bfloat16, tag="wih")
                nc.sync.dma_start(wih_sb[:], wih_in[l])
                whh_sb = wts.tile([128, 2, 3 * H], dt.bfloat16, tag="whh")
                nc.sync.dma_start(whh_sb[:], whh_in[l])
                brz_sb = wts.tile([128, 4], dt.float32, tag="brz")
                nc.sync.dma_start(brz_sb[:], brz_in[l])
                bin_sb = wts.tile([128, 2], dt.float32, tag="bin")
                nc.sync.dma_start(bin_sb[:], bin_in[l])
                bhn_sb = wts.tile([128, 2], dt.float32, tag="bhn")
                nc.sync.dma_start(bhn_sb[:], bhn_in[l])

                # ---- gather-group streaming
                cglob = 0
                gg_tiles = {}

                def need(c, l=l, tbl=tbl, gg_tiles=gg_tiles):
                    gg = c // 8
                    while len(gg_tiles) == 0 or max(gg_tiles) < gg:
                        g_ = 0 if not gg_tiles else max(gg_tiles) + 1
                        rem = 8 if g_ < ngg - 1 else rem_last
                        Gt = gth.tile([128, rem, H], dt.bfloat16, tag="G",
                                      bufs=GPREF, name=f"G_{l}_{g_}")
                        nc.gpsimd.dma_gather(
                            Gt[:], tbl[:],
                            idx_sb[:, g_ * 64:g_ * 64 + rem * 8],
                            num_idxs=rem * 128, num_idxs_reg=rem * 128,
                            elem_size=H, queue_num=g_ % 4)
                        St = gth.tile([128, 8, WIN], dt.bfloat16, tag="S",
                                      bufs=GPREF, name=f"S_{l}_{g_}")
                        nc.sync.dma_start(St[:], smat_in[g_])
                        gg_tiles[g_] = (Gt, St)
                        if len(gg_tiles) > GPREF:
                            del gg_tiles[min(gg_tiles)]
                    return gg_tiles[gg], c % 8

                # ---- aggregation + message + GRU per half-graph (512 nodes)
                for q in range(GPC):
                    for s2 in range(2):
                        A_sb = wrk.tile([128, T, 2, 4, WIN], dt.bfloat16,
                                        tag="A", bufs=2)
                        for wl in range(4):
                            w = q * WPG + s2 * 4 + wl
                            for th in range(T // 2):
                                pa = ps.tile([128, 512], dt.float32,
                                             tag="agg", bufs=2)
                                for ti in range(2):
                                    t = th * 2 + ti
                                    nchunks = int(budget[w, t])
                                    for hc in range(2):
                                        off = (ti * 2 + hc) * 128
                                        for ci in range(nchunks):
                                            (Gt, St), j = need(cglob + ci)
                                            nc.tensor.matmul(
                                                pa[:, off:off + 128],
                                                Gt[:, j, hc * 128:(hc + 1) * 128],
                                                St[:, j, :],
                                                start=(ci == 0),
                                                stop=(ci == nchunks - 1))
                                    cglob += nchunks
                                dst_ap = A_sb[:, th * 2:th * 2 + 2, :, wl, :]
                                src_ap = pa.rearrange("p (t c k) -> p t c k",
                                                      t=2, c=2)
                                if th % 2 == 0:
                                    nc.scalar.copy(dst_ap, src_ap)
                                else:
                                    nc.vector.tensor_copy(dst_ap, src_ap)

                        # ---- message matmuls for this 512-node slice
                        mT_sb = wrk.tile([128, 2, 512], dt.bfloat16,
                                         tag="mT", bufs=2)
                        nbase = q * MAXN + s2 * 512
                        for hm in range(2):
                            pm = ps.tile([128, 512], dt.float32, tag="mT",
                                         bufs=2)
                            nc.tensor.matmul(
                                pm[:], bm_sb[:, hm * 128:(hm + 1) * 128],
                                counts_sb[:, nbase:nbase + 512],
                                start=True, stop=False)
                            for t in range(T):
                                for hk in range(2):
                                    nc.tensor.matmul(
                                        pm[:],
                                        wm_sb[:, hk, t, hm * 128:(hm + 1) * 128],
                                        A_sb[:, t, hk, :, :],
                                        start=False, stop=(t == T - 1 and hk == 1))
                            nc.vector.tensor_copy(mT_sb[:, hm, :], pm[:])

                        # ---- GRU for these 512 nodes
                        nsl = slice(nbase, nbase + 512)
                        r_sb = grup.tile([128, 2, 512], dt.float32, tag="r",
                                         bufs=2)
                        z_sb = grup.tile([128, 2, 512], dt.float32, tag="z",
                                         bufs=2)
                        for gm in range(4):
                            pg = ps.tile([128, 512], dt.float32, tag="gru",
                                         bufs=3)
                            gsl = slice(gm * 128, (gm + 1) * 128)
                            nc.tensor.matmul(pg[:], wih_sb[:, 0, gsl],
                                             mT_sb[:, 0, :],
                                             start=True, stop=False)
                            nc.tensor.matmul(pg[:], wih_sb[:, 1, gsl],
                                             mT_sb[:, 1, :],
                                             start=False, stop=False)
                            nc.tensor.matmul(pg[:], whh_sb[:, 0, gsl],
                                             hT_sb[:, 0, nsl],
                                             start=False, stop=False)
                            nc.tensor.matmul(pg[:], whh_sb[:, 1, gsl],
                                             hT_sb[:, 1, nsl],
                                             start=False, stop=True)
                            dstg = r_sb[:, gm, :] if gm < 2 else z_sb[:, gm - 2, :]
                            nc.scalar.activation(
                                dstg, pg[:],
                                mybir.ActivationFunctionType.Sigmoid,
                                bias=brz_sb[:, gm:gm + 1])
                        nns, zds = [], []
                        for hc in range(2):
                            gsl = slice((4 + hc) * 128, (5 + hc) * 128)
                            ph = ps.tile([128, 512], dt.float32, tag="gru",
                                         bufs=3)
                            nc.tensor.matmul(ph[:], whh_sb[:, 0, gsl],
                                             hT_sb[:, 0, nsl],
                                             start=True, stop=False)
                            nc.tensor.matmul(ph[:], whh_sb[:, 1, gsl],
                                             hT_sb[:, 1, nsl],
                                             start=False, stop=True)
                            hnb = grup.tile([128, 512], dt.float32, tag="gt",
                                            bufs=3)
                            nc.vector.tensor_scalar_add(hnb[:], ph[:],
                                                        bhn_sb[:, hc:hc + 1])
                            rhn = grup.tile([128, 512], dt.float32, tag="gt",
                                            bufs=3)
                            nc.vector.tensor_mul(rhn[:], r_sb[:, hc, :], hnb[:])
                            pi = ps.tile([128, 512], dt.float32, tag="gru",
                                         bufs=3)
                            nc.tensor.matmul(pi[:], wih_sb[:, 0, gsl],
                                             mT_sb[:, 0, :],
                                             start=True, stop=False)
                            nc.tensor.matmul(pi[:], wih_sb[:, 1, gsl],
                                             mT_sb[:, 1, :],
                                             start=False, stop=True)
                            tsum = grup.tile([128, 512], dt.float32, tag="gt",
                                             bufs=3)
                            nc.vector.tensor_add(tsum[:], pi[:], rhn[:])
                            nn = grup.tile([128, 512], dt.float32, tag="nnb",
                                           bufs=2)
                            nc.scalar.activation(
                                nn[:], tsum[:],
                                mybir.ActivationFunctionType.Tanh,
                                bias=bin_sb[:, hc:hc + 1])
                            hprev = grup.tile([128, 512], dt.float32, tag="gt",
                                              bufs=3)
                            nc.vector.tensor_copy(hprev[:], hT_sb[:, hc, nsl])
                            d_ = grup.tile([128, 512], dt.float32, tag="gt",
                                           bufs=3)
                            nc.vector.tensor_sub(d_[:], hprev[:], nn[:])
                            zd = grup.tile([128, 512], dt.float32, tag="zdb",
                                           bufs=2)
                            nc.vector.tensor_mul(zd[:], z_sb[:, hc, :], d_[:])
                            nns.append(nn)
                            zds.append(zd)
                        for hc in range(2):
                            if l < L - 1:
                                nc.vector.tensor_add(hT_sb[:, hc, nsl],
                                                     nns[hc][:], zds[hc][:])
                            else:
                                hf = grup.tile([128, 512], dt.float32,
                                               tag="hf", bufs=2)
                                nc.vector.tensor_add(hf[:], nns[hc][:], zds[hc][:])
                                rs = grup.tile([128, 1], dt.float32, tag="rs",
                                               bufs=16)
                                nc.vector.tensor_reduce(
                                    rs[:], hf[:], axis=mybir.AxisListType.X,
                                    op=mybir.AluOpType.add)
                                rsums[(q, hc, s2)] = rs
                    # graph q's h fully updated -> stage for next layer's table
                    if l < L - 1:
                        stage_graph(l + 1, q)
                assert cglob == ctot, (cglob, ctot)

            # ---- readout
            for q in range(GPC):
                for hc in range(2):
                    nc.vector.tensor_add(outsb[:, hc, q:q + 1],
                                         rsums[(q, hc, 0)][:],
                                         rsums[(q, hc, 1)][:])
            nc.sync.dma_start(out_t.rearrange("c p g -> p c g"), outsb[:])

    nc.compile()
    return nc


def kernel(**inputs):
    meta, in_maps = _prep(**inputs)
    nc = _build(meta)
    res = run_bass_kernel_spmd(nc, in_maps, core_ids=list(range(NCORES)))
    GPC = meta["GPC"]
    out = np.zeros((meta["B"], H), np.float32)
    for c in range(NCORES):
        ot = res.results[c]["outT"]          # [2, 128, GPC]
        for g in range(GPC):
            out[c * GPC + g] = np.concatenate([ot[0, :, g], ot[1, :, g]])
    return out


# revision 35
# speedup vs baseline: 2.2710x; 1.0441x over previous
"""BatchGGNNEncoder Trainium2 kernel: 8-core SPMD, dst-sharded message passing.

Full inputs in, full output out. Internally:
  - core c owns nodes [c*4096, (c+1)*4096) = graphs [4c, 4c+4) (data parallel).
  - aggregate-first GGNN layer:
        A_t[v] = sum_{e: dst=v, type=t} h[src_e]         (one-hot matmuls, PSUM)
        m      = sum_t A_t @ Wm[t].T + counts_t * bm[t]  (dense matmuls)
        h      = GRU(m, h)                               (matmuls + DVE/ACT)
  - h table (bf16, node-major) lives in DRAM; rebuilt per layer via TWO
    AllGathers (first half fires mid-layer to hide latency); per-edge h[src]
    rows fetched with dma_gather (the Q7 descriptor-emission stream is the
    critical path: ~8.5us per 1024 rows, so everything else hides under it).
  - nodes are permuted within each graph to balance (type, 128-dst-window)
    group sizes so the compiled program structure is identical on all 8 cores.
"""
import numpy as np
import ml_dtypes

import concourse.bass as bass
import concourse.bacc as bacc
import concourse.mybir as mybir
import concourse.tile as tile
from concourse.bass_utils import run_bass_kernel_spmd

BF16 = ml_dtypes.bfloat16

# problem constants (hardcoded per harness contract)
MAXN, F, H, T, L = 1024, 215, 256, 8, 3
NCORES = 8
WIN = 128                     # dst window (one-hot free width)
WPG = MAXN // WIN             # 8 windows per graph
GPREF = 7                     # gather groups in flight
KPRE = 6                      # groups batch-prepped during each AG wait


def _balance_graph(deg):
    """Assign 1024 nodes (deg: [1024, T] type-degrees) to 8 windows of 128.
    Window 7 takes the heaviest 128 nodes (cap 381/type); the rest fill
    windows 0..6 under a 256/type cap (2 chunks). A repair pass swaps nodes
    to clear residual over-cap windows, then windows are reordered so any
    remaining 3-chunk windows sit at the highest positions — aligning the
    cross-core max that sets the budget."""
    tot = deg.sum(1)
    order = np.argsort(-tot, kind="stable")
    wsum = np.zeros((WPG, T), np.float64)
    wcnt = np.zeros(WPG, np.int64)
    members = [[] for _ in range(WPG)]
    CAP, CAP7 = 256.0, 381.0
    rest = []
    for nd in order:
        if wcnt[7] < 128 and ((wsum[7] + deg[nd]) <= CAP7).all():
            members[7].append(nd)
            wsum[7] += deg[nd]
            wcnt[7] += 1
        else:
            rest.append(nd)
    for nd in rest:
        d = deg[nd]
        ns = wsum[:7] + d
        feas = (wcnt[:7] < 128) & (ns <= CAP).all(axis=1)
        if feas.any():
            load = np.where(feas, ns.max(axis=1), np.inf)
            best = int(np.argmin(load))
        else:
            nsall = wsum + d
            dcost = (np.ceil(nsall / 128) - np.ceil(wsum / 128)).sum(axis=1)
            dcost[wcnt >= 128] = np.inf
            best = int(np.argmin(dcost))
        members[best].append(nd)
        wsum[best] += d
        wcnt[best] += 1

    # repair: swap nodes to pull windows 0..6 under 256 per type (w7 < 384)
    wof = [np.array(m, np.int64) for m in members]
    dg = [deg[m] for m in wof]
    cap = np.full(WPG, CAP)
    cap[7] = 383.0
    for _ in range(400):
        ws = np.stack([d.sum(0) for d in dg])
        over = ws - cap[:, None]
        if (over <= 0).all():
            break
        w, t = np.unravel_index(np.argmax(over), over.shape)
        done = False
        for w2 in np.argsort(ws[:, t]):
            if w2 == w:
                continue
            da, db = dg[w], dg[w2]
            gain = da[:, t][:, None] - db[:, t][None, :]   # [na, nb]
            nsw = ws[w][None, None, :] - da[:, None, :] + db[None, :, :]
            nsw2 = ws[w2][None, None, :] + da[:, None, :] - db[None, :, :]
            ok = ((nsw <= np.maximum(ws[w], cap[w])[None, None, :]).all(2)
                  & (nsw2 <= cap[w2][None, None]).all(0 * 0 + 2)
                  & (gain > 0))
            if ok.any():
                a, b = np.unravel_index(
                    np.argmax(np.where(ok, gain, -1)), gain.shape)
                wof[w][a], wof[w2][b] = wof[w2][b], wof[w][a]
                dg[w] = deg[wof[w]]
                dg[w2] = deg[wof[w2]]
                done = True
                break
        if not done:
            break
    # reorder windows: fewest chunks first, heavy windows at high positions
    chunks = np.array([np.ceil(d.sum(0) / 128).sum() for d in dg])
    perm = np.argsort(chunks, kind="stable")
    return [wof[p] for p in perm]


def _prep(node_features, edge_index, edge_type, Wp, bp, Wm, bm, Wih, Whh, bih, bhh):
    """Host-side sharding/packing. Returns (meta, in_maps)."""
    x = np.asarray(node_features, np.float32)
    B = x.shape[0]
    N = B * MAXN
    GPC = B // NCORES             # graphs per core
    NB = GPC * MAXN               # nodes per core
    NWIN = GPC * WPG              # windows per core
    HALF = NB // 2                # nodes per AG half (2 graphs)
    src = np.asarray(edge_index[0]).astype(np.int64)
    dst = np.asarray(edge_index[1]).astype(np.int64)
    et = np.asarray(edge_type).astype(np.int64)

    # per-(node, type) in-degree
    cnt = np.zeros((N, T), np.int64)
    np.add.at(cnt, (dst, et), 1)

    # balance windows within each graph -> node permutation
    old2new = np.empty(N, np.int64)
    for g in range(B):
        mem = _balance_graph(cnt[g * MAXN:(g + 1) * MAXN])
        for w in range(WPG):
            pos = g * MAXN + w * WIN + np.arange(WIN)
            old2new[g * MAXN + mem[w]] = pos
    new2old = np.argsort(old2new)

    src_n = old2new[src]
    dst_n = old2new[dst]

    src_row = src_n                # table row = global node id (single AG)

    # group edges per core: key = ((gslot*WPG + w)*T + t)
    core = dst_n // NB
    rel = dst_n % NB
    col = rel % WIN
    key = (rel // WIN) * T + et
    NGRP = NWIN * T

    gsizes = np.zeros((NCORES, NGRP), np.int64)
    for c in range(NCORES):
        m = core == c
        gsizes[c] = np.bincount(key[m], minlength=NGRP)
    budget = np.ceil(gsizes.max(axis=0) / 128).astype(np.int64)  # chunks/group
    budget = np.maximum(budget, 1)
    ctot = int(budget.sum())
    ngg = (ctot + 7) // 8          # gather groups of <=8 chunks
    rem_last = ctot - 8 * (ngg - 1)
    nslots = ctot * 128
    gbase = np.concatenate([[0], np.cumsum(budget)])[:-1] * 128  # slot base

    # per-core slot arrays
    idx_maps, smat_maps = [], []
    counts_maps, xT_maps = [], []
    for c in range(NCORES):
        m = core == c
        kc, cc, sc = key[m], col[m], src_row[m]
        order = np.argsort(kc, kind="stable")
        kc, cc, sc = kc[order], cc[order], sc[order]
        grp_start = np.searchsorted(kc, np.arange(NGRP), side="left")
        rank = np.arange(kc.size) - grp_start[kc]
        slot = gbase[kc] + rank
        src16 = np.zeros(nslots, np.int16)
        scol = np.full(nslots, -1, np.int64)
        src16[slot] = sc.astype(np.int16)
        scol[slot] = cc
        # idx: wrapped [16, nslots/16] replicated to 128 partitions
        idx = np.tile(src16.reshape(nslots // 16, 16).T, (8, 1)).copy()
        idx_maps.append(idx)
        # one-hot S: [ngg, 128, 8, WIN] bf16 (last group zero-padded)
        smat = np.zeros((ngg * 8 * 128, WIN), BF16)
        valid = scol >= 0
        smat[np.nonzero(valid)[0], scol[valid]] = 1
        smat = smat.reshape(ngg, 8, 128, WIN)
        smat = np.ascontiguousarray(smat.transpose(0, 2, 1, 3))  # [ngg,128,8,WIN]
        smat_maps.append(smat)
        # counts (new order), [T, NB] bf16
        cslice = cnt[new2old[c * NB:(c + 1) * NB]]
        counts_maps.append(np.ascontiguousarray(cslice.T).astype(BF16))
        # xT [128, 2, NB] bf16: [p, k, node] = x[node, k*128+p]
        xs = x.reshape(N, F)[new2old[c * NB:(c + 1) * NB]]
        xp = np.zeros((NB, 2 * 128), np.float32)
        xp[:, :F] = xs
        xT = np.ascontiguousarray(xp.reshape(NB, 2, 128).transpose(2, 1, 0))
        xT_maps.append(xT.astype(BF16))

    # node-major padded x shards: layer 0 aggregates x directly (aggregation
    # is linear in h0 = Wp x + bp, so Wp folds into Wm[0] and bp into bm[0]
    # on the host); the device AllGathers the shards into the layer-0 table
    xtab = np.zeros((N, 2 * 128), np.float32)
    xtab[:, :F] = x.reshape(N, F)[new2old]
    xtab = xtab.astype(BF16)

    # weights (shared across cores)
    Wp = np.asarray(Wp, np.float32); bp_ = np.asarray(bp, np.float32)
    Wm_ = np.asarray(Wm, np.float32); bm_ = np.asarray(bm, np.float32)
    Wih_ = np.asarray(Wih, np.float32); Whh_ = np.asarray(Whh, np.float32)
    bih_ = np.asarray(bih, np.float32); bhh_ = np.asarray(bhh, np.float32)

    wpT = np.zeros((128, 2, H), np.float32)          # [p, fk, h']
    wpt = Wp.T                                       # [F, H]
    wpT[:, 0, :] = wpt[0:128]
    wpT[:F - 128, 1, :] = wpt[128:F]
    wp_in = wpT.astype(BF16)
    bp_in = np.ascontiguousarray(bp_.reshape(2, 128).T)          # [128, 2]

    wm_in = np.ascontiguousarray(                     # [L, 128, 2, T, H]
        Wm_.transpose(0, 1, 3, 2)
        .reshape(L, T, 2, 128, H).transpose(0, 3, 2, 1, 4)).astype(BF16)
    bm_in = bm_.astype(BF16).copy()                   # [L, T, H]
    # layer 0 in x-space: Wt[t] = Wm[0,t] @ Wp (pad in-dim F->256),
    # bm~[0,t] = bm[0,t] + Wm[0,t] @ bp
    wt0 = np.einsum("toh,hf->tfo", Wm_[0], Wp)        # [T, F, H]
    wt0p = np.zeros((T, 256, H), np.float32)
    wt0p[:, :F, :] = wt0
    wm_in[0] = np.ascontiguousarray(
        wt0p.reshape(T, 2, 128, H).transpose(2, 1, 0, 3)).astype(BF16)
    bm_in[0] = (bm_[0] + np.einsum("toh,h->to", Wm_[0], bp_)).astype(BF16)
    wih_in = np.ascontiguousarray(                    # [L, 128, 2, 3H]
        Wih_.transpose(0, 2, 1).reshape(L, 2, 128, 3 * H).transpose(0, 2, 1, 3)
    ).astype(BF16)
    whh_in = np.ascontiguousarray(
        Whh_.transpose(0, 2, 1).reshape(L, 2, 128, 3 * H).transpose(0, 2, 1, 3)
    ).astype(BF16)
    brz = bih_[:, :2 * H] + bhh_[:, :2 * H]
    brz_in = np.ascontiguousarray(brz.reshape(L, 4, 128).transpose(0, 2, 1))
    bin_in = np.ascontiguousarray(bih_[:, 2 * H:].reshape(L, 2, 128).transpose(0, 2, 1))
    bhn_in = np.ascontiguousarray(bhh_[:, 2 * H:].reshape(L, 2, 128).transpose(0, 2, 1))
    id128 = np.eye(128, dtype=BF16)

    in_maps = []
    for c in range(NCORES):
        in_maps.append({
            "xT": xT_maps[c], "xshard": xtab[c * NB:(c + 1) * NB],
            "idx": idx_maps[c],
            "smat": smat_maps[c], "countsT": counts_maps[c],
            "wpT": wp_in, "bp": bp_in, "wmT": wm_in, "bmT": bm_in,
            "wihT": wih_in, "whhT": whh_in,
            "brz": brz_in, "bin_": bin_in, "bhn": bhn_in, "id128": id128,
        })
    meta = dict(B=B, N=N, GPC=GPC, NB=NB, NWIN=NWIN, HALF=HALF,
                budget=budget.reshape(NWIN, T), ctot=ctot, ngg=ngg,
                rem_last=rem_last, new2old=new2old)
    return meta, in_maps


def _build(meta):
    """Build the SPMD Bass program (identical across cores)."""
    dt = mybir.dt
    N, NB, GPC, NWIN = meta["N"], meta["NB"], meta["GPC"], meta["NWIN"]
    HALF = meta["HALF"]
    budget, ngg, ctot = meta["budget"], meta["ngg"], meta["ctot"]
    rem_last = meta["rem_last"]
    SLOT16 = ctot * 8

    nc = bacc.Bacc("TRN2", target_bir_lowering=False, debug=False,
                   enable_asserts=False, num_devices=NCORES,
                   num_swdge_queues=4)

    # ---- I/O
    xT_in = nc.dram_tensor("xT", [128, 2, NB], dt.bfloat16, kind="ExternalInput").ap()
    xshard_in = nc.dram_tensor("xshard", [NB, 2 * 128], dt.bfloat16, kind="ExternalInput").ap()
    idx_in = nc.dram_tensor("idx", [128, SLOT16], dt.int16, kind="ExternalInput").ap()
    smat_in = nc.dram_tensor("smat", [ngg, 128, 8, WIN], dt.bfloat16, kind="ExternalInput").ap()
    counts_in = nc.dram_tensor("countsT", [T, NB], dt.bfloat16, kind="ExternalInput").ap()
    wp_in = nc.dram_tensor("wpT", [128, 2, H], dt.bfloat16, kind="ExternalInput").ap()
    bp_in = nc.dram_tensor("bp", [128, 2], dt.float32, kind="ExternalInput").ap()
    wm_in = nc.dram_tensor("wmT", [L, 128, 2, T, H], dt.bfloat16, kind="ExternalInput").ap()
    bm_in = nc.dram_tensor("bmT", [L, T, H], dt.bfloat16, kind="ExternalInput").ap()
    wih_in = nc.dram_tensor("wihT", [L, 128, 2, 3 * H], dt.bfloat16, kind="ExternalInput").ap()
    whh_in = nc.dram_tensor("whhT", [L, 128, 2, 3 * H], dt.bfloat16, kind="ExternalInput").ap()
    brz_in = nc.dram_tensor("brz", [L, 128, 4], dt.float32, kind="ExternalInput").ap()
    bin_in = nc.dram_tensor("bin_", [L, 128, 2], dt.float32, kind="ExternalInput").ap()
    bhn_in = nc.dram_tensor("bhn", [L, 128, 2], dt.float32, kind="ExternalInput").ap()
    id_in = nc.dram_tensor("id128", [128, 128], dt.bfloat16, kind="ExternalInput").ap()
    out_t = nc.dram_tensor("outT", [2, 128, GPC], dt.float32, kind="ExternalOutput").ap()

    groups = [list(range(NCORES))]

    with tile.TileContext(nc) as tc:
        with (
            tc.tile_pool(name="per", bufs=1) as per,       # persistent SBUF
            tc.tile_pool(name="wts", bufs=2) as wts,       # per-layer weights
            tc.tile_pool(name="gth", bufs=GPREF) as gth,   # gather/S stream
            tc.tile_pool(name="wrk", bufs=2) as wrk,       # A/mT/staging
            tc.tile_pool(name="gru", bufs=2) as grup,      # GRU temps
            tc.tile_pool(name="ps", bufs=1, space="PSUM") as ps,
            tc.tile_pool(name="dram", bufs=2, space="DRAM") as dram,
        ):
            # persistent loads
            idx_sb = per.tile([128, SLOT16], dt.int16)
            nc.sync.dma_start(idx_sb[:], idx_in[:])
            counts_sb = per.tile([T, NB], dt.bfloat16)
            nc.sync.dma_start(counts_sb[:], counts_in[:])
            wp_sb = per.tile([128, 2, H], dt.bfloat16)
            nc.sync.dma_start(wp_sb[:], wp_in[:])
            bp_sb = per.tile([128, 2], dt.float32)
            nc.sync.dma_start(bp_sb[:], bp_in[:])
            id_sb = per.tile([128, 128], dt.bfloat16)
            nc.sync.dma_start(id_sb[:], id_in[:])
            hT_sb = per.tile([128, 2, NB], dt.bfloat16)
            outsb = per.tile([128, 2, GPC], dt.float32)
            nc.vector.memset(outsb[:], 0.0)

            # layer-0 gathers read the (node-major, padded) x table, built by
            # AllGathering each core's x shard (cheaper than a 16MB in-DRAM
            # copy of a replicated input; collectives can't read kernel IO,
            # so bounce the shard through a DRAM tile first)
            agin0 = dram.tile([NB, 2 * 128], dt.bfloat16, tag="agin0", bufs=1,
                              name="agin0")
            nc.sync.dma_start(agin0[:], xshard_in[:])
            xtab_d = dram.tile([N, 2 * 128], dt.bfloat16, tag="xtab", bufs=1,
                               addr_space="Shared", name="xtab_d")
            nc.gpsimd.collective_compute(
                "AllGather", mybir.AluOpType.bypass, replica_groups=groups,
                ins=[agin0.opt()], outs=[xtab_d.opt()])
            # per-layer table + AG staging rings (DRAM), layers 1..L-1
            tbls, agins = [None], [None]
            for l in range(1, L):
                tbls.append(dram.tile([N, H], dt.bfloat16, tag="tbl", bufs=2,
                                      addr_space="Shared", name=f"tbl{l}"))
                agins.append(dram.tile([NB, H], dt.bfloat16, tag="agin",
                                       bufs=2, name=f"agin{l}"))

            def stage_graph(l, q):
                """PE-transpose graph q's h (h-major) to node-major, DMA to
                agin, and fire the half-AllGathers for layer l's table."""
                agin, tbl = agins[l], tbls[l]
                stg = wrk.tile([128, WPG, H], dt.bfloat16, tag="stg", bufs=2)
                for wl in range(WPG):
                    nb = q * MAXN + wl * WIN
                    for hc in range(2):
                        tp = ps.tile([128, 128], dt.bfloat16, tag="agg", bufs=2)
                        nc.tensor.transpose(tp[:], hT_sb[:, hc, nb:nb + WIN],
                                            id_sb[:])
                        nc.scalar.copy(stg[:, wl, hc * 128:(hc + 1) * 128], tp[:])
                nc.sync.dma_start(
                    agin[q * MAXN:(q + 1) * MAXN].rearrange(
                        "(w p) h -> p w h", p=128), stg[:])
                if q == GPC - 1:
                    nc.gpsimd.collective_compute(
                        "AllGather", mybir.AluOpType.bypass,
                        replica_groups=groups,
                        ins=[agin.opt()], outs=[tbl.opt()])

            # ---- input projection: hT = Wp @ xT + bp, stage per graph
            xs_ld = []
            for s in range(NB // 512):
                xs = wrk.tile([128, 2, 512], dt.bfloat16, tag="xs", bufs=3)
                nc.sync.dma_start(xs[:], xT_in[:, :, s * 512:(s + 1) * 512])
                for hm in range(2):
                    pm = ps.tile([128, 512], dt.float32, tag="mT", bufs=2)
                    nc.tensor.matmul(pm[:], wp_sb[:, 0, hm * 128:(hm + 1) * 128],
                                     xs[:, 0, :], start=True, stop=False)
                    nc.tensor.matmul(pm[:], wp_sb[:, 1, hm * 128:(hm + 1) * 128],
                                     xs[:, 1, :], start=False, stop=True)
                    nc.vector.tensor_scalar_add(
                        hT_sb[:, hm, s * 512:(s + 1) * 512],
                        pm[:], bp_sb[:, hm:hm + 1])

            rsums = {}
            for l in range(L):
                tbl = xtab_d if l == 0 else tbls[l]
                # ---- layer weights
                wm_sb = wts.tile([128, 2, T, H], dt.bfloat16, tag="wm")
                nc.sync.dma_start(wm_sb[:], wm_in[l])
                bm_sb = wts.tile([T, H], dt.bfloat16, tag="bm")
                nc.sync.dma_start(bm_sb[:], bm_in[l])
                wih_sb = wts.tile([128, 2, 3 * H], dt.bfloat16, tag="wih")
                nc.sync.dma_start(wih_sb[:], wih_in[l])
                whh_sb = wts.tile([128, 2, 3 * H], dt.bfloat16, tag="whh")
                nc.sync.dma_start(whh_sb[:], whh_in[l])
                brz_sb = wts.tile([128, 4], dt.float32, tag="brz")
                nc.sync.dma_start(brz_sb[:], brz_in[l])
                bin_sb = wts.tile([128, 2], dt.float32, tag="bin")
                nc.sync.dma_start(bin_sb[:], bin_in[l])
                bhn_sb = wts.tile([128, 2], dt.float32, tag="bhn")
                nc.sync.dma_start(bhn_sb[:], bhn_in[l])

                # ---- gather-group streaming
                cglob = 0
                gg_tiles = {}

                def need(c, l=l, tbl=tbl, gg_tiles=gg_tiles):
                    gg = c // 8
                    while len(gg_tiles) == 0 or max(gg_tiles) < gg:
                        g_ = 0 if not gg_tiles else max(gg_tiles) + 1
                        rem = 8 if g_ < ngg - 1 else rem_last
                        Gt = gth.tile([128, rem, H], dt.bfloat16, tag="G",
                                      bufs=GPREF, name=f"G_{l}_{g_}")
                        nc.gpsimd.dma_gather(
                            Gt[:], tbl[:],
                            idx_sb[:, g_ * 64:g_ * 64 + rem * 8],
                            num_idxs=rem * 128, num_idxs_reg=rem * 128,
                            elem_size=H, queue_num=g_ % 4)
                        St = gth.tile([128, 8, WIN], dt.bfloat16, tag="S",
                                      bufs=GPREF, name=f"S_{l}_{g_}")
                        nc.sync.dma_start(St[:], smat_in[g_])
                        gg_tiles[g_] = (Gt, St)
                        if len(gg_tiles) > GPREF:
                            del gg_tiles[min(gg_tiles)]
                    return gg_tiles[gg], c % 8

                # ---- aggregation + message + GRU per half-graph (512 nodes)
                for q in range(GPC):
                    for s2 in range(2):
                        A_sb = wrk.tile([128, T, 2, 4, WIN], dt.bfloat16,
                                        tag="A", bufs=2)
                        for wl in range(4):
                            w = q * WPG + s2 * 4 + wl
                            for th in range(T // 2):
                                pa = ps.tile([128, 512], dt.float32,
                                             tag="agg", bufs=2)
                                for ti in range(2):
                                    t = th * 2 + ti
                                    nchunks = int(budget[w, t])
                                    for hc in range(2):
                                        off = (ti * 2 + hc) * 128
                                        for ci in range(nchunks):
                                            (Gt, St), j = need(cglob + ci)
                                            nc.tensor.matmul(
                                                pa[:, off:off + 128],
                                                Gt[:, j, hc * 128:(hc + 1) * 128],
                                                St[:, j, :],
                                                start=(ci == 0),
                                                stop=(ci == nchunks - 1))
                                    cglob += nchunks
                                dst_ap = A_sb[:, th * 2:th * 2 + 2, :, wl, :]
                                src_ap = pa.rearrange("p (t c k) -> p t c k",
                                                      t=2, c=2)
                                if th % 2 == 0:
                                    nc.scalar.copy(dst_ap, src_ap)
                                else:
                                    nc.vector.tensor_copy(dst_ap, src_ap)

                        # ---- message matmuls for this 512-node slice
                        mT_sb = wrk.tile([128, 2, 512], dt.bfloat16,
                                         tag="mT", bufs=2)
                        nbase = q * MAXN + s2 * 512
                        for hm in range(2):
                            pm = ps.tile([128, 512], dt.float32, tag="mT",
                                         bufs=2)
                            nc.tensor.matmul(
                                pm[:], bm_sb[:, hm * 128:(hm + 1) * 128],
                                counts_sb[:, nbase:nbase + 512],
                                start=True, stop=False)
                            for t in range(T):
                                for hk in range(2):
                                    nc.tensor.matmul(
                                        pm[:],
                                        wm_sb[:, hk, t, hm * 128:(hm + 1) * 128],
                                        A_sb[:, t, hk, :, :],
                                        start=False, stop=(t == T - 1 and hk == 1))
                            nc.vector.tensor_copy(mT_sb[:, hm, :], pm[:])

                        # ---- GRU for these 512 nodes
                        nsl = slice(nbase, nbase + 512)
                        r_sb = grup.tile([128, 2, 512], dt.float32, tag="r",
                                         bufs=2)
                        z_sb = grup.tile([128, 2, 512], dt.float32, tag="z",
                                         bufs=2)
                        for gm in range(4):
                            pg = ps.tile([128, 512], dt.float32, tag="gru",
                                         bufs=3)
                            gsl = slice(gm * 128, (gm + 1) * 128)
                            nc.tensor.matmul(pg[:], wih_sb[:, 0, gsl],
                                             mT_sb[:, 0, :],
                                             start=True, stop=False)
                            nc.tensor.matmul(pg[:], wih_sb[:, 1, gsl],
                                             mT_sb[:, 1, :],
                                             start=False, stop=False)
                            nc.tensor.matmul(pg[:], whh_sb[:, 0, gsl],
                                             hT_sb[:, 0, nsl],
                                             start=False, stop=False)
                            nc.tensor.matmul(pg[:], whh_sb[:, 1, gsl],
                                             hT_sb[:, 1, nsl],
                                             start=False, stop=True)
                            dstg = r_sb[:, gm, :] if gm < 2 else z_sb[:, gm - 2, :]
                            nc.scalar.activation(
                                dstg, pg[:],
                                mybir.ActivationFunctionType.Sigmoid,
                                bias=brz_sb[:, gm:gm + 1])
                        nns, zds = [], []
                        for hc in range(2):
                            gsl = slice((4 + hc) * 128, (5 + hc) * 128)
                            ph = ps.tile([128, 512], dt.float32, tag="gru",
                                         bufs=3)
                            nc.tensor.matmul(ph[:], whh_sb[:, 0, gsl],
                                             hT_sb[:, 0, nsl],
                                             start=True, stop=False)
                            nc.tensor.matmul(ph[:], whh_sb[:, 1, gsl],
                                             hT_sb[:, 1, nsl],
                                             start=False, stop=True)
                            hnb = grup.tile([128, 512], dt.float32, tag="gt",
                                            bufs=3)
                            nc.vector.tensor_scalar_add(hnb[:], ph[:],
                                                        bhn_sb[:, hc:hc + 1])
                            rhn = grup.tile([128, 512], dt.float32, tag="gt",
                                            bufs=3)
                            nc.vector.tensor_mul(rhn[:], r_sb[:, hc, :], hnb[:])
                            pi = ps.tile([128, 512], dt.float32, tag="gru",
                                         bufs=3)
                            nc.tensor.matmul(pi[:], wih_sb[:, 0, gsl],
                                             mT_sb[:, 0, :],
                                             start=True, stop=False)
                            nc.tensor.matmul(pi[:], wih_sb[:, 1, gsl],
                                             mT_sb[:, 1, :],
                                             start=False, stop=True)
                            tsum = grup.tile([128, 512], dt.float32, tag="gt",
                                             bufs=3)
                            nc.vector.tensor_add(tsum[:], pi[:], rhn[:])
                            nn = grup.tile([128, 512], dt.float32, tag="nnb",
                                           bufs=2)
                            nc.scalar.activation(
                                nn[:], tsum[:],
                                mybir.ActivationFunctionType.Tanh,
                                bias=bin_sb[:, hc:hc + 1])
                            hprev = grup.tile([128, 512], dt.float32, tag="gt",
                                              bufs=3)
                            nc.vector.tensor_copy(hprev[:], hT_sb[:, hc, nsl])
                            d_ = grup.tile([128, 512], dt.float32, tag="gt",
                                           bufs=3)
                            nc.vector.tensor_sub(d_[:], hprev[:], nn[:])
                            zd = grup.tile([128, 512], dt.float32, tag="zdb",
                                           bufs=2)
                            nc.vector.tensor_mul(zd[:], z_sb[:, hc, :], d_[:])
                            nns.append(nn)
                            zds.append(zd)
                        for hc in range(2):
                            if l < L - 1:
                                nc.vector.tensor_add(hT_sb[:, hc, nsl],
                                                     nns[hc][:], zds[hc][:])
                            else:
                                hf = grup.tile([128, 512], dt.float32,
                                               tag="hf", bufs=2)
                                nc.vector.tensor_add(hf[:], nns[hc][:], zds[hc][:])
                                rs = grup.tile([128, 1], dt.float32, tag="rs",
                                               bufs=16)
                                nc.vector.tensor_reduce(
                                    rs[:], hf[:], axis=mybir.AxisListType.X,
                                    op=mybir.AluOpType.add)
                                rsums[(q, hc, s2)] = rs
                    # graph q's h fully updated -> stage for next layer's table
                    if l < L - 1:
                        stage_graph(l + 1, q)
                assert cglob == ctot, (cglob, ctot)

            # ---- readout
            for q in range(GPC):
                for hc in range(2):
                    nc.vector.tensor_add(outsb[:, hc, q:q + 1],
                                         rsums[(q, hc, 0)][:],
                                         rsums[(q, hc, 1)][:])
            nc.sync.dma_start(out_t.rearrange("c p g -> p c g"), outsb[:])

    nc.compile()
    return nc


def kernel(**inputs):
    meta, in_maps = _prep(**inputs)
    nc = _build(meta)
    res = run_bass_kernel_spmd(nc, in_maps, core_ids=list(range(NCORES)))
    GPC = meta["GPC"]
    out = np.zeros((meta["B"], H), np.float32)
    for c in range(NCORES):
        ot = res.results[c]["outT"]          # [2, 128, GPC]
        for g in range(GPC):
            out[c * GPC + g] = np.concatenate([ot[0, :, g], ot[1, :, g]])
    return out


# revision 36
# speedup vs baseline: 2.2751x; 1.0018x over previous
"""BatchGGNNEncoder Trainium2 kernel: 8-core SPMD, dst-sharded message passing.

Full inputs in, full output out. Internally:
  - core c owns nodes [c*4096, (c+1)*4096) = graphs [4c, 4c+4) (data parallel).
  - aggregate-first GGNN layer:
        A_t[v] = sum_{e: dst=v, type=t} h[src_e]         (one-hot matmuls, PSUM)
        m      = sum_t A_t @ Wm[t].T + counts_t * bm[t]  (dense matmuls)
        h      = GRU(m, h)                               (matmuls + DVE/ACT)
  - h table (bf16, node-major) lives in DRAM; rebuilt per layer via TWO
    AllGathers (first half fires mid-layer to hide latency); per-edge h[src]
    rows fetched with dma_gather (the Q7 descriptor-emission stream is the
    critical path: ~8.5us per 1024 rows, so everything else hides under it).
  - nodes are permuted within each graph to balance (type, 128-dst-window)
    group sizes so the compiled program structure is identical on all 8 cores.
"""
import numpy as np
import ml_dtypes

import concourse.bass as bass
import concourse.bacc as bacc
import concourse.mybir as mybir
import concourse.tile as tile
from concourse.bass_utils import run_bass_kernel_spmd

BF16 = ml_dtypes.bfloat16

# problem constants (hardcoded per harness contract)
MAXN, F, H, T, L = 1024, 215, 256, 8, 3
NCORES = 8
WIN = 128                     # dst window (one-hot free width)
WPG = MAXN // WIN             # 8 windows per graph
GPREF = 8                     # gather groups in flight
KPRE = 6                      # groups batch-prepped during each AG wait


def _balance_graph(deg):
    """Assign 1024 nodes (deg: [1024, T] type-degrees) to 8 windows of 128.
    Window 7 takes the heaviest 128 nodes (cap 381/type); the rest fill
    windows 0..6 under a 256/type cap (2 chunks). A repair pass swaps nodes
    to clear residual over-cap windows, then windows are reordered so any
    remaining 3-chunk windows sit at the highest positions — aligning the
    cross-core max that sets the budget."""
    tot = deg.sum(1)
    order = np.argsort(-tot, kind="stable")
    wsum = np.zeros((WPG, T), np.float64)
    wcnt = np.zeros(WPG, np.int64)
    members = [[] for _ in range(WPG)]
    CAP, CAP7 = 256.0, 381.0
    rest = []
    for nd in order:
        if wcnt[7] < 128 and ((wsum[7] + deg[nd]) <= CAP7).all():
            members[7].append(nd)
            wsum[7] += deg[nd]
            wcnt[7] += 1
        else:
            rest.append(nd)
    for nd in rest:
        d = deg[nd]
        ns = wsum[:7] + d
        feas = (wcnt[:7] < 128) & (ns <= CAP).all(axis=1)
        if feas.any():
            load = np.where(feas, ns.max(axis=1), np.inf)
            best = int(np.argmin(load))
        else:
            nsall = wsum + d
            dcost = (np.ceil(nsall / 128) - np.ceil(wsum / 128)).sum(axis=1)
            dcost[wcnt >= 128] = np.inf
            best = int(np.argmin(dcost))
        members[best].append(nd)
        wsum[best] += d
        wcnt[best] += 1

    # repair: swap nodes to pull windows 0..6 under 256 per type (w7 < 384)
    wof = [np.array(m, np.int64) for m in members]
    dg = [deg[m] for m in wof]
    cap = np.full(WPG, CAP)
    cap[7] = 383.0
    for _ in range(400):
        ws = np.stack([d.sum(0) for d in dg])
        over = ws - cap[:, None]
        if (over <= 0).all():
            break
        w, t = np.unravel_index(np.argmax(over), over.shape)
        done = False
        for w2 in np.argsort(ws[:, t]):
            if w2 == w:
                continue
            da, db = dg[w], dg[w2]
            gain = da[:, t][:, None] - db[:, t][None, :]   # [na, nb]
            nsw = ws[w][None, None, :] - da[:, None, :] + db[None, :, :]
            nsw2 = ws[w2][None, None, :] + da[:, None, :] - db[None, :, :]
            ok = ((nsw <= np.maximum(ws[w], cap[w])[None, None, :]).all(2)
                  & (nsw2 <= cap[w2][None, None]).all(0 * 0 + 2)
                  & (gain > 0))
            if ok.any():
                a, b = np.unravel_index(
                    np.argmax(np.where(ok, gain, -1)), gain.shape)
                wof[w][a], wof[w2][b] = wof[w2][b], wof[w][a]
                dg[w] = deg[wof[w]]
                dg[w2] = deg[wof[w2]]
                done = True
                break
        if not done:
            break
    # reorder windows: fewest chunks first, heavy windows at high positions
    chunks = np.array([np.ceil(d.sum(0) / 128).sum() for d in dg])
    perm = np.argsort(chunks, kind="stable")
    return [wof[p] for p in perm]


def _prep(node_features, edge_index, edge_type, Wp, bp, Wm, bm, Wih, Whh, bih, bhh):
    """Host-side sharding/packing. Returns (meta, in_maps)."""
    x = np.asarray(node_features, np.float32)
    B = x.shape[0]
    N = B * MAXN
    GPC = B // NCORES             # graphs per core
    NB = GPC * MAXN               # nodes per core
    NWIN = GPC * WPG              # windows per core
    HALF = NB // 2                # nodes per AG half (2 graphs)
    src = np.asarray(edge_index[0]).astype(np.int64)
    dst = np.asarray(edge_index[1]).astype(np.int64)
    et = np.asarray(edge_type).astype(np.int64)

    # per-(node, type) in-degree
    cnt = np.zeros((N, T), np.int64)
    np.add.at(cnt, (dst, et), 1)

    # balance windows within each graph -> node permutation
    old2new = np.empty(N, np.int64)
    for g in range(B):
        mem = _balance_graph(cnt[g * MAXN:(g + 1) * MAXN])
        for w in range(WPG):
            pos = g * MAXN + w * WIN + np.arange(WIN)
            old2new[g * MAXN + mem[w]] = pos
    new2old = np.argsort(old2new)

    src_n = old2new[src]
    dst_n = old2new[dst]

    src_row = src_n                # table row = global node id (single AG)

    # group edges per core: key = ((gslot*WPG + w)*T + t)
    core = dst_n // NB
    rel = dst_n % NB
    col = rel % WIN
    key = (rel // WIN) * T + et
    NGRP = NWIN * T

    gsizes = np.zeros((NCORES, NGRP), np.int64)
    for c in range(NCORES):
        m = core == c
        gsizes[c] = np.bincount(key[m], minlength=NGRP)
    budget = np.ceil(gsizes.max(axis=0) / 128).astype(np.int64)  # chunks/group
    budget = np.maximum(budget, 1)
    ctot = int(budget.sum())
    ngg = (ctot + 7) // 8          # gather groups of <=8 chunks
    rem_last = ctot - 8 * (ngg - 1)
    nslots = ctot * 128
    gbase = np.concatenate([[0], np.cumsum(budget)])[:-1] * 128  # slot base

    # per-core slot arrays
    idx_maps, smat_maps = [], []
    counts_maps, xT_maps = [], []
    for c in range(NCORES):
        m = core == c
        kc, cc, sc = key[m], col[m], src_row[m]
        order = np.argsort(kc, kind="stable")
        kc, cc, sc = kc[order], cc[order], sc[order]
        grp_start = np.searchsorted(kc, np.arange(NGRP), side="left")
        rank = np.arange(kc.size) - grp_start[kc]
        slot = gbase[kc] + rank
        src16 = np.zeros(nslots, np.int16)
        scol = np.full(nslots, -1, np.int64)
        src16[slot] = sc.astype(np.int16)
        scol[slot] = cc
        # idx: wrapped [16, nslots/16] replicated to 128 partitions
        idx = np.tile(src16.reshape(nslots // 16, 16).T, (8, 1)).copy()
        idx_maps.append(idx)
        # one-hot S: [ngg, 128, 8, WIN] bf16 (last group zero-padded)
        smat = np.zeros((ngg * 8 * 128, WIN), BF16)
        valid = scol >= 0
        smat[np.nonzero(valid)[0], scol[valid]] = 1
        smat = smat.reshape(ngg, 8, 128, WIN)
        smat = np.ascontiguousarray(smat.transpose(0, 2, 1, 3))  # [ngg,128,8,WIN]
        smat_maps.append(smat)
        # counts (new order), [T, NB] bf16
        cslice = cnt[new2old[c * NB:(c + 1) * NB]]
        counts_maps.append(np.ascontiguousarray(cslice.T).astype(BF16))
        # xT [128, 2, NB] bf16: [p, k, node] = x[node, k*128+p]
        xs = x.reshape(N, F)[new2old[c * NB:(c + 1) * NB]]
        xp = np.zeros((NB, 2 * 128), np.float32)
        xp[:, :F] = xs
        xT = np.ascontiguousarray(xp.reshape(NB, 2, 128).transpose(2, 1, 0))
        xT_maps.append(xT.astype(BF16))

    # node-major padded x shards: layer 0 aggregates x directly (aggregation
    # is linear in h0 = Wp x + bp, so Wp folds into Wm[0] and bp into bm[0]
    # on the host); the device AllGathers the shards into the layer-0 table
    xtab = np.zeros((N, 2 * 128), np.float32)
    xtab[:, :F] = x.reshape(N, F)[new2old]
    xtab = xtab.astype(BF16)

    # weights (shared across cores)
    Wp = np.asarray(Wp, np.float32); bp_ = np.asarray(bp, np.float32)
    Wm_ = np.asarray(Wm, np.float32); bm_ = np.asarray(bm, np.float32)
    Wih_ = np.asarray(Wih, np.float32); Whh_ = np.asarray(Whh, np.float32)
    bih_ = np.asarray(bih, np.float32); bhh_ = np.asarray(bhh, np.float32)

    wpT = np.zeros((128, 2, H), np.float32)          # [p, fk, h']
    wpt = Wp.T                                       # [F, H]
    wpT[:, 0, :] = wpt[0:128]
    wpT[:F - 128, 1, :] = wpt[128:F]
    wp_in = wpT.astype(BF16)
    bp_in = np.ascontiguousarray(bp_.reshape(2, 128).T)          # [128, 2]

    wm_in = np.ascontiguousarray(                     # [L, 128, 2, T, H]
        Wm_.transpose(0, 1, 3, 2)
        .reshape(L, T, 2, 128, H).transpose(0, 3, 2, 1, 4)).astype(BF16)
    bm_in = bm_.astype(BF16).copy()                   # [L, T, H]
    # layer 0 in x-space: Wt[t] = Wm[0,t] @ Wp (pad in-dim F->256),
    # bm~[0,t] = bm[0,t] + Wm[0,t] @ bp
    wt0 = np.einsum("toh,hf->tfo", Wm_[0], Wp)        # [T, F, H]
    wt0p = np.zeros((T, 256, H), np.float32)
    wt0p[:, :F, :] = wt0
    wm_in[0] = np.ascontiguousarray(
        wt0p.reshape(T, 2, 128, H).transpose(2, 1, 0, 3)).astype(BF16)
    bm_in[0] = (bm_[0] + np.einsum("toh,h->to", Wm_[0], bp_)).astype(BF16)
    wih_in = np.ascontiguousarray(                    # [L, 128, 2, 3H]
        Wih_.transpose(0, 2, 1).reshape(L, 2, 128, 3 * H).transpose(0, 2, 1, 3)
    ).astype(BF16)
    whh_in = np.ascontiguousarray(
        Whh_.transpose(0, 2, 1).reshape(L, 2, 128, 3 * H).transpose(0, 2, 1, 3)
    ).astype(BF16)
    brz = bih_[:, :2 * H] + bhh_[:, :2 * H]
    brz_in = np.ascontiguousarray(brz.reshape(L, 4, 128).transpose(0, 2, 1))
    bin_in = np.ascontiguousarray(bih_[:, 2 * H:].reshape(L, 2, 128).transpose(0, 2, 1))
    bhn_in = np.ascontiguousarray(bhh_[:, 2 * H:].reshape(L, 2, 128).transpose(0, 2, 1))
    id128 = np.eye(128, dtype=BF16)

    in_maps = []
    for c in range(NCORES):
        in_maps.append({
            "xT": xT_maps[c], "xshard": xtab[c * NB:(c + 1) * NB],
            "idx": idx_maps[c],
            "smat": smat_maps[c], "countsT": counts_maps[c],
            "wpT": wp_in, "bp": bp_in, "wmT": wm_in, "bmT": bm_in,
            "wihT": wih_in, "whhT": whh_in,
            "brz": brz_in, "bin_": bin_in, "bhn": bhn_in, "id128": id128,
        })
    meta = dict(B=B, N=N, GPC=GPC, NB=NB, NWIN=NWIN, HALF=HALF,
                budget=budget.reshape(NWIN, T), ctot=ctot, ngg=ngg,
                rem_last=rem_last, new2old=new2old)
    return meta, in_maps


def _build(meta):
    """Build the SPMD Bass program (identical across cores)."""
    dt = mybir.dt
    N, NB, GPC, NWIN = meta["N"], meta["NB"], meta["GPC"], meta["NWIN"]
    HALF = meta["HALF"]
    budget, ngg, ctot = meta["budget"], meta["ngg"], meta["ctot"]
    rem_last = meta["rem_last"]
    SLOT16 = ctot * 8

    nc = bacc.Bacc("TRN2", target_bir_lowering=False, debug=False,
                   enable_asserts=False, num_devices=NCORES,
                   num_swdge_queues=4)

    # ---- I/O
    xT_in = nc.dram_tensor("xT", [128, 2, NB], dt.bfloat16, kind="ExternalInput").ap()
    xshard_in = nc.dram_tensor("xshard", [NB, 2 * 128], dt.bfloat16, kind="ExternalInput").ap()
    idx_in = nc.dram_tensor("idx", [128, SLOT16], dt.int16, kind="ExternalInput").ap()
    smat_in = nc.dram_tensor("smat", [ngg, 128, 8, WIN], dt.bfloat16, kind="ExternalInput").ap()
    counts_in = nc.dram_tensor("countsT", [T, NB], dt.bfloat16, kind="ExternalInput").ap()
    wp_in = nc.dram_tensor("wpT", [128, 2, H], dt.bfloat16, kind="ExternalInput").ap()
    bp_in = nc.dram_tensor("bp", [128, 2], dt.float32, kind="ExternalInput").ap()
    wm_in = nc.dram_tensor("wmT", [L, 128, 2, T, H], dt.bfloat16, kind="ExternalInput").ap()
    bm_in = nc.dram_tensor("bmT", [L, T, H], dt.bfloat16, kind="ExternalInput").ap()
    wih_in = nc.dram_tensor("wihT", [L, 128, 2, 3 * H], dt.bfloat16, kind="ExternalInput").ap()
    whh_in = nc.dram_tensor("whhT", [L, 128, 2, 3 * H], dt.bfloat16, kind="ExternalInput").ap()
    brz_in = nc.dram_tensor("brz", [L, 128, 4], dt.float32, kind="ExternalInput").ap()
    bin_in = nc.dram_tensor("bin_", [L, 128, 2], dt.float32, kind="ExternalInput").ap()
    bhn_in = nc.dram_tensor("bhn", [L, 128, 2], dt.float32, kind="ExternalInput").ap()
    id_in = nc.dram_tensor("id128", [128, 128], dt.bfloat16, kind="ExternalInput").ap()
    out_t = nc.dram_tensor("outT", [2, 128, GPC], dt.float32, kind="ExternalOutput").ap()

    groups = [list(range(NCORES))]

    with tile.TileContext(nc) as tc:
        with (
            tc.tile_pool(name="per", bufs=1) as per,       # persistent SBUF
            tc.tile_pool(name="wts", bufs=2) as wts,       # per-layer weights
            tc.tile_pool(name="gth", bufs=GPREF) as gth,   # gather/S stream
            tc.tile_pool(name="wrk", bufs=2) as wrk,       # A/mT/staging
            tc.tile_pool(name="gru", bufs=2) as grup,      # GRU temps
            tc.tile_pool(name="ps", bufs=1, space="PSUM") as ps,
            tc.tile_pool(name="dram", bufs=2, space="DRAM") as dram,
        ):
            # persistent loads
            idx_sb = per.tile([128, SLOT16], dt.int16)
            nc.sync.dma_start(idx_sb[:], idx_in[:])
            counts_sb = per.tile([T, NB], dt.bfloat16)
            nc.sync.dma_start(counts_sb[:], counts_in[:])
            wp_sb = per.tile([128, 2, H], dt.bfloat16)
            nc.sync.dma_start(wp_sb[:], wp_in[:])
            bp_sb = per.tile([128, 2], dt.float32)
            nc.sync.dma_start(bp_sb[:], bp_in[:])
            id_sb = per.tile([128, 128], dt.bfloat16)
            nc.sync.dma_start(id_sb[:], id_in[:])
            hT_sb = per.tile([128, 2, NB], dt.bfloat16)
            outsb = per.tile([128, 2, GPC], dt.float32)
            nc.vector.memset(outsb[:], 0.0)

            # layer-0 gathers read the (node-major, padded) x table, built by
            # AllGathering each core's x shard (cheaper than a 16MB in-DRAM
            # copy of a replicated input; collectives can't read kernel IO,
            # so bounce the shard through a DRAM tile first)
            agin0 = dram.tile([NB, 2 * 128], dt.bfloat16, tag="agin0", bufs=1,
                              name="agin0")
            nc.sync.dma_start(agin0[:], xshard_in[:])
            xtab_d = dram.tile([N, 2 * 128], dt.bfloat16, tag="xtab", bufs=1,
                               addr_space="Shared", name="xtab_d")
            nc.gpsimd.collective_compute(
                "AllGather", mybir.AluOpType.bypass, replica_groups=groups,
                ins=[agin0.opt()], outs=[xtab_d.opt()])
            # per-layer table + AG staging rings (DRAM), layers 1..L-1
            tbls, agins = [None], [None]
            for l in range(1, L):
                tbls.append(dram.tile([N, H], dt.bfloat16, tag="tbl", bufs=2,
                                      addr_space="Shared", name=f"tbl{l}"))
                agins.append(dram.tile([NB, H], dt.bfloat16, tag="agin",
                                       bufs=2, name=f"agin{l}"))

            def stage_graph(l, q):
                """PE-transpose graph q's h (h-major) to node-major, DMA to
                agin, and fire the half-AllGathers for layer l's table."""
                agin, tbl = agins[l], tbls[l]
                stg = wrk.tile([128, WPG, H], dt.bfloat16, tag="stg", bufs=2)
                for wl in range(WPG):
                    nb = q * MAXN + wl * WIN
                    for hc in range(2):
                        tp = ps.tile([128, 128], dt.bfloat16, tag="agg", bufs=2)
                        nc.tensor.transpose(tp[:], hT_sb[:, hc, nb:nb + WIN],
                                            id_sb[:])
                        nc.scalar.copy(stg[:, wl, hc * 128:(hc + 1) * 128], tp[:])
                nc.sync.dma_start(
                    agin[q * MAXN:(q + 1) * MAXN].rearrange(
                        "(w p) h -> p w h", p=128), stg[:])
                if q == GPC - 1:
                    nc.gpsimd.collective_compute(
                        "AllGather", mybir.AluOpType.bypass,
                        replica_groups=groups,
                        ins=[agin.opt()], outs=[tbl.opt()])

            # ---- input projection: hT = Wp @ xT + bp, stage per graph
            xs_ld = []
            for s in range(NB // 512):
                xs = wrk.tile([128, 2, 512], dt.bfloat16, tag="xs", bufs=3)
                nc.sync.dma_start(xs[:], xT_in[:, :, s * 512:(s + 1) * 512])
                for hm in range(2):
                    pm = ps.tile([128, 512], dt.float32, tag="mT", bufs=2)
                    nc.tensor.matmul(pm[:], wp_sb[:, 0, hm * 128:(hm + 1) * 128],
                                     xs[:, 0, :], start=True, stop=False)
                    nc.tensor.matmul(pm[:], wp_sb[:, 1, hm * 128:(hm + 1) * 128],
                                     xs[:, 1, :], start=False, stop=True)
                    nc.vector.tensor_scalar_add(
                        hT_sb[:, hm, s * 512:(s + 1) * 512],
                        pm[:], bp_sb[:, hm:hm + 1])

            rsums = {}
            for l in range(L):
                tbl = xtab_d if l == 0 else tbls[l]
                # ---- layer weights
                wm_sb = wts.tile([128, 2, T, H], dt.bfloat16, tag="wm")
                nc.sync.dma_start(wm_sb[:], wm_in[l])
                bm_sb = wts.tile([T, H], dt.bfloat16, tag="bm")
                nc.sync.dma_start(bm_sb[:], bm_in[l])
                wih_sb = wts.tile([128, 2, 3 * H], dt.bfloat16, tag="wih")
                nc.sync.dma_start(wih_sb[:], wih_in[l])
                whh_sb = wts.tile([128, 2, 3 * H], dt.bfloat16, tag="whh")
                nc.sync.dma_start(whh_sb[:], whh_in[l])
                brz_sb = wts.tile([128, 4], dt.float32, tag="brz")
                nc.sync.dma_start(brz_sb[:], brz_in[l])
                bin_sb = wts.tile([128, 2], dt.float32, tag="bin")
                nc.sync.dma_start(bin_sb[:], bin_in[l])
                bhn_sb = wts.tile([128, 2], dt.float32, tag="bhn")
                nc.sync.dma_start(bhn_sb[:], bhn_in[l])

                # ---- gather-group streaming
                cglob = 0
                gg_tiles = {}

                def need(c, l=l, tbl=tbl, gg_tiles=gg_tiles):
                    gg = c // 8
                    while len(gg_tiles) == 0 or max(gg_tiles) < gg:
                        g_ = 0 if not gg_tiles else max(gg_tiles) + 1
                        rem = 8 if g_ < ngg - 1 else rem_last
                        Gt = gth.tile([128, rem, H], dt.bfloat16, tag="G",
                                      bufs=GPREF, name=f"G_{l}_{g_}")
                        nc.gpsimd.dma_gather(
                            Gt[:], tbl[:],
                            idx_sb[:, g_ * 64:g_ * 64 + rem * 8],
                            num_idxs=rem * 128, num_idxs_reg=rem * 128,
                            elem_size=H, queue_num=g_ % 4)
                        St = gth.tile([128, 8, WIN], dt.bfloat16, tag="S",
                                      bufs=GPREF, name=f"S_{l}_{g_}")
                        nc.sync.dma_start(St[:], smat_in[g_])
                        gg_tiles[g_] = (Gt, St)
                        if len(gg_tiles) > GPREF:
                            del gg_tiles[min(gg_tiles)]
                    return gg_tiles[gg], c % 8

                # ---- aggregation + message + GRU per half-graph (512 nodes)
                for q in range(GPC):
                    for s2 in range(2):
                        A_sb = wrk.tile([128, T, 2, 4, WIN], dt.bfloat16,
                                        tag="A", bufs=2)
                        for wl in range(4):
                            w = q * WPG + s2 * 4 + wl
                            for th in range(T // 2):
                                pa = ps.tile([128, 512], dt.float32,
                                             tag="agg", bufs=2)
                                for ti in range(2):
                                    t = th * 2 + ti
                                    nchunks = int(budget[w, t])
                                    for hc in range(2):
                                        off = (ti * 2 + hc) * 128
                                        for ci in range(nchunks):
                                            (Gt, St), j = need(cglob + ci)
                                            nc.tensor.matmul(
                                                pa[:, off:off + 128],
                                                Gt[:, j, hc * 128:(hc + 1) * 128],
                                                St[:, j, :],
                                                start=(ci == 0),
                                                stop=(ci == nchunks - 1))
                                    cglob += nchunks
                                dst_ap = A_sb[:, th * 2:th * 2 + 2, :, wl, :]
                                src_ap = pa.rearrange("p (t c k) -> p t c k",
                                                      t=2, c=2)
                                if th % 2 == 0:
                                    nc.scalar.copy(dst_ap, src_ap)
                                else:
                                    nc.vector.tensor_copy(dst_ap, src_ap)

                        # ---- message matmuls for this 512-node slice
                        mT_sb = wrk.tile([128, 2, 512], dt.bfloat16,
                                         tag="mT", bufs=2)
                        nbase = q * MAXN + s2 * 512
                        for hm in range(2):
                            pm = ps.tile([128, 512], dt.float32, tag="mT",
                                         bufs=2)
                            nc.tensor.matmul(
                                pm[:], bm_sb[:, hm * 128:(hm + 1) * 128],
                                counts_sb[:, nbase:nbase + 512],
                                start=True, stop=False)
                            for t in range(T):
                                for hk in range(2):
                                    nc.tensor.matmul(
                                        pm[:],
                                        wm_sb[:, hk, t, hm * 128:(hm + 1) * 128],
                                        A_sb[:, t, hk, :, :],
                                        start=False, stop=(t == T - 1 and hk == 1))
                            nc.vector.tensor_copy(mT_sb[:, hm, :], pm[:])

                        # ---- GRU for these 512 nodes
                        nsl = slice(nbase, nbase + 512)
                        r_sb = grup.tile([128, 2, 512], dt.float32, tag="r",
                                         bufs=2)
                        z_sb = grup.tile([128, 2, 512], dt.float32, tag="z",
                                         bufs=2)
                        for gm in range(4):
                            pg = ps.tile([128, 512], dt.float32, tag="gru",
                                         bufs=3)
                            gsl = slice(gm * 128, (gm + 1) * 128)
                            nc.tensor.matmul(pg[:], wih_sb[:, 0, gsl],
                                             mT_sb[:, 0, :],
                                             start=True, stop=False)
                            nc.tensor.matmul(pg[:], wih_sb[:, 1, gsl],
                                             mT_sb[:, 1, :],
                                             start=False, stop=False)
                            nc.tensor.matmul(pg[:], whh_sb[:, 0, gsl],
                                             hT_sb[:, 0, nsl],
                                             start=False, stop=False)
                            nc.tensor.matmul(pg[:], whh_sb[:, 1, gsl],
                                             hT_sb[:, 1, nsl],
                                             start=False, stop=True)
                            dstg = r_sb[:, gm, :] if gm < 2 else z_sb[:, gm - 2, :]
                            nc.scalar.activation(
                                dstg, pg[:],
                                mybir.ActivationFunctionType.Sigmoid,
                                bias=brz_sb[:, gm:gm + 1])
                        nns, zds = [], []
                        for hc in range(2):
                            gsl = slice((4 + hc) * 128, (5 + hc) * 128)
                            ph = ps.tile([128, 512], dt.float32, tag="gru",
                                         bufs=3)
                            nc.tensor.matmul(ph[:], whh_sb[:, 0, gsl],
                                             hT_sb[:, 0, nsl],
                                             start=True, stop=False)
                            nc.tensor.matmul(ph[:], whh_sb[:, 1, gsl],
                                             hT_sb[:, 1, nsl],
                                             start=False, stop=True)
                            hnb = grup.tile([128, 512], dt.float32, tag="gt",
                                            bufs=3)
                            nc.vector.tensor_scalar_add(hnb[:], ph[:],
                                                        bhn_sb[:, hc:hc + 1])
                            rhn = grup.tile([128, 512], dt.float32, tag="gt",
                                            bufs=3)
                            nc.vector.tensor_mul(rhn[:], r_sb[:, hc, :], hnb[:])
                            pi = ps.tile([128, 512], dt.float32, tag="gru",
                                         bufs=3)
                            nc.tensor.matmul(pi[:], wih_sb[:, 0, gsl],
                                             mT_sb[:, 0, :],
                                             start=True, stop=False)
                            nc.tensor.matmul(pi[:], wih_sb[:, 1, gsl],
                                             mT_sb[:, 1, :],
                                             start=False, stop=True)
                            tsum = grup.tile([128, 512], dt.float32, tag="gt",
                                             bufs=3)
                            nc.vector.tensor_add(tsum[:], pi[:], rhn[:])
                            nn = grup.tile([128, 512], dt.float32, tag="nnb",
                                           bufs=2)
                            nc.scalar.activation(
                                nn[:], tsum[:],
                                mybir.ActivationFunctionType.Tanh,
                                bias=bin_sb[:, hc:hc + 1])
                            hprev = grup.tile([128, 512], dt.float32, tag="gt",
                                              bufs=3)
                            nc.vector.tensor_copy(hprev[:], hT_sb[:, hc, nsl])
                            d_ = grup.tile([128, 512], dt.float32, tag="gt",
                                           bufs=3)
                            nc.vector.tensor_sub(d_[:], hprev[:], nn[:])
                            zd = grup.tile([128, 512], dt.float32, tag="zdb",
                                           bufs=2)
                            nc.vector.tensor_mul(zd[:], z_sb[:, hc, :], d_[:])
                            nns.append(nn)
                            zds.append(zd)
                        for hc in range(2):
                            if l < L - 1:
                                nc.vector.tensor_add(hT_sb[:, hc, nsl],
                                                     nns[hc][:], zds[hc][:])
                            else:
                                hf = grup.tile([128, 512], dt.float32,
                                               tag="hf", bufs=2)
                                nc.vector.tensor_add(hf[:], nns[hc][:], zds[hc][:])
                                rs = grup.tile([128, 1], dt.float32, tag="rs",
                                               bufs=16)
                                nc.vector.tensor_reduce(
                                    rs[:], hf[:], axis=mybir.AxisListType.X,
                                    op=mybir.AluOpType.add)
                                rsums[(q, hc, s2)] = rs
                    # graph q's h fully updated -> stage for next layer's table
                    if l < L - 1:
                        stage_graph(l + 1, q)
                assert cglob == ctot, (cglob, ctot)

            # ---- readout
            for q in range(GPC):
                for hc in range(2):
                    nc.vector.tensor_add(outsb[:, hc, q:q + 1],
                                         rsums[(q, hc, 0)][:],
                                         rsums[(q, hc, 1)][:])
            nc.sync.dma_start(out_t.rearrange("c p g -> p c g"), outsb[:])

    nc.compile()
    return nc


def kernel(**inputs):
    meta, in_maps = _prep(**inputs)
    nc = _build(meta)
    res = run_bass_kernel_spmd(nc, in_maps, core_ids=list(range(NCORES)))
    GPC = meta["GPC"]
    out = np.zeros((meta["B"], H), np.float32)
    for c in range(NCORES):
        ot = res.results[c]["outT"]          # [2, 128, GPC]
        for g in range(GPC):
            out[c * GPC + g] = np.concatenate([ot[0, :, g], ot[1, :, g]])
    return out
